# revision 1
# baseline (speedup 1.0000x reference)
"""Trainium2 Bass kernel for MinimalResonanceLayer (8-core SPMD).

Sharding: core c handles batch b = c//4 and local heads [ (c%4)*4, (c%4)*4+4 ).
Each head's resonance recurrence runs fully on-core (E^T resident in SBUF,
bf16); the head-concat + FFN uses one 8-core AllToAll, with per-core
divergence encoded in an input mask so the program stays SPMD-uniform.

State is kept in rotated coordinates z~ = K*exp(-i*alpha)*z so the
Kuramoto-Sakaguchi rotation folds into the PSUM copy-out scalars.
"""
import math
import numpy as np

import concourse.bass as bass
import concourse.tile as tile
from concourse import bacc, mybir
from concourse.masks import make_identity

# ---- problem constants (hardcoded per contest contract) ----
B, S_FULL, D, H, HD = 2, 2048, 1024, 16, 64
DFF = 2 * D
MU, ALPHA, K_COUP, DT, STEPS, MIX = 1.0, 0.1, 3.0, 0.02, 5, 0.3
N_CORES = 8
NHL = 4  # heads per core

CA, SA = math.cos(ALPHA), math.sin(ALPHA)
C1V = K_COUP * CA            # c1
C2V = K_COUP * SA            # c2
CC1 = MU - K_COUP            # -2.0
R21 = C2V / C1V              # tan(alpha)
W1S = C1V - C2V * C2V / C1V  # pass-1 roa scale
W2S = -2.0 * C2V             # pass-1 rob scale
M2 = (1.0 - MIX) * CA / K_COUP
M3 = (1.0 - MIX) * SA / K_COUP
SCL = 1.0 / math.sqrt(HD)
INVK = 1.0 / K_COUP
GC0 = math.sqrt(2.0 / math.pi)

F32 = mybir.dt.float32
F32R = mybir.dt.float32r
BF16 = mybir.dt.bfloat16
ALU = mybir.AluOpType
AF = mybir.ActivationFunctionType


def r(ap):
    """bitcast fp32 AP -> float32r for full-rate PE streaming."""
    return ap.bitcast(F32R)


def build_nc(S=S_FULL, fake_cc=False):
    """Build the 8-core SPMD program. S = sequence length (per batch).

    fake_cc=True replaces the AllToAll with a local DMA so the program is
    collective-free (for TimelineSim cost modeling only).
    """
    nc = bacc.Bacc("TRN2", target_bir_lowering=False, debug=False,
                   num_devices=N_CORES)

    def din(name, shape):
        return nc.dram_tensor(name, shape, F32, kind="ExternalInput").ap()

    TOK = S // 4
    io = dict(
        x_full=din("x_full", [S, D]),
        x_heads=din("x_heads", [S, NHL * HD]),
        x_tok=din("x_tok", [TOK, D]),
        wq_d=din("wq", [NHL * HD, HD]),
        wk_d=din("wk", [NHL * HD, HD]),
        wv_d=din("wv", [NHL * HD, HD]),
        wo_d=din("wo", [NHL * HD, HD]),
        om_d=din("omega", [NHL, HD]),
        g1_d=din("g1h", [NHL * HD]),
        be1_d=din("be1h", [NHL * HD]),
        g2_d=din("g2", [D]),
        be2_d=din("be2", [D]),
        w1_d=din("w1", [D, DFF]),
        bf1_d=din("bf1", [DFF]),
        w2_d=din("w2", [DFF, D]),
        bf2_d=din("bf2", [D]),
        gm_d=din("gmask", [N_CORES]),
        out_d=nc.dram_tensor("out", [TOK, D], F32, kind="ExternalOutput").ap(),
    )

    with tile.TileContext(nc) as tc:
        _body(nc, tc, io, S, fake_cc)

    nc.compile()
    return nc


def _body(nc, tc, io, S, fake_cc=False):
    NMB = S // 128          # token blocks of 128
    NG = NMB // 4           # groups of 4 blocks (512 tokens)
    TOK = S // 4            # FFN tokens per core (= B*S/8)
    TT4 = TOK // 128        # FFN token tiles
    NSL = S // 512          # 512-wide column slices of S
    HC = NHL * HD           # 256 head cols per core

    x_full, x_heads, x_tok = io["x_full"], io["x_heads"], io["x_tok"]
    wq_d, wk_d, wv_d, wo_d = io["wq_d"], io["wk_d"], io["wv_d"], io["wo_d"]
    om_d, g1_d, be1_d = io["om_d"], io["g1_d"], io["be1_d"]
    g2_d, be2_d = io["g2_d"], io["be2_d"]
    w1_d, bf1_d, w2_d, bf2_d = io["w1_d"], io["bf1_d"], io["w2_d"], io["bf2_d"]
    gm_d, out_d = io["gm_d"], io["out_d"]

    from contextlib import ExitStack
    ctx = ExitStack()
    sing = ctx.enter_context(tc.tile_pool(name="sing", bufs=1))
    dram = ctx.enter_context(tc.tile_pool(name="dram", bufs=1, space="DRAM"))

    # ---- whole-kernel constants ----
    ident = sing.tile([128, 128], F32)
    make_identity(nc, ident)
    identb = sing.tile([128, 128], BF16)
    nc.vector.tensor_copy(out=identb, in_=ident)
    epsT = sing.tile([128, 1], F32)
    nc.vector.memset(epsT, 1e-5)
    maskbc = sing.tile([128, N_CORES], F32)
    nc.sync.dma_start(out=maskbc, in_=gm_d[None, :].to_broadcast([128, N_CORES]))
    xattn = sing.tile([128, NMB, HC], F32)

    cc_in = dram.tile([N_CORES * TOK, HC], F32)
    cc_out = dram.tile([N_CORES * TOK, HC], F32)

    # =================== attention super-phase ===================
    with ExitStack() as actx:
        big = actx.enter_context(tc.tile_pool(name="big", bufs=1))
        onesc1 = big.tile([128, HD], BF16)
        nc.vector.memset(onesc1, C1V)
        g1bc = big.tile([128, HC], F32)
        nc.sync.dma_start(out=g1bc, in_=g1_d[None, :].to_broadcast([128, HC]))
        be1bc = big.tile([128, HC], F32)
        nc.sync.dma_start(out=be1bc, in_=be1_d[None, :].to_broadcast([128, HC]))
        wstage = big.tile([64, 4, NHL, HD], F32)
        for wi, wd in enumerate([wq_d, wk_d, wv_d, wo_d]):
            nc.sync.dma_start(out=wstage[:, wi, :, :],
                              in_=wd.rearrange("(h p) e -> p h e", p=HD))
        wq_sb = big.tile([64, NHL, HD], F32R)
        nc.vector.tensor_copy(out=wq_sb, in_=wstage[:, 0, :, :])
        wk_sb = big.tile([64, NHL, HD], F32R)
        nc.vector.tensor_copy(out=wk_sb, in_=wstage[:, 1, :, :])
        wv_sb = big.tile([64, NHL, HD], F32R)
        nc.vector.tensor_copy(out=wv_sb, in_=wstage[:, 2, :, :])
        wo_sb = big.tile([64, NHL, HD], F32R)
        nc.vector.tensor_copy(out=wo_sb, in_=wstage[:, 3, :, :])

        xnh = big.tile([128, NMB, HC], BF16)
        ET = big.tile([128, NMB, S], BF16)
        state = big.tile([128, NMB, 128], F32)
        statebf = big.tile([128, NMB, 128], BF16)
        pred = big.tile([128, NMB, 128], BF16)
        d1t = big.tile([128, NMB, 128], BF16)
        d2t = big.tile([128, NMB, 128], BF16)
        vb1 = big.tile([128, NMB, 128], BF16)
        v_wide = big.tile([128, NMB, HD], F32)
        attnv = big.tile([128, NMB, HD], F32)
        rz = big.tile([128, NMB, HD], BF16)
        om_st = big.tile([128, HD], F32)
        om_bc = big.tile([128, NMB, HD], BF16)
        qT = big.tile([64, S], BF16)
        kT = big.tile([64, S], BF16)

        # ---------------- LN1 ----------------
        with tc.tile_pool(name="ln", bufs=4) as ln, \
             tc.tile_pool(name="lns", bufs=6) as lns:
            for t in range(NMB):
                xt = ln.tile([128, D], F32, tag="xt")
                nc.sync.dma_start(out=xt, in_=x_full[t * 128:(t + 1) * 128, :])
                xh = ln.tile([128, HC], F32, tag="xh")
                nc.sync.dma_start(out=xh, in_=x_heads[t * 128:(t + 1) * 128, :])
                st = lns.tile([128, 2, 6], F32, tag="st")
                for sg in range(2):
                    nc.vector.bn_stats(out=st[:, sg, :],
                                       in_=xt[:, sg * 512:(sg + 1) * 512])
                mv = lns.tile([128, 2], F32, tag="mv")
                nc.vector.bn_aggr(out=mv, in_=st)
                rstd = lns.tile([128, 1], F32, tag="rstd")
                nc.scalar.activation(out=rstd, in_=mv[:, 1:2], func=AF.Sqrt,
                                     bias=epsT, scale=1.0)
                nc.vector.reciprocal(out=rstd, in_=rstd)
                nb = lns.tile([128, 1], F32, tag="nb")
                nc.vector.tensor_scalar(out=nb, in0=mv[:, 0:1], scalar1=rstd,
                                        scalar2=-1.0, op0=ALU.mult, op1=ALU.mult)
                xs = lns.tile([128, HC], F32, tag="xs")
                nc.scalar.activation(out=xs, in_=xh, func=AF.Identity,
                                     scale=rstd, bias=nb)
                nc.vector.tensor_mul(out=xs, in0=xs, in1=g1bc)
                nc.gpsimd.tensor_add(out=xnh[:, t, :], in0=xs, in1=be1bc)

        # ---------------- per-head resonance ----------------
        with tc.tile_pool(name="xhTp", bufs=1) as xhTp, \
             tc.tile_pool(name="pmisc", bufs=2, space="PSUM") as pmisc, \
             tc.tile_pool(name="pssc", bufs=2, space="PSUM") as psscp, \
             tc.tile_pool(name="psg", bufs=4, space="PSUM") as psgp, \
             tc.tile_pool(name="scr", bufs=2) as scr, \
             tc.tile_pool(name="mts", bufs=3) as mts:

            for h in range(NHL):
                src = bass.AP(tensor=om_d.tensor, offset=om_d.offset + h * HD,
                              ap=[[0, 128], [1, HD]])
                nc.sync.dma_start(out=om_st, in_=src)
                bcin = bass.AP(tensor=om_st.tensor, offset=om_st.offset,
                               ap=[om_st.ap[0], [0, NMB], om_st.ap[1]])
                nc.vector.tensor_copy(out=om_bc, in_=bcin)

                # --- xhT for this head: [64, S] ---
                xhT = xhTp.tile([64, S], F32R, tag="xhT")
                for t in range(NMB):
                    pt = pmisc.tile([64, 128], BF16, tag="pm")
                    nc.tensor.transpose(pt, xnh[:, t, h * HD:(h + 1) * HD], identb)
                    nc.scalar.copy(out=xhT[:, t * 128:(t + 1) * 128], in_=pt)

                # --- projections qT, kT ---
                for sl in range(NSL):
                    pq = pmisc.tile([64, 512], F32, tag="pm")
                    nc.tensor.matmul(pq, wq_sb[:, h, :],
                                     xhT[:, sl * 512:(sl + 1) * 512],
                                     start=True, stop=True)
                    nc.scalar.copy(out=qT[:, sl * 512:(sl + 1) * 512], in_=pq)
                    pk = pmisc.tile([64, 512], F32, tag="pm")
                    nc.tensor.matmul(pk, wk_sb[:, h, :],
                                     xhT[:, sl * 512:(sl + 1) * 512],
                                     start=True, stop=True)
                    nc.scalar.copy(out=kT[:, sl * 512:(sl + 1) * 512], in_=pk)

                # --- v in [s, d] layout ---
                for t in range(NMB):
                    pv = pmisc.tile([128, HD], F32, tag="pm")
                    nc.tensor.matmul(pv, xhT[:, t * 128:(t + 1) * 128],
                                     wv_sb[:, h, :], start=True, stop=True)
                    nc.scalar.copy(out=v_wide[:, t, :], in_=pv)

                # --- state0 = K e^{-ia} v ; pass-1 rhs [v | ones] ---
                nc.scalar.activation(out=state[:, :, 0:HD], in_=v_wide,
                                     func=AF.Copy, scale=C1V)
                nc.scalar.activation(out=state[:, :, HD:128], in_=v_wide,
                                     func=AF.Copy, scale=-C2V)
                nc.vector.tensor_copy(out=vb1[:, :, 0:HD], in_=v_wide)
                nc.vector.memset(vb1[:, :, HD:128], 1.0)
                nc.gpsimd.tensor_copy(out=statebf, in_=state)

                # --- scores^T -> exp -> E^T (bf16) ---
                for k in range(NMB):
                    for sl in range(NSL):
                        c0 = sl * 512
                        ps = psscp.tile([128, 512], F32, tag="ps")
                        nc.tensor.matmul(ps, kT[:, k * 128:(k + 1) * 128],
                                         qT[:, c0:c0 + 512],
                                         start=True, stop=True)
                        nc.scalar.activation(out=ET[:, k, c0:c0 + 512],
                                             in_=ps, func=AF.Exp, scale=SCL)

                # --- 10 Heun passes ---
                for p in range(1, 2 * STEPS + 1):
                    odd = (p % 2 == 1)
                    rhs = vb1 if p == 1 else (statebf if odd else pred)
                    dout = d1t if odd else d2t
                    xin = statebf if odd else pred

                    for g in range(NG):
                        if p == 1:
                            # per-block psum accumulators: each in its own bank
                            # so the k-walks can overlap the exp production.
                            mls = [psgp.tile([128, 128], F32, tag="pg",
                                             name=f"pg1_{h}_{g}_{ml}")
                                   for ml in range(4)]
                            for ml in range(4):
                                mb = g * 4 + ml
                                for k in range(NMB):
                                    nc.tensor.matmul(mls[ml],
                                                     ET[:, k, mb * 128:(mb + 1) * 128],
                                                     rhs[:, k, :],
                                                     start=(k == 0),
                                                     stop=(k == NMB - 1))
                            pg = scr.tile([128, 4, 128], F32, tag="cpg1")
                            for ml in range(4):
                                nc.vector.tensor_copy(out=pg[:, ml, :], in_=mls[ml])
                        else:
                            pg = psgp.tile([128, 4, 128], F32, tag="pg")
                            for ml in range(4):
                                mb = g * 4 + ml
                                for k in range(NMB):
                                    nc.tensor.matmul(pg[:, ml, :],
                                                     ET[:, k, mb * 128:(mb + 1) * 128],
                                                     rhs[:, k, :],
                                                     start=(k == 0), stop=(k == NMB - 1))
                        gs = slice(g * 4, g * 4 + 4)
                        pA = pg[:, :, 0:HD]
                        pB = pg[:, :, HD:128]
                        roa = scr.tile([128, 4, HD], BF16, tag="roa")
                        rob = scr.tile([128, 4, HD], BF16, tag="rob")
                        if p == 1:
                            # psum = [w | Zrep]; build rz = c1/Z, attnv = c1*w/Z
                            for ml in range(4):
                                rcp = scr.tile([128, 1], F32, tag="rcp")
                                nc.vector.reciprocal(out=rcp,
                                                     in_=pg[:, ml, HD:HD + 1])
                                nc.vector.tensor_scalar_mul(
                                    out=rz[:, g * 4 + ml, :],
                                    in0=onesc1, scalar1=rcp)
                            nc.vector.tensor_mul(out=attnv[:, gs, :], in0=pA,
                                                 in1=rz[:, gs, :])
                            nc.scalar.activation(out=roa, in_=pA, func=AF.Copy,
                                                 scale=W1S)
                            nc.scalar.activation(out=rob, in_=pA, func=AF.Copy,
                                                 scale=W2S)
                        else:
                            cpA = scr.tile([128, 4, HD], BF16, tag="cpA")
                            nc.scalar.copy(out=cpA, in_=pA)
                            nc.vector.scalar_tensor_tensor(
                                out=roa, in0=pB, scalar=R21, in1=cpA,
                                op0=ALU.mult, op1=ALU.add)
                            nc.vector.scalar_tensor_tensor(
                                out=rob, in0=cpA, scalar=-R21, in1=pB,
                                op0=ALU.mult, op1=ALU.add)
                        nc.vector.tensor_mul(out=roa, in0=roa, in1=rz[:, gs, :])
                        nc.vector.tensor_mul(out=rob, in0=rob, in1=rz[:, gs, :])

                        XA = xin[:, gs, 0:HD]
                        XB = xin[:, gs, HD:128]
                        aa = scr.tile([128, 4, HD], BF16, tag="aa")
                        bb = scr.tile([128, 4, HD], BF16, tag="bb")
                        nc.scalar.activation(out=aa, in_=XA, func=AF.Square,
                                             scale=INVK)
                        nc.scalar.activation(out=bb, in_=XB, func=AF.Square,
                                             scale=INVK)
                        # aa <- -(aa+bb) = -r2
                        nc.vector.scalar_tensor_tensor(out=aa, in0=aa, scalar=-1.0,
                                                       in1=bb, op0=ALU.mult,
                                                       op1=ALU.subtract)
                        uA = scr.tile([128, 4, HD], BF16, tag="uA")
                        uB = scr.tile([128, 4, HD], BF16, tag="uB")
                        nc.vector.scalar_tensor_tensor(out=uA, in0=aa, scalar=CC1,
                                                       in1=XA, op0=ALU.add,
                                                       op1=ALU.mult)
                        nc.vector.scalar_tensor_tensor(out=uB, in0=aa, scalar=CC1,
                                                       in1=XB, op0=ALU.add,
                                                       op1=ALU.mult)
                        omv = om_bc[:, gs, :]
                        omA = scr.tile([128, 4, HD], BF16, tag="omA")
                        omB = scr.tile([128, 4, HD], BF16, tag="omB")
                        nc.vector.tensor_mul(out=omB, in0=omv, in1=XB)
                        nc.gpsimd.tensor_mul(out=omA, in0=omv, in1=XA)
                        # uA <- uA - omB ; uB <- uB + omA
                        nc.vector.tensor_sub(out=uA, in0=uA, in1=omB)
                        nc.vector.tensor_add(out=uB, in0=uB, in1=omA)
                        nc.vector.tensor_add(out=dout[:, gs, 0:HD], in0=uA, in1=roa)
                        nc.gpsimd.tensor_add(out=dout[:, gs, HD:128], in0=uB,
                                             in1=rob)

                        if odd:
                            nc.vector.scalar_tensor_tensor(
                                out=pred[:, gs, :], in0=dout[:, gs, :], scalar=DT,
                                in1=state[:, gs, :], op0=ALU.mult, op1=ALU.add)
                        else:
                            nc.vector.scalar_tensor_tensor(
                                out=state[:, gs, :], in0=d1t[:, gs, :],
                                scalar=DT / 2, in1=state[:, gs, :],
                                op0=ALU.mult, op1=ALU.add)
                            nc.vector.scalar_tensor_tensor(
                                out=state[:, gs, :], in0=d2t[:, gs, :],
                                scalar=DT / 2, in1=state[:, gs, :],
                                op0=ALU.mult, op1=ALU.add)
                            if p < 2 * STEPS:
                                nc.gpsimd.tensor_copy(out=statebf[:, gs, 0:HD],
                                                      in_=state[:, gs, 0:HD])
                                nc.scalar.copy(out=statebf[:, gs, HD:128],
                                               in_=state[:, gs, HD:128])

                # --- readout: mixed -> @Wo -> xattn cols ---
                nc.scalar.activation(out=attnv, in_=attnv, func=AF.Copy,
                                     scale=MIX / C1V)
                nc.vector.scalar_tensor_tensor(out=attnv, in0=state[:, :, 0:HD],
                                               scalar=M2, in1=attnv,
                                               op0=ALU.mult, op1=ALU.add)
                nc.vector.scalar_tensor_tensor(out=attnv, in0=state[:, :, HD:128],
                                               scalar=-M3, in1=attnv,
                                               op0=ALU.mult, op1=ALU.add)
                for t in range(NMB):
                    pt = pmisc.tile([64, 128], F32, tag="pm")
                    nc.tensor.transpose(pt, attnv[:, t, :], ident)
                    mt = mts.tile([64, 128], F32R, tag="mt")
                    nc.scalar.copy(out=mt, in_=pt)
                    po = pmisc.tile([128, HD], F32, tag="pm")
                    nc.tensor.matmul(po, mt, wo_sb[:, h, :],
                                     start=True, stop=True)
                    nc.scalar.copy(out=xattn[:, t, h * HD:(h + 1) * HD], in_=po)

    # ======================= AllToAll =======================
    with tc.tile_pool(name="ccs", bufs=8) as ccs:
        for j in range(N_CORES):
            for tt in range(TT4):
                mb0 = (j % 4) * TT4 + tt
                stg = ccs.tile([128, HC], F32, tag="stg")
                nc.vector.tensor_scalar_mul(out=stg, in0=xattn[:, mb0, :],
                                            scalar1=maskbc[:, j:j + 1])
                nc.sync.dma_start(
                    out=cc_in[j * TOK + tt * 128:j * TOK + (tt + 1) * 128, :],
                    in_=stg)
        if fake_cc:
            nc.sync.dma_start(out=cc_out, in_=cc_in)
        else:
            nc.gpsimd.collective_compute(
                "AllToAll", ALU.bypass,
                replica_groups=[list(range(N_CORES))],
                ins=[cc_in.opt()], outs=[cc_out.opt()])

    # ======================= FFN =======================
    with tc.tile_pool(name="ffw", bufs=1) as ffw, \
         tc.tile_pool(name="ffa", bufs=3) as ffa, \
         tc.tile_pool(name="ffs", bufs=4) as ffs, \
         tc.tile_pool(name="w1p", bufs=4) as w1p, \
         tc.tile_pool(name="w2p", bufs=3) as w2p, \
         tc.tile_pool(name="psf", bufs=2, space="PSUM") as psfp, \
         tc.tile_pool(name="pso", bufs=1, space="PSUM") as psop, \
         tc.tile_pool(name="pstf", bufs=2, space="PSUM") as pstf:

        g2bc = ffw.tile([128, D], F32)
        nc.sync.dma_start(out=g2bc, in_=g2_d[None, :].to_broadcast([128, D]))
        be2bc = ffw.tile([128, D], F32)
        nc.sync.dma_start(out=be2bc, in_=be2_d[None, :].to_broadcast([128, D]))
        bf2bc = ffw.tile([128, D], F32)
        nc.sync.dma_start(out=bf2bc, in_=bf2_d[None, :].to_broadcast([128, D]))
        bf1sb = ffw.tile([128, DFF // 128], F32)
        nc.sync.dma_start(out=bf1sb, in_=bf1_d.rearrange("(f p) -> p f", p=128))
        bf1h = ffw.tile([128, DFF // 128], F32)
        nc.scalar.activation(out=bf1h, in_=bf1sb, func=AF.Copy, scale=0.5)
        x1_all = ffw.tile([128, TT4, D], F32)
        xn1T = ffw.tile([128, D // 128, TOK], F32R)
        hT = ffw.tile([128, DFF // 128, TOK], BF16)

        cc_a = ffw.tile([128, TT4, D], F32)
        cc_b = ffw.tile([128, TT4, D], F32)
        for tt in range(TT4):
            for kk in range(4):
                nc.sync.dma_start(out=cc_a[:, tt, kk * HC:(kk + 1) * HC],
                                  in_=cc_out[kk * TOK + tt * 128:
                                             kk * TOK + (tt + 1) * 128, :])
                nc.sync.dma_start(out=cc_b[:, tt, kk * HC:(kk + 1) * HC],
                                  in_=cc_out[(kk + 4) * TOK + tt * 128:
                                             (kk + 4) * TOK + (tt + 1) * 128, :])
        for tt in range(TT4):
            xa = ffa.tile([128, D], F32, tag="xa")
            nc.vector.tensor_add(out=xa, in0=cc_a[:, tt, :], in1=cc_b[:, tt, :])
            xtk = ffa.tile([128, D], F32, tag="xtk")
            nc.sync.dma_start(out=xtk, in_=x_tok[tt * 128:(tt + 1) * 128, :])
            nc.gpsimd.tensor_add(out=x1_all[:, tt, :], in0=xtk, in1=xa)
            # LN2
            st = ffs.tile([128, 2, 6], F32, tag="st")
            for sg in range(2):
                nc.vector.bn_stats(out=st[:, sg, :],
                                   in_=x1_all[:, tt, sg * 512:(sg + 1) * 512])
            mv = ffs.tile([128, 2], F32, tag="mv")
            nc.vector.bn_aggr(out=mv, in_=st)
            rstd = ffs.tile([128, 1], F32, tag="rstd")
            nc.scalar.activation(out=rstd, in_=mv[:, 1:2], func=AF.Sqrt,
                                 bias=epsT, scale=1.0)
            nc.vector.reciprocal(out=rstd, in_=rstd)
            xn1 = ffa.tile([128, D], F32, tag="xn1")
            nc.vector.tensor_scalar(out=xn1, in0=x1_all[:, tt, :],
                                    scalar1=mv[:, 0:1], scalar2=rstd,
                                    op0=ALU.subtract, op1=ALU.mult)
            nc.vector.tensor_mul(out=xn1, in0=xn1, in1=g2bc)
            nc.gpsimd.tensor_add(out=xn1, in0=xn1, in1=be2bc)
            for dd in range(D // 128):
                pt = pstf.tile([128, 128], F32, tag="pt")
                nc.tensor.transpose(pt, xn1[:, dd * 128:(dd + 1) * 128], ident)
                nc.scalar.copy(out=xn1T[:, dd, tt * 128:(tt + 1) * 128], in_=pt)

        # h^T = gelu(W1^T @ xn1^T + bf1)
        for f in range(DFF // 128):
            w1f = w1p.tile([128, D // 128, 128], F32, tag="w1f")
            nc.sync.dma_start(
                out=w1f,
                in_=w1_d.rearrange("(dd p) ff -> p dd ff",
                                   p=128)[:, :, f * 128:(f + 1) * 128])
            w1fr = w1p.tile([128, D // 128, 128], F32R, tag="w1fr")
            nc.gpsimd.tensor_copy(out=w1fr, in_=w1f)
            ph = psfp.tile([128, TOK], F32, tag="ph")
            for dd in range(D // 128):
                nc.tensor.matmul(ph, w1fr[:, dd, :], xn1T[:, dd, :],
                                 start=(dd == 0), stop=(dd == D // 128 - 1))
            # gelu (tanh approx), computed on y = x/2:
            #   gelu(x) = y*(1+tanh(y*(2*c0 + 8*c3*y^2))), c0=sqrt(2/pi), c3=0.044715*c0
            gy = ffa.tile([128, TOK], F32, tag="gy")
            nc.scalar.activation(out=gy, in_=ph, func=AF.Identity, scale=0.5,
                                 bias=bf1h[:, f:f + 1])
            gt = ffa.tile([128, TOK], F32, tag="gt")
            nc.scalar.activation(out=gt, in_=gy, func=AF.Square, scale=1.0)
            nc.vector.tensor_scalar(out=gt, in0=gt, scalar1=8 * 0.044715 * GC0,
                                    scalar2=2 * GC0, op0=ALU.mult, op1=ALU.add)
            nc.vector.tensor_mul(out=gt, in0=gt, in1=gy)
            nc.scalar.activation(out=gt, in_=gt, func=AF.Tanh, scale=1.0)
            nc.vector.scalar_tensor_tensor(out=hT[:, f, :], in0=gt, scalar=1.0,
                                           in1=gy, op0=ALU.add, op1=ALU.mult)

        # out = x1 + h @ W2 + bf2   (W2 streamed, bf16)
        for dh in range(D // 512):
            pos = [psop.tile([128, 512], F32, tag=f"po{tt}", name=f"po{tt}") for tt in range(TT4)]
            for f in range(DFF // 128):
                w2s = w2p.tile([128, 512], F32, tag="w2s")
                nc.sync.dma_start(out=w2s,
                                  in_=w2_d[f * 128:(f + 1) * 128,
                                           dh * 512:(dh + 1) * 512])
                w2b = w2p.tile([128, 512], BF16, tag="w2b")
                nc.gpsimd.tensor_copy(out=w2b, in_=w2s)
                for tt in range(TT4):
                    nc.tensor.matmul(pos[tt], hT[:, f, tt * 128:(tt + 1) * 128],
                                     w2b, start=(f == 0),
                                     stop=(f == DFF // 128 - 1))
            for tt in range(TT4):
                o1 = ffa.tile([128, 512], F32, tag="o1")
                nc.vector.tensor_add(out=o1, in0=pos[tt],
                                     in1=x1_all[:, tt, dh * 512:(dh + 1) * 512])
                nc.vector.tensor_add(out=o1, in0=o1,
                                     in1=bf2bc[:, dh * 512:(dh + 1) * 512])
                nc.sync.dma_start(out=out_d[tt * 128:(tt + 1) * 128,
                                            dh * 512:(dh + 1) * 512], in_=o1)

    ctx.close()


# ======================= host-side driver =======================

def shard_inputs(inputs, S=S_FULL):
    """Build per-core in_maps from full inputs."""
    x = np.ascontiguousarray(inputs["x"], dtype=np.float32)
    TOK = S // 4
    in_maps = []
    for c in range(N_CORES):
        b = c // 4
        hg = c % 4
        hsl = slice(hg * NHL, (hg + 1) * NHL)            # global head indices
        csl = slice(hg * NHL * HD, (hg + 1) * NHL * HD)  # head cols in D
        rsl = slice(hg * TOK, (hg + 1) * TOK)            # FFN token rows
        m = {
            "x_full": x[b],
            "x_heads": x[b][:, csl],
            "x_tok": x[b][rsl, :],
            "wq": inputs["Wq"][hsl].reshape(NHL * HD, HD),
            "wk": inputs["Wk"][hsl].reshape(NHL * HD, HD),
            "wv": inputs["Wv"][hsl].reshape(NHL * HD, HD),
            "wo": inputs["Wo"][hsl].reshape(NHL * HD, HD),
            "omega": inputs["omega"][hsl],
            "g1h": inputs["g1"][csl],
            "be1h": inputs["be1"][csl],
            "g2": inputs["g2"], "be2": inputs["be2"],
            "w1": inputs["W1"], "bf1": inputs["bf1"],
            "w2": inputs["W2"], "bf2": inputs["bf2"],
            "gmask": np.array([1.0 if j // 4 == b else 0.0
                               for j in range(N_CORES)], dtype=np.float32),
        }
        in_maps.append({k: np.ascontiguousarray(v, dtype=np.float32)
                        for k, v in m.items()})
    return in_maps


def assemble_output(results, S=S_FULL):
    TOK = S // 4
    out = np.zeros((B, S, D), dtype=np.float32)
    for c in range(N_CORES):
        b, hg = c // 4, c % 4
        out[b, hg * TOK:(hg + 1) * TOK, :] = results[c]["out"]
    return out


_NC_CACHE = {}


def kernel(**inputs):
    from concourse.bass_utils import run_bass_kernel_spmd
    S = inputs["x"].shape[1]
    if S not in _NC_CACHE:
        _NC_CACHE[S] = build_nc(S)
    nc = _NC_CACHE[S]
    in_maps = shard_inputs(inputs, S)
    res = run_bass_kernel_spmd(nc, in_maps, core_ids=list(range(N_CORES)))
    return assemble_output(res.results, S)



# revision 19
# speedup vs baseline: 1.2408x; 1.2408x over previous
"""Trainium2 Bass kernel for MinimalResonanceLayer (8-core SPMD).

Sharding: core c handles batch b = c//4 and local heads [ (c%4)*4, (c%4)*4+4 ).
Each head's resonance recurrence runs fully on-core (E^T resident in SBUF,
bf16); the head-concat + FFN uses one 8-core AllToAll, with per-core
divergence encoded in an input mask so the program stays SPMD-uniform.

State is kept in rotated coordinates z~ = K*exp(-i*alpha)*z so the
Kuramoto-Sakaguchi rotation folds into the PSUM copy-out scalars.
"""
import math
import numpy as np

import concourse.bass as bass
import concourse.tile as tile
from concourse import bacc, mybir
from concourse.masks import make_identity

# ---- problem constants (hardcoded per contest contract) ----
B, S_FULL, D, H, HD = 2, 2048, 1024, 16, 64
DFF = 2 * D
MU, ALPHA, K_COUP, DT, STEPS, MIX = 1.0, 0.1, 3.0, 0.02, 5, 0.3
N_CORES = 8
NHL = 4  # heads per core

CA, SA = math.cos(ALPHA), math.sin(ALPHA)
C1V = K_COUP * CA            # c1
C2V = K_COUP * SA            # c2
CC1 = MU - K_COUP            # -2.0
R21 = C2V / C1V              # tan(alpha)
W1S = C1V - C2V * C2V / C1V  # pass-1 roa scale
W2S = -2.0 * C2V             # pass-1 rob scale
M2 = (1.0 - MIX) * CA / K_COUP
M3 = (1.0 - MIX) * SA / K_COUP
SCL = 1.0 / math.sqrt(HD)
INVK = 1.0 / K_COUP
GC0 = math.sqrt(2.0 / math.pi)

F32 = mybir.dt.float32
F32R = mybir.dt.float32r
BF16 = mybir.dt.bfloat16
ALU = mybir.AluOpType
AF = mybir.ActivationFunctionType


def r(ap):
    """bitcast fp32 AP -> float32r for full-rate PE streaming."""
    return ap.bitcast(F32R)


def build_nc(S=S_FULL, fake_cc=False):
    """Build the 8-core SPMD program. S = sequence length (per batch).

    fake_cc=True replaces the AllToAll with a local DMA so the program is
    collective-free (for TimelineSim cost modeling only).
    """
    nc = bacc.Bacc("TRN2", target_bir_lowering=False, debug=False,
                   num_devices=N_CORES)

    def din(name, shape):
        return nc.dram_tensor(name, shape, F32, kind="ExternalInput").ap()

    TOK = S // 4
    io = dict(
        x_full=din("x_full", [S, D]),
        x_heads=din("x_heads", [S, NHL * HD]),
        x_tok=din("x_tok", [TOK, D]),
        wq_d=din("wq", [NHL * HD, HD]),
        wk_d=din("wk", [NHL * HD, HD]),
        wv_d=din("wv", [NHL * HD, HD]),
        wo_d=din("wo", [NHL * HD, HD]),
        om_d=din("omega", [NHL, HD]),
        g1_d=din("g1h", [NHL * HD]),
        be1_d=din("be1h", [NHL * HD]),
        g2_d=din("g2", [D]),
        be2_d=din("be2", [D]),
        w1_d=din("w1", [D, DFF]),
        bf1_d=din("bf1", [DFF]),
        w2_d=din("w2", [DFF, D]),
        bf2_d=din("bf2", [D]),
        gm_d=din("gmask", [N_CORES]),
        out_d=nc.dram_tensor("out", [TOK, D], F32, kind="ExternalOutput").ap(),
    )

    with tile.TileContext(nc) as tc:
        _body(nc, tc, io, S, fake_cc)

    nc.compile()
    return nc


def _body(nc, tc, io, S, fake_cc=False):
    NMB = S // 128          # token blocks of 128
    NG = NMB // 4           # groups of 4 blocks (512 tokens)
    TOK = S // 4            # FFN tokens per core (= B*S/8)
    TT4 = TOK // 128        # FFN token tiles
    NSL = S // 512          # 512-wide column slices of S
    HC = NHL * HD           # 256 head cols per core

    x_full, x_heads, x_tok = io["x_full"], io["x_heads"], io["x_tok"]
    wq_d, wk_d, wv_d, wo_d = io["wq_d"], io["wk_d"], io["wv_d"], io["wo_d"]
    om_d, g1_d, be1_d = io["om_d"], io["g1_d"], io["be1_d"]
    g2_d, be2_d = io["g2_d"], io["be2_d"]
    w1_d, bf1_d, w2_d, bf2_d = io["w1_d"], io["bf1_d"], io["w2_d"], io["bf2_d"]
    gm_d, out_d = io["gm_d"], io["out_d"]

    from contextlib import ExitStack
    ctx = ExitStack()
    sing = ctx.enter_context(tc.tile_pool(name="sing", bufs=1))
    dram = ctx.enter_context(tc.tile_pool(name="dram", bufs=1, space="DRAM"))

    # ---- whole-kernel constants ----
    ident = sing.tile([128, 128], F32)
    make_identity(nc, ident)
    identb = sing.tile([128, 128], BF16)
    nc.vector.tensor_copy(out=identb, in_=ident)
    epsT = sing.tile([128, 1], F32)
    nc.vector.memset(epsT, 1e-5)
    maskbc = sing.tile([128, N_CORES], F32)
    nc.sync.dma_start(out=maskbc, in_=gm_d[None, :].to_broadcast([128, N_CORES]))
    xattn = sing.tile([128, NMB, HC], BF16)
    # signed rotation coefficient row: [+R21 | -R21] (for s1 = R21v * wswap)
    R21v = sing.tile([128, 128], BF16)
    nc.vector.memset(R21v[:, 0:HD], R21)
    nc.vector.memset(R21v[:, HD:128], -R21)

    cc_in = dram.tile([N_CORES * TOK, HC], F32)
    cc_out = dram.tile([N_CORES * TOK, HC], F32)

    # =================== attention super-phase ===================
    with ExitStack() as actx:
        big = actx.enter_context(tc.tile_pool(name="big", bufs=1))
        with tc.tile_pool(name="gstg", bufs=1) as gstg:
            g1f = gstg.tile([128, HC], F32)
            nc.sync.dma_start(out=g1f, in_=g1_d[None, :].to_broadcast([128, HC]))
            be1f = gstg.tile([128, HC], F32)
            nc.sync.dma_start(out=be1f,
                              in_=be1_d[None, :].to_broadcast([128, HC]))
            g1bc = big.tile([128, HC], BF16)
            nc.vector.tensor_copy(out=g1bc, in_=g1f)
            be1bc = big.tile([128, HC], BF16)
            nc.vector.tensor_copy(out=be1bc, in_=be1f)
        wq_sb = big.tile([64, NHL, HD], BF16)
        wk_sb = big.tile([64, NHL, HD], BF16)
        wv_sb = big.tile([64, NHL, HD], BF16)
        wo_bf = big.tile([64, NHL, HD], BF16)
        with tc.tile_pool(name="wstg", bufs=1) as wstg:
            wstage = wstg.tile([64, 4, NHL, HD], F32)
            for wi, wd in enumerate([wq_d, wk_d, wv_d, wo_d]):
                nc.sync.dma_start(out=wstage[:, wi, :, :],
                                  in_=wd.rearrange("(h p) e -> p h e", p=HD))
            nc.vector.tensor_copy(out=wq_sb, in_=wstage[:, 0, :, :])
            nc.vector.tensor_copy(out=wk_sb, in_=wstage[:, 1, :, :])
            nc.vector.tensor_copy(out=wv_sb, in_=wstage[:, 2, :, :])
            nc.vector.tensor_copy(out=wo_bf, in_=wstage[:, 3, :, :])

        xnh = big.tile([128, NMB, HC], BF16)
        X = big.tile([128, NMB, 128], BF16)     # state [XA | XB], bf16
        Xp = big.tile([128, NMB, 128], BF16)    # Heun predictor
        tsum = big.tile([128, NMB, 128], BF16)  # Xp + X (for corrector)
        vb1 = big.tile([128, NMB, 128], BF16)   # [v | ones] pass-1 rhs
        attnv = big.tile([128, NMB, HD], BF16)  # A @ v
        rz2dt = big.tile([128, NMB, HD], BF16)   # DT*c1/Z (bcast to halves)
        om_st = big.tile([128, HD], F32)
        omdt = big.tile([128, 128], BF16)       # [-DT*omega | +DT*omega]
        nc.vector.memset(vb1[:, :, HD:128], 1.0)

        def swap_ap(t, gs):
            """halves-swapped view of t[:, gs, :]: [...,[XB|XA],...]"""
            base = t[:, gs, :]
            return bass.AP(tensor=base.tensor, offset=base.offset + HD,
                           ap=[base.ap[0], base.ap[1], [-HD, 2], [1, HD]])

        def bc2_ap(t):
            """[128, g, 64] -> [128, g, 2, 64] broadcast of the half dim"""
            return bass.AP(tensor=t.tensor, offset=t.offset,
                           ap=[t.ap[0], t.ap[1], [0, 2], [1, HD]])

        def row_ap(t, g=4):
            """[128, 128] const row -> [128, g, 128] group-broadcast"""
            return bass.AP(tensor=t.tensor, offset=t.offset,
                           ap=[t.ap[0], [0, g], [1, 128]])

        # ---------------- LN1 ----------------
        with tc.tile_pool(name="ln", bufs=4) as ln, \
             tc.tile_pool(name="lns", bufs=6) as lns:
            for t in range(NMB):
                xt = ln.tile([128, D], F32, tag="xt")
                nc.sync.dma_start(out=xt, in_=x_full[t * 128:(t + 1) * 128, :])
                xh = ln.tile([128, HC], F32, tag="xh")
                nc.sync.dma_start(out=xh, in_=x_heads[t * 128:(t + 1) * 128, :])
                st = lns.tile([128, 2, 6], F32, tag="st")
                for sg in range(2):
                    nc.vector.bn_stats(out=st[:, sg, :],
                                       in_=xt[:, sg * 512:(sg + 1) * 512])
                mv = lns.tile([128, 2], F32, tag="mv")
                nc.vector.bn_aggr(out=mv, in_=st)
                rstd = lns.tile([128, 1], F32, tag="rstd")
                nc.scalar.activation(out=rstd, in_=mv[:, 1:2], func=AF.Sqrt,
                                     bias=epsT, scale=1.0)
                nc.vector.reciprocal(out=rstd, in_=rstd)
                nb = lns.tile([128, 1], F32, tag="nb")
                nc.vector.tensor_scalar(out=nb, in0=mv[:, 0:1], scalar1=rstd,
                                        scalar2=-1.0, op0=ALU.mult, op1=ALU.mult)
                xs = lns.tile([128, HC], F32, tag="xs")
                nc.scalar.activation(out=xs, in_=xh, func=AF.Identity,
                                     scale=rstd, bias=nb)
                nc.vector.tensor_mul(out=xs, in0=xs, in1=g1bc)
                nc.gpsimd.tensor_add(out=xnh[:, t, :], in0=xs, in1=be1bc)

        # ---------------- per-head resonance ----------------
        with tc.tile_pool(name="xhTp", bufs=2) as xhTp, \
             tc.tile_pool(name="etp", bufs=2) as etp, \
             tc.tile_pool(name="qkp", bufs=2) as qkp, \
             tc.tile_pool(name="pmisc", bufs=2, space="PSUM") as pmisc, \
             tc.tile_pool(name="pssc", bufs=2, space="PSUM") as psscp, \
             tc.tile_pool(name="psg", bufs=4, space="PSUM") as psgp, \
             tc.tile_pool(name="scr", bufs=2) as scr, \
             tc.tile_pool(name="mts", bufs=3) as mts:

            def emit_prep(h, ET, qT, kT, xhT):
                """Per-head prep as thunks: xhT transposes, q/k proj,
                scores+exp, then v-proj (v-proj last: WAR on vb1 must
                land after the previous head's pass-1 reads)."""
                th = []

                def omth():
                    src = bass.AP(tensor=om_d.tensor,
                                  offset=om_d.offset + h * HD,
                                  ap=[[0, 128], [1, HD]])
                    nc.sync.dma_start(out=om_st, in_=src)
                    nc.vector.tensor_scalar_mul(out=omdt[:, 0:HD], in0=om_st,
                                                scalar1=-DT)
                    nc.vector.tensor_scalar_mul(out=omdt[:, HD:128], in0=om_st,
                                                scalar1=DT)
                th.append(omth)

                def xhTth(t):
                    pt = pmisc.tile([64, 128], BF16, tag="pm")
                    nc.tensor.transpose(pt, xnh[:, t, h * HD:(h + 1) * HD],
                                        identb)
                    nc.scalar.copy(out=xhT[:, t * 128:(t + 1) * 128], in_=pt)
                for t in range(NMB):
                    th.append(lambda t=t: xhTth(t))

                def projth(sl):
                    pq = pmisc.tile([64, 512], F32, tag="pm")
                    nc.tensor.matmul(pq, wq_sb[:, h, :],
                                     xhT[:, sl * 512:(sl + 1) * 512],
                                     start=True, stop=True)
                    nc.scalar.copy(out=qT[:, sl * 512:(sl + 1) * 512], in_=pq)
                    pk = pmisc.tile([64, 512], F32, tag="pm")
                    nc.tensor.matmul(pk, wk_sb[:, h, :],
                                     xhT[:, sl * 512:(sl + 1) * 512],
                                     start=True, stop=True)
                    nc.scalar.copy(out=kT[:, sl * 512:(sl + 1) * 512], in_=pk)
                for sl in range(NSL):
                    th.append(lambda sl=sl: projth(sl))

                def scoreth(k, sl):
                    c0 = sl * 512
                    ps = psscp.tile([128, 512], F32, tag="ps")
                    nc.tensor.matmul(ps, kT[:, k * 128:(k + 1) * 128],
                                     qT[:, c0:c0 + 512],
                                     start=True, stop=True)
                    nc.scalar.activation(out=ET[:, k, c0:c0 + 512],
                                         in_=ps, func=AF.Exp, scale=SCL)
                for k in range(NMB):
                    for sl in range(NSL):
                        th.append(lambda k=k, sl=sl: scoreth(k, sl))

                def vth(t):
                    pv = pmisc.tile([128, HD], F32, tag="pm")
                    nc.tensor.matmul(pv, xhT[:, t * 128:(t + 1) * 128],
                                     wv_sb[:, h, :], start=True, stop=True)
                    nc.scalar.copy(out=vb1[:, t, 0:HD], in_=pv)
                for t in range(NMB):
                    th.append(lambda t=t: vth(t))
                return th

            def hbufs(h):
                return (etp.tile([128, NMB, S], BF16, tag="ET",
                                 name=f"ET_{h}"),
                        qkp.tile([64, S], BF16, tag="qT", name=f"qT_{h}"),
                        qkp.tile([64, S], BF16, tag="kT", name=f"kT_{h}"),
                        xhTp.tile([64, S], BF16, tag="xhT", name=f"xhT_{h}"))

            hbuf = {0: hbufs(0)}
            for f in emit_prep(0, *hbuf[0]):
                f()
            for h in range(NHL):
                ET, qT, kT, xhT = hbuf[h]
                if h + 1 < NHL:
                    hbuf[h + 1] = hbufs(h + 1)
                    nextq = emit_prep(h + 1, *hbuf[h + 1])
                else:
                    nextq = []
                # v-proj thunks (the last 16) must land after pass-1 of head
                # h finishes reading vb1; draining starts at pass 2 so the
                # in-order PE queue never stalls on the WAR.
                nslots = (2 * STEPS - 1) * NG
                per_slot = max(1, -(-len(nextq) // nslots))

                # --- 10 Heun passes (all-bf16 chain, DT folded in) ---
                for p in range(1, 2 * STEPS + 1):
                    odd = (p % 2 == 1)
                    rhs = vb1 if p == 1 else (X if odd else Xp)
                    xin = X if odd else Xp

                    for g in range(NG):
                        pg = psgp.tile([128, 4, 128], F32, tag="pg")
                        for ml in range(4):
                            mb = g * 4 + ml
                            for k in range(NMB):
                                nc.tensor.matmul(pg[:, ml, :],
                                                 ET[:, k, mb * 128:(mb + 1) * 128],
                                                 rhs[:, k, :],
                                                 start=(k == 0), stop=(k == NMB - 1))
                        gs = slice(g * 4, g * 4 + 4)
                        w = scr.tile([128, 4, 128], BF16, tag="w")
                        nc.scalar.copy(out=w, in_=pg)
                        if p == 1:
                            # psum cols [HD:128] hold Z (replicated);
                            # rz2dt = DT*c1/Z, attnv = (A@v) = pA/Z
                            rcp = scr.tile([128, 4], F32, tag="rcp")
                            nc.vector.reciprocal(out=rcp, in_=pg[:, :, HD:HD + 1])
                            rcpb = scr.tile([128, 4], BF16, tag="rcpb")
                            nc.vector.tensor_copy(out=rcpb, in_=rcp)
                            rb64b = bass.AP(tensor=rcpb.tensor, offset=rcpb.offset,
                                            ap=[rcpb.ap[0], [1, 4], [0, HD]])
                            nc.vector.tensor_scalar_mul(out=rz2dt[:, gs, :],
                                                        in0=rb64b,
                                                        scalar1=DT * C1V)
                            rb64 = bass.AP(tensor=rcpb.tensor, offset=rcpb.offset,
                                           ap=[rcpb.ap[0], [1, 4], [0, HD]])
                            nc.vector.tensor_mul(out=attnv[:, gs, :],
                                                 in0=w[:, :, 0:HD], in1=rb64)
                            # state init X0 = [c1*v | -c2*v]
                            nc.vector.tensor_scalar_mul(out=X[:, gs, 0:HD],
                                                        in0=vb1[:, gs, 0:HD],
                                                        scalar1=C1V)
                            nc.vector.tensor_scalar_mul(out=X[:, gs, HD:128],
                                                        in0=vb1[:, gs, 0:HD],
                                                        scalar1=-C2V)
                            # rotated coupling via W1S/W2S (w holds E@v only);
                            # write cols [HD:] first, then scale [0:HD] in place
                            nc.vector.tensor_scalar_mul(out=w[:, :, HD:128],
                                                        in0=w[:, :, 0:HD],
                                                        scalar1=W2S)
                            nc.vector.tensor_scalar_mul(out=w[:, :, 0:HD],
                                                        in0=w[:, :, 0:HD],
                                                        scalar1=W1S)
                        else:
                            wsw = bass.AP(tensor=w.tensor, offset=w.offset + HD,
                                          ap=[w.ap[0], w.ap[1], [-HD, 2], [1, HD]])
                            s1 = scr.tile([128, 4, 128], BF16, tag="s1")
                            nc.vector.tensor_mul(out=s1, in0=wsw, in1=row_ap(R21v))
                            nc.vector.tensor_add(out=w, in0=w, in1=s1)
                        ro = scr.tile([128, 4, 128], BF16, tag="ro")
                        rzb = rz2dt[:, gs, :]
                        rzb = bass.AP(tensor=rzb.tensor, offset=rzb.offset,
                                      ap=[rzb.ap[0], rzb.ap[1], [0, 2], [1, HD]])
                        nc.vector.tensor_mul(out=ro, in0=w, in1=rzb)
                        # elementwise drift, DT-scaled: dd = DT*f_local + ro
                        sq = scr.tile([128, 4, 128], BF16, tag="sq")
                        nc.scalar.activation(out=sq, in_=xin[:, gs, :],
                                             func=AF.Square, scale=1.0)
                        r2h = scr.tile([128, 4, HD], BF16, tag="r2h")
                        nc.vector.tensor_add(out=r2h, in0=sq[:, :, 0:HD],
                                             in1=sq[:, :, HD:128])
                        mtl = scr.tile([128, 4, HD], BF16, tag="mtl")
                        nc.vector.tensor_scalar(out=mtl, in0=r2h,
                                                scalar1=-DT * INVK * INVK,
                                                scalar2=DT * CC1,
                                                op0=ALU.mult, op1=ALU.add)
                        u = scr.tile([128, 4, 128], BF16, tag="u")
                        nc.vector.tensor_mul(out=u, in0=bc2_ap(mtl),
                                             in1=xin[:, gs, :])
                        cross = scr.tile([128, 4, 128], BF16, tag="cross")
                        nc.gpsimd.tensor_mul(out=cross, in0=row_ap(omdt),
                                             in1=swap_ap(xin, gs))
                        nc.vector.tensor_add(out=u, in0=u, in1=cross)
                        dd = u
                        nc.vector.tensor_add(out=dd, in0=u, in1=ro)
                        if odd:
                            nc.vector.tensor_add(out=Xp[:, gs, :],
                                                 in0=X[:, gs, :], in1=dd)
                            nc.gpsimd.tensor_add(out=tsum[:, gs, :],
                                                 in0=Xp[:, gs, :], in1=X[:, gs, :])
                        else:
                            # X' = 0.5*(Xp + X + dd2)
                            nc.vector.tensor_add(out=dd, in0=tsum[:, gs, :],
                                                 in1=dd)
                            nc.vector.tensor_scalar_mul(out=X[:, gs, :], in0=dd,
                                                        scalar1=0.5)
                        if p >= 2:
                            for _ in range(min(per_slot, len(nextq))):
                                nextq.pop(0)()
                for f in nextq:
                    f()
                nextq = []

                # --- readout: mixed -> @Wo -> xattn cols ---
                nc.vector.tensor_scalar_mul(out=attnv, in0=attnv, scalar1=MIX)
                nc.vector.scalar_tensor_tensor(out=attnv, in0=X[:, :, 0:HD],
                                               scalar=M2, in1=attnv,
                                               op0=ALU.mult, op1=ALU.add)
                nc.vector.scalar_tensor_tensor(out=attnv, in0=X[:, :, HD:128],
                                               scalar=-M3, in1=attnv,
                                               op0=ALU.mult, op1=ALU.add)
                mixv = attnv
                for t in range(NMB):
                    pt = pmisc.tile([64, 128], BF16, tag="pm")
                    nc.tensor.transpose(pt, mixv[:, t, :], identb)
                    mt = mts.tile([64, 128], BF16, tag="mt")
                    nc.scalar.copy(out=mt, in_=pt)
                    po = pmisc.tile([128, HD], F32, tag="pm")
                    nc.tensor.matmul(po, mt, wo_bf[:, h, :],
                                     start=True, stop=True)
                    nc.scalar.copy(out=xattn[:, t, h * HD:(h + 1) * HD], in_=po)

    # ======================= AllToAll =======================
    with tc.tile_pool(name="ccs", bufs=8) as ccs:
        for j in range(N_CORES):
            for tt in range(TT4):
                mb0 = (j % 4) * TT4 + tt
                stg = ccs.tile([128, HC], F32, tag="stg")
                nc.vector.tensor_scalar_mul(out=stg, in0=xattn[:, mb0, :],
                                            scalar1=maskbc[:, j:j + 1])
                nc.sync.dma_start(
                    out=cc_in[j * TOK + tt * 128:j * TOK + (tt + 1) * 128, :],
                    in_=stg)
        if fake_cc:
            nc.sync.dma_start(out=cc_out, in_=cc_in)
        else:
            nc.gpsimd.collective_compute(
                "AllToAll", ALU.bypass,
                replica_groups=[list(range(N_CORES))],
                ins=[cc_in.opt()], outs=[cc_out.opt()])

    # ======================= FFN =======================
    with tc.tile_pool(name="ffw", bufs=1) as ffw, \
         tc.tile_pool(name="ffa", bufs=3) as ffa, \
         tc.tile_pool(name="ffs", bufs=4) as ffs, \
         tc.tile_pool(name="w1p", bufs=4) as w1p, \
         tc.tile_pool(name="w2p", bufs=3) as w2p, \
         tc.tile_pool(name="psf", bufs=2, space="PSUM") as psfp, \
         tc.tile_pool(name="pso", bufs=1, space="PSUM") as psop, \
         tc.tile_pool(name="pstf", bufs=2, space="PSUM") as pstf:

        g2bc = ffw.tile([128, D], F32)
        nc.sync.dma_start(out=g2bc, in_=g2_d[None, :].to_broadcast([128, D]))
        be2bc = ffw.tile([128, D], F32)
        nc.sync.dma_start(out=be2bc, in_=be2_d[None, :].to_broadcast([128, D]))
        bf2bc = ffw.tile([128, D], F32)
        nc.sync.dma_start(out=bf2bc, in_=bf2_d[None, :].to_broadcast([128, D]))
        bf1sb = ffw.tile([128, DFF // 128], F32)
        nc.sync.dma_start(out=bf1sb, in_=bf1_d.rearrange("(f p) -> p f", p=128))
        bf1h = ffw.tile([128, DFF // 128], F32)
        nc.scalar.activation(out=bf1h, in_=bf1sb, func=AF.Copy, scale=0.5)
        x1_all = ffw.tile([128, TT4, D], F32)
        xn1T = ffw.tile([128, D // 128, TOK], F32R)
        hT = ffw.tile([128, DFF // 128, TOK], BF16)

        cc_a = ffw.tile([128, TT4, D], F32)
        cc_b = ffw.tile([128, TT4, D], F32)
        for tt in range(TT4):
            for kk in range(4):
                nc.sync.dma_start(out=cc_a[:, tt, kk * HC:(kk + 1) * HC],
                                  in_=cc_out[kk * TOK + tt * 128:
                                             kk * TOK + (tt + 1) * 128, :])
                nc.sync.dma_start(out=cc_b[:, tt, kk * HC:(kk + 1) * HC],
                                  in_=cc_out[(kk + 4) * TOK + tt * 128:
                                             (kk + 4) * TOK + (tt + 1) * 128, :])
        for tt in range(TT4):
            xa = ffa.tile([128, D], F32, tag="xa")
            nc.vector.tensor_add(out=xa, in0=cc_a[:, tt, :], in1=cc_b[:, tt, :])
            xtk = ffa.tile([128, D], F32, tag="xtk")
            nc.sync.dma_start(out=xtk, in_=x_tok[tt * 128:(tt + 1) * 128, :])
            nc.gpsimd.tensor_add(out=x1_all[:, tt, :], in0=xtk, in1=xa)
            # LN2
            st = ffs.tile([128, 2, 6], F32, tag="st")
            for sg in range(2):
                nc.vector.bn_stats(out=st[:, sg, :],
                                   in_=x1_all[:, tt, sg * 512:(sg + 1) * 512])
            mv = ffs.tile([128, 2], F32, tag="mv")
            nc.vector.bn_aggr(out=mv, in_=st)
            rstd = ffs.tile([128, 1], F32, tag="rstd")
            nc.scalar.activation(out=rstd, in_=mv[:, 1:2], func=AF.Sqrt,
                                 bias=epsT, scale=1.0)
            nc.vector.reciprocal(out=rstd, in_=rstd)
            xn1 = ffa.tile([128, D], F32, tag="xn1")
            nc.vector.tensor_scalar(out=xn1, in0=x1_all[:, tt, :],
                                    scalar1=mv[:, 0:1], scalar2=rstd,
                                    op0=ALU.subtract, op1=ALU.mult)
            nc.vector.tensor_mul(out=xn1, in0=xn1, in1=g2bc)
            nc.gpsimd.tensor_add(out=xn1, in0=xn1, in1=be2bc)
            for dd in range(D // 128):
                pt = pstf.tile([128, 128], F32, tag="pt")
                nc.tensor.transpose(pt, xn1[:, dd * 128:(dd + 1) * 128], ident)
                nc.scalar.copy(out=xn1T[:, dd, tt * 128:(tt + 1) * 128], in_=pt)

        # h^T = gelu(W1^T @ xn1^T + bf1)
        for f in range(DFF // 128):
            w1f = w1p.tile([128, D // 128, 128], F32, tag="w1f")
            nc.sync.dma_start(
                out=w1f,
                in_=w1_d.rearrange("(dd p) ff -> p dd ff",
                                   p=128)[:, :, f * 128:(f + 1) * 128])
            w1fr = w1p.tile([128, D // 128, 128], F32R, tag="w1fr")
            nc.gpsimd.tensor_copy(out=w1fr, in_=w1f)
            ph = psfp.tile([128, TOK], F32, tag="ph")
            for dd in range(D // 128):
                nc.tensor.matmul(ph, w1fr[:, dd, :], xn1T[:, dd, :],
                                 start=(dd == 0), stop=(dd == D // 128 - 1))
            # gelu (tanh approx), computed on y = x/2:
            #   gelu(x) = y*(1+tanh(y*(2*c0 + 8*c3*y^2))), c0=sqrt(2/pi), c3=0.044715*c0
            gy = ffa.tile([128, TOK], F32, tag="gy")
            nc.scalar.activation(out=gy, in_=ph, func=AF.Identity, scale=0.5,
                                 bias=bf1h[:, f:f + 1])
            gt = ffa.tile([128, TOK], F32, tag="gt")
            nc.scalar.activation(out=gt, in_=gy, func=AF.Square, scale=1.0)
            nc.vector.tensor_scalar(out=gt, in0=gt, scalar1=8 * 0.044715 * GC0,
                                    scalar2=2 * GC0, op0=ALU.mult, op1=ALU.add)
            nc.vector.tensor_mul(out=gt, in0=gt, in1=gy)
            nc.scalar.activation(out=gt, in_=gt, func=AF.Tanh, scale=1.0)
            nc.vector.scalar_tensor_tensor(out=hT[:, f, :], in0=gt, scalar=1.0,
                                           in1=gy, op0=ALU.add, op1=ALU.mult)

        # out = x1 + h @ W2 + bf2   (W2 streamed, bf16)
        for dh in range(D // 512):
            pos = [psop.tile([128, 512], F32, tag=f"po{tt}", name=f"po{tt}") for tt in range(TT4)]
            for f in range(DFF // 128):
                w2s = w2p.tile([128, 512], F32, tag="w2s")
                nc.sync.dma_start(out=w2s,
                                  in_=w2_d[f * 128:(f + 1) * 128,
                                           dh * 512:(dh + 1) * 512])
                w2b = w2p.tile([128, 512], BF16, tag="w2b")
                nc.gpsimd.tensor_copy(out=w2b, in_=w2s)
                for tt in range(TT4):
                    nc.tensor.matmul(pos[tt], hT[:, f, tt * 128:(tt + 1) * 128],
                                     w2b, start=(f == 0),
                                     stop=(f == DFF // 128 - 1))
            for tt in range(TT4):
                o1 = ffa.tile([128, 512], F32, tag="o1")
                nc.vector.tensor_add(out=o1, in0=pos[tt],
                                     in1=x1_all[:, tt, dh * 512:(dh + 1) * 512])
                nc.vector.tensor_add(out=o1, in0=o1,
                                     in1=bf2bc[:, dh * 512:(dh + 1) * 512])
                nc.sync.dma_start(out=out_d[tt * 128:(tt + 1) * 128,
                                            dh * 512:(dh + 1) * 512], in_=o1)

    ctx.close()


# ======================= host-side driver =======================

def shard_inputs(inputs, S=S_FULL):
    """Build per-core in_maps from full inputs."""
    x = np.ascontiguousarray(inputs["x"], dtype=np.float32)
    TOK = S // 4
    in_maps = []
    for c in range(N_CORES):
        b = c // 4
        hg = c % 4
        hsl = slice(hg * NHL, (hg + 1) * NHL)            # global head indices
        csl = slice(hg * NHL * HD, (hg + 1) * NHL * HD)  # head cols in D
        rsl = slice(hg * TOK, (hg + 1) * TOK)            # FFN token rows
        m = {
            "x_full": x[b],
            "x_heads": x[b][:, csl],
            "x_tok": x[b][rsl, :],
            "wq": inputs["Wq"][hsl].reshape(NHL * HD, HD),
            "wk": inputs["Wk"][hsl].reshape(NHL * HD, HD),
            "wv": inputs["Wv"][hsl].reshape(NHL * HD, HD),
            "wo": inputs["Wo"][hsl].reshape(NHL * HD, HD),
            "omega": inputs["omega"][hsl],
            "g1h": inputs["g1"][csl],
            "be1h": inputs["be1"][csl],
            "g2": inputs["g2"], "be2": inputs["be2"],
            "w1": inputs["W1"], "bf1": inputs["bf1"],
            "w2": inputs["W2"], "bf2": inputs["bf2"],
            "gmask": np.array([1.0 if j // 4 == b else 0.0
                               for j in range(N_CORES)], dtype=np.float32),
        }
        in_maps.append({k: np.ascontiguousarray(v, dtype=np.float32)
                        for k, v in m.items()})
    return in_maps


def assemble_output(results, S=S_FULL):
    TOK = S // 4
    out = np.zeros((B, S, D), dtype=np.float32)
    for c in range(N_CORES):
        b, hg = c // 4, c % 4
        out[b, hg * TOK:(hg + 1) * TOK, :] = results[c]["out"]
    return out


_NC_CACHE = {}


def kernel(**inputs):
    from concourse.bass_utils import run_bass_kernel_spmd
    S = inputs["x"].shape[1]
    if S not in _NC_CACHE:
        _NC_CACHE[S] = build_nc(S)
    nc = _NC_CACHE[S]
    in_maps = shard_inputs(inputs, S)
    res = run_bass_kernel_spmd(nc, in_maps, core_ids=list(range(N_CORES)))
    return assemble_output(res.results, S)



# revision 26
# speedup vs baseline: 1.2599x; 1.0154x over previous
"""Trainium2 Bass kernel for MinimalResonanceLayer (8-core SPMD).

Sharding: core c handles batch b = c//4 and local heads [ (c%4)*4, (c%4)*4+4 ).
Each head's resonance recurrence runs fully on-core (E^T resident in SBUF,
bf16); the head-concat + FFN uses one 8-core AllToAll, with per-core
divergence encoded in an input mask so the program stays SPMD-uniform.

State is kept in rotated coordinates z~ = K*exp(-i*alpha)*z so the
Kuramoto-Sakaguchi rotation folds into the PSUM copy-out scalars.
"""
import math
import numpy as np

import concourse.bass as bass
import concourse.tile as tile
from concourse import bacc, mybir
from concourse.masks import make_identity

# ---- problem constants (hardcoded per contest contract) ----
B, S_FULL, D, H, HD = 2, 2048, 1024, 16, 64
DFF = 2 * D
MU, ALPHA, K_COUP, DT, STEPS, MIX = 1.0, 0.1, 3.0, 0.02, 5, 0.3
N_CORES = 8
NHL = 4  # heads per core

CA, SA = math.cos(ALPHA), math.sin(ALPHA)
C1V = K_COUP * CA            # c1
C2V = K_COUP * SA            # c2
CC1 = MU - K_COUP            # -2.0
R21 = C2V / C1V              # tan(alpha)
W1S = C1V - C2V * C2V / C1V  # pass-1 roa scale
W2S = -2.0 * C2V             # pass-1 rob scale
M2 = (1.0 - MIX) * CA / K_COUP
M3 = (1.0 - MIX) * SA / K_COUP
SCL = 1.0 / math.sqrt(HD)
INVK = 1.0 / K_COUP
GC0 = math.sqrt(2.0 / math.pi)

F32 = mybir.dt.float32
F32R = mybir.dt.float32r
BF16 = mybir.dt.bfloat16
ALU = mybir.AluOpType
AF = mybir.ActivationFunctionType


def r(ap):
    """bitcast fp32 AP -> float32r for full-rate PE streaming."""
    return ap.bitcast(F32R)


def build_nc(S=S_FULL, fake_cc=False):
    """Build the 8-core SPMD program. S = sequence length (per batch).

    fake_cc=True replaces the AllToAll with a local DMA so the program is
    collective-free (for TimelineSim cost modeling only).
    """
    nc = bacc.Bacc("TRN2", target_bir_lowering=False, debug=False,
                   num_devices=N_CORES)

    def din(name, shape):
        return nc.dram_tensor(name, shape, F32, kind="ExternalInput").ap()

    TOK = S // 4
    io = dict(
        x_full=din("x_full", [S, D]),
        x_heads=din("x_heads", [S, NHL * HD]),
        x_tok=din("x_tok", [TOK, D]),
        wq_d=din("wq", [NHL * HD, HD]),
        wk_d=din("wk", [NHL * HD, HD]),
        wv_d=din("wv", [NHL * HD, HD]),
        wo_d=din("wo", [NHL * HD, HD]),
        om_d=din("omega", [NHL, HD]),
        g1_d=din("g1h", [NHL * HD]),
        be1_d=din("be1h", [NHL * HD]),
        g2_d=din("g2", [D]),
        be2_d=din("be2", [D]),
        w1_d=din("w1", [D, DFF]),
        bf1_d=din("bf1", [DFF]),
        w2_d=din("w2", [DFF, D]),
        bf2_d=din("bf2", [D]),
        gm_d=din("gmask", [N_CORES]),
        out_d=nc.dram_tensor("out", [TOK, D], F32, kind="ExternalOutput").ap(),
    )

    with tile.TileContext(nc) as tc:
        _body(nc, tc, io, S, fake_cc)

    nc.compile()
    return nc


def _body(nc, tc, io, S, fake_cc=False):
    NMB = S // 128          # token blocks of 128
    NG = NMB // 4           # groups of 4 blocks (512 tokens)
    TOK = S // 4            # FFN tokens per core (= B*S/8)
    TT4 = TOK // 128        # FFN token tiles
    NSL = S // 512          # 512-wide column slices of S
    HC = NHL * HD           # 256 head cols per core

    x_full, x_heads, x_tok = io["x_full"], io["x_heads"], io["x_tok"]
    wq_d, wk_d, wv_d, wo_d = io["wq_d"], io["wk_d"], io["wv_d"], io["wo_d"]
    om_d, g1_d, be1_d = io["om_d"], io["g1_d"], io["be1_d"]
    g2_d, be2_d = io["g2_d"], io["be2_d"]
    w1_d, bf1_d, w2_d, bf2_d = io["w1_d"], io["bf1_d"], io["w2_d"], io["bf2_d"]
    gm_d, out_d = io["gm_d"], io["out_d"]

    from contextlib import ExitStack
    ctx = ExitStack()
    sing = ctx.enter_context(tc.tile_pool(name="sing", bufs=1))
    dram = ctx.enter_context(tc.tile_pool(name="dram", bufs=1, space="DRAM"))

    # ---- whole-kernel constants ----
    ident = sing.tile([128, 128], F32)
    make_identity(nc, ident)
    identb = sing.tile([128, 128], BF16)
    nc.vector.tensor_copy(out=identb, in_=ident)
    epsT = sing.tile([128, 1], F32)
    nc.vector.memset(epsT, 1e-5)
    maskbc = sing.tile([128, N_CORES], F32)
    nc.sync.dma_start(out=maskbc, in_=gm_d[None, :].to_broadcast([128, N_CORES]))
    # signed rotation coefficient row: [+R21 | -R21] (for s1 = R21v * wswap)
    R21v = sing.tile([128, 128], BF16)
    nc.vector.memset(R21v[:, 0:HD], R21)
    nc.vector.memset(R21v[:, HD:128], -R21)

    cc_in = dram.tile([TOK // 128, N_CORES * 128, HC], F32)
    cc_out = dram.tile([TOK // 128, N_CORES * 128, HC], F32)

    # =================== attention super-phase ===================
    with ExitStack() as actx:
        big = actx.enter_context(tc.tile_pool(name="big", bufs=1))
        with tc.tile_pool(name="gstg", bufs=1) as gstg:
            g1f = gstg.tile([128, HC], F32)
            nc.sync.dma_start(out=g1f, in_=g1_d[None, :].to_broadcast([128, HC]))
            be1f = gstg.tile([128, HC], F32)
            nc.sync.dma_start(out=be1f,
                              in_=be1_d[None, :].to_broadcast([128, HC]))
            g1bc = big.tile([128, HC], BF16)
            nc.vector.tensor_copy(out=g1bc, in_=g1f)
            be1bc = big.tile([128, HC], BF16)
            nc.vector.tensor_copy(out=be1bc, in_=be1f)
        wq_sb = big.tile([64, NHL, HD], BF16)
        wk_sb = big.tile([64, NHL, HD], BF16)
        wv_sb = big.tile([64, NHL, HD], BF16)
        wo_bf = big.tile([64, NHL, HD], BF16)
        with tc.tile_pool(name="wstg", bufs=1) as wstg:
            wstage = wstg.tile([64, 4, NHL, HD], F32)
            for wi, wd in enumerate([wq_d, wk_d, wv_d, wo_d]):
                nc.sync.dma_start(out=wstage[:, wi, :, :],
                                  in_=wd.rearrange("(h p) e -> p h e", p=HD))
            nc.vector.tensor_copy(out=wq_sb, in_=wstage[:, 0, :, :])
            nc.vector.tensor_copy(out=wk_sb, in_=wstage[:, 1, :, :])
            nc.vector.tensor_copy(out=wv_sb, in_=wstage[:, 2, :, :])
            nc.vector.tensor_copy(out=wo_bf, in_=wstage[:, 3, :, :])

        xnh = big.tile([128, NMB, HC], BF16)
        X = big.tile([128, NMB, 128], BF16)     # state [XA | XB], bf16
        Xp = big.tile([128, NMB, 128], BF16)    # Heun predictor
        tsum = big.tile([128, NMB, 128], BF16)  # Xp + X (for corrector)
        vb1 = big.tile([128, NMB, 128], BF16)   # [v | ones] pass-1 rhs
        attnv = big.tile([128, NMB, HD], BF16)  # A @ v
        rz2dt = big.tile([128, NMB, HD], BF16)   # DT*c1/Z (bcast to halves)
        om_st = big.tile([128, HD], F32)
        omdt = big.tile([128, 128], BF16)       # [-DT*omega | +DT*omega]
        nc.vector.memset(vb1[:, :, HD:128], 1.0)

        def swap_ap(t, gs):
            """halves-swapped view of t[:, gs, :]: [...,[XB|XA],...]"""
            base = t[:, gs, :]
            return bass.AP(tensor=base.tensor, offset=base.offset + HD,
                           ap=[base.ap[0], base.ap[1], [-HD, 2], [1, HD]])

        def bc2_ap(t):
            """[128, g, 64] -> [128, g, 2, 64] broadcast of the half dim"""
            return bass.AP(tensor=t.tensor, offset=t.offset,
                           ap=[t.ap[0], t.ap[1], [0, 2], [1, HD]])

        def row_ap(t, g=4):
            """[128, 128] const row -> [128, g, 128] group-broadcast"""
            return bass.AP(tensor=t.tensor, offset=t.offset,
                           ap=[t.ap[0], [0, g], [1, 128]])

        # ---------------- LN1 ----------------
        with tc.tile_pool(name="ln", bufs=4) as ln, \
             tc.tile_pool(name="lns", bufs=6) as lns:
            for t in range(NMB):
                xt = ln.tile([128, D], F32, tag="xt")
                nc.sync.dma_start(out=xt, in_=x_full[t * 128:(t + 1) * 128, :])
                xh = ln.tile([128, HC], F32, tag="xh")
                nc.sync.dma_start(out=xh, in_=x_heads[t * 128:(t + 1) * 128, :])
                st = lns.tile([128, 2, 6], F32, tag="st")
                for sg in range(2):
                    nc.vector.bn_stats(out=st[:, sg, :],
                                       in_=xt[:, sg * 512:(sg + 1) * 512])
                mv = lns.tile([128, 2], F32, tag="mv")
                nc.vector.bn_aggr(out=mv, in_=st)
                rstd = lns.tile([128, 1], F32, tag="rstd")
                nc.scalar.activation(out=rstd, in_=mv[:, 1:2], func=AF.Sqrt,
                                     bias=epsT, scale=1.0)
                nc.vector.reciprocal(out=rstd, in_=rstd)
                nb = lns.tile([128, 1], F32, tag="nb")
                nc.vector.tensor_scalar(out=nb, in0=mv[:, 0:1], scalar1=rstd,
                                        scalar2=-1.0, op0=ALU.mult, op1=ALU.mult)
                xs = lns.tile([128, HC], F32, tag="xs")
                nc.scalar.activation(out=xs, in_=xh, func=AF.Identity,
                                     scale=rstd, bias=nb)
                nc.vector.tensor_mul(out=xs, in0=xs, in1=g1bc)
                nc.gpsimd.tensor_add(out=xnh[:, t, :], in0=xs, in1=be1bc)

        # ---------------- per-head resonance ----------------
        with tc.tile_pool(name="xhTp", bufs=2) as xhTp, \
             tc.tile_pool(name="etp", bufs=2) as etp, \
             tc.tile_pool(name="qkp", bufs=2) as qkp, \
             tc.tile_pool(name="pmisc", bufs=2, space="PSUM") as pmisc, \
             tc.tile_pool(name="pssc", bufs=2, space="PSUM") as psscp, \
             tc.tile_pool(name="psg", bufs=4, space="PSUM") as psgp, \
             tc.tile_pool(name="scr", bufs=2) as scr, \
             tc.tile_pool(name="mts", bufs=2) as mts:

            def emit_prep(h, ET, qT, kT, xhT):
                """Per-head prep as thunks: xhT transposes, q/k proj,
                scores+exp, then v-proj (v-proj last: WAR on vb1 must
                land after the previous head's pass-1 reads)."""
                th = []

                def omth():
                    src = bass.AP(tensor=om_d.tensor,
                                  offset=om_d.offset + h * HD,
                                  ap=[[0, 128], [1, HD]])
                    nc.sync.dma_start(out=om_st, in_=src)
                    nc.vector.tensor_scalar_mul(out=omdt[:, 0:HD], in0=om_st,
                                                scalar1=-DT)
                    nc.vector.tensor_scalar_mul(out=omdt[:, HD:128], in0=om_st,
                                                scalar1=DT)
                th.append(omth)

                def xhTth(t):
                    pt = pmisc.tile([64, 128], BF16, tag="pm")
                    nc.tensor.transpose(pt, xnh[:, t, h * HD:(h + 1) * HD],
                                        identb)
                    nc.scalar.copy(out=xhT[:, t * 128:(t + 1) * 128], in_=pt)
                for t in range(NMB):
                    th.append(lambda t=t: xhTth(t))

                def projth(sl):
                    pq = pmisc.tile([64, 512], F32, tag="pm")
                    nc.tensor.matmul(pq, wq_sb[:, h, :],
                                     xhT[:, sl * 512:(sl + 1) * 512],
                                     start=True, stop=True)
                    nc.scalar.copy(out=qT[:, sl * 512:(sl + 1) * 512], in_=pq)
                    pk = pmisc.tile([64, 512], F32, tag="pm")
                    nc.tensor.matmul(pk, wk_sb[:, h, :],
                                     xhT[:, sl * 512:(sl + 1) * 512],
                                     start=True, stop=True)
                    nc.scalar.copy(out=kT[:, sl * 512:(sl + 1) * 512], in_=pk)
                for sl in range(NSL):
                    th.append(lambda sl=sl: projth(sl))

                def scoreth(k, sl):
                    c0 = sl * 512
                    ps = psscp.tile([128, 512], F32, tag="ps")
                    nc.tensor.matmul(ps, kT[:, k * 128:(k + 1) * 128],
                                     qT[:, c0:c0 + 512],
                                     start=True, stop=True)
                    nc.scalar.activation(out=ET[:, k, c0:c0 + 512],
                                         in_=ps, func=AF.Exp, scale=SCL)
                for sl in range(NSL):
                    for k in range(NMB):
                        th.append(lambda k=k, sl=sl: scoreth(k, sl))

                def vth(t):
                    pv = pmisc.tile([128, HD], F32, tag="pm")
                    nc.tensor.matmul(pv, xhT[:, t * 128:(t + 1) * 128],
                                     wv_sb[:, h, :], start=True, stop=True)
                    nc.scalar.copy(out=vb1[:, t, 0:HD], in_=pv)
                for t in range(NMB):
                    th.append(lambda t=t: vth(t))
                return th

            def hbufs(h):
                return (etp.tile([128, NMB, S], BF16, tag="ET",
                                 name=f"ET_{h}"),
                        qkp.tile([64, S], BF16, tag="qT", name=f"qT_{h}"),
                        qkp.tile([64, S], BF16, tag="kT", name=f"kT_{h}"),
                        xhTp.tile([64, S], BF16, tag="xhT", name=f"xhT_{h}"))

            hbuf = {0: hbufs(0)}
            for f in emit_prep(0, *hbuf[0]):
                f()
            for h in range(NHL):
                ET, qT, kT, xhT = hbuf[h]
                if h + 1 < NHL:
                    hbuf[h + 1] = hbufs(h + 1)
                    nextq = emit_prep(h + 1, *hbuf[h + 1])
                else:
                    nextq = []
                # v-proj thunks (the last 16) must land after pass-1 of head
                # h finishes reading vb1; draining starts at pass 2 so the
                # in-order PE queue never stalls on the WAR.
                nslots = (2 * STEPS - 1) * NG
                per_slot = max(1, -(-len(nextq) // nslots))

                # --- 10 Heun passes (all-bf16 chain, DT folded in) ---
                for p in range(1, 2 * STEPS + 1):
                    odd = (p % 2 == 1)
                    rhs = vb1 if p == 1 else (X if odd else Xp)
                    xin = X if odd else Xp

                    for g in range(NG):
                        pg = psgp.tile([128, 4, 128], F32, tag="pg")
                        for ml in range(4):
                            mb = g * 4 + ml
                            for k in range(NMB):
                                nc.tensor.matmul(pg[:, ml, :],
                                                 ET[:, k, mb * 128:(mb + 1) * 128],
                                                 rhs[:, k, :],
                                                 start=(k == 0), stop=(k == NMB - 1))
                        gs = slice(g * 4, g * 4 + 4)
                        w = scr.tile([128, 4, 128], BF16, tag="w")
                        nc.scalar.copy(out=w, in_=pg)
                        if p == 1:
                            # psum cols [HD:128] hold Z (replicated);
                            # rz2dt = DT*c1/Z, attnv = (A@v) = pA/Z
                            rcp = scr.tile([128, 4], F32, tag="rcp")
                            nc.vector.reciprocal(out=rcp, in_=pg[:, :, HD:HD + 1])
                            rcpb = scr.tile([128, 4], BF16, tag="rcpb")
                            nc.vector.tensor_copy(out=rcpb, in_=rcp)
                            rb64b = bass.AP(tensor=rcpb.tensor, offset=rcpb.offset,
                                            ap=[rcpb.ap[0], [1, 4], [0, HD]])
                            nc.vector.tensor_scalar_mul(out=rz2dt[:, gs, :],
                                                        in0=rb64b,
                                                        scalar1=DT * C1V)
                            rb64 = bass.AP(tensor=rcpb.tensor, offset=rcpb.offset,
                                           ap=[rcpb.ap[0], [1, 4], [0, HD]])
                            nc.vector.tensor_mul(out=attnv[:, gs, :],
                                                 in0=w[:, :, 0:HD], in1=rb64)
                            # state init X0 = [c1*v | -c2*v]
                            nc.vector.tensor_scalar_mul(out=X[:, gs, 0:HD],
                                                        in0=vb1[:, gs, 0:HD],
                                                        scalar1=C1V)
                            nc.vector.tensor_scalar_mul(out=X[:, gs, HD:128],
                                                        in0=vb1[:, gs, 0:HD],
                                                        scalar1=-C2V)
                            # rotated coupling via W1S/W2S (w holds E@v only);
                            # write cols [HD:] first, then scale [0:HD] in place
                            nc.vector.tensor_scalar_mul(out=w[:, :, HD:128],
                                                        in0=w[:, :, 0:HD],
                                                        scalar1=W2S)
                            nc.vector.tensor_scalar_mul(out=w[:, :, 0:HD],
                                                        in0=w[:, :, 0:HD],
                                                        scalar1=W1S)
                        else:
                            wsw = bass.AP(tensor=w.tensor, offset=w.offset + HD,
                                          ap=[w.ap[0], w.ap[1], [-HD, 2], [1, HD]])
                            s1 = scr.tile([128, 4, 128], BF16, tag="s1")
                            nc.vector.tensor_mul(out=s1, in0=wsw, in1=row_ap(R21v))
                            nc.vector.tensor_add(out=w, in0=w, in1=s1)
                        ro = scr.tile([128, 4, 128], BF16, tag="ro")
                        rzb = rz2dt[:, gs, :]
                        rzb = bass.AP(tensor=rzb.tensor, offset=rzb.offset,
                                      ap=[rzb.ap[0], rzb.ap[1], [0, 2], [1, HD]])
                        nc.vector.tensor_mul(out=ro, in0=w, in1=rzb)
                        # elementwise drift, DT-scaled: dd = DT*f_local + ro
                        sq = scr.tile([128, 4, 128], BF16, tag="sq")
                        nc.scalar.activation(out=sq, in_=xin[:, gs, :],
                                             func=AF.Square, scale=1.0)
                        r2h = scr.tile([128, 4, HD], BF16, tag="r2h")
                        nc.vector.tensor_add(out=r2h, in0=sq[:, :, 0:HD],
                                             in1=sq[:, :, HD:128])
                        mtl = scr.tile([128, 4, HD], BF16, tag="mtl")
                        nc.vector.tensor_scalar(out=mtl, in0=r2h,
                                                scalar1=-DT * INVK * INVK,
                                                scalar2=DT * CC1,
                                                op0=ALU.mult, op1=ALU.add)
                        u = scr.tile([128, 4, 128], BF16, tag="u")
                        nc.vector.tensor_mul(out=u, in0=bc2_ap(mtl),
                                             in1=xin[:, gs, :])
                        cross = scr.tile([128, 4, 128], BF16, tag="cross")
                        nc.gpsimd.tensor_mul(out=cross, in0=row_ap(omdt),
                                             in1=swap_ap(xin, gs))
                        nc.vector.tensor_add(out=u, in0=u, in1=cross)
                        dd = u
                        nc.vector.tensor_add(out=dd, in0=u, in1=ro)
                        if odd:
                            nc.vector.tensor_add(out=Xp[:, gs, :],
                                                 in0=X[:, gs, :], in1=dd)
                            nc.gpsimd.tensor_add(out=tsum[:, gs, :],
                                                 in0=Xp[:, gs, :], in1=X[:, gs, :])
                        else:
                            # X' = 0.5*(Xp + X + dd2)
                            nc.vector.tensor_add(out=dd, in0=tsum[:, gs, :],
                                                 in1=dd)
                            nc.vector.tensor_scalar_mul(out=X[:, gs, :], in0=dd,
                                                        scalar1=0.5)
                        if p >= 2:
                            for _ in range(min(per_slot, len(nextq))):
                                nextq.pop(0)()
                for f in nextq:
                    f()
                nextq = []

                # --- readout: mixed -> @Wo -> xattn cols ---
                nc.vector.tensor_scalar_mul(out=attnv, in0=attnv, scalar1=MIX)
                nc.vector.scalar_tensor_tensor(out=attnv, in0=X[:, :, 0:HD],
                                               scalar=M2, in1=attnv,
                                               op0=ALU.mult, op1=ALU.add)
                nc.vector.scalar_tensor_tensor(out=attnv, in0=X[:, :, HD:128],
                                               scalar=-M3, in1=attnv,
                                               op0=ALU.mult, op1=ALU.add)
                mixv = attnv
                xatth = mts.tile([128, NMB, HD], BF16, tag="xatth",
                                 name=f"xatth_{h}")
                for t in range(NMB):
                    pt = pmisc.tile([64, 128], BF16, tag="pm")
                    nc.tensor.transpose(pt, mixv[:, t, :], identb)
                    mt = mts.tile([64, 128], BF16, tag="mt")
                    nc.scalar.copy(out=mt, in_=pt)
                    po = pmisc.tile([128, HD], F32, tag="pm")
                    nc.tensor.matmul(po, mt, wo_bf[:, h, :],
                                     start=True, stop=True)
                    nc.scalar.copy(out=xatth[:, t, :], in_=po)

                # --- stage this head's slice of cc_in (masked) ---
                for j in range(N_CORES):
                    t0 = (j % 4) * TT4
                    stg = mts.tile([128, TT4, HD], F32, tag="stg",
                                   name=f"stg_{h}_{j}")
                    nc.vector.tensor_scalar_mul(
                        out=stg,
                        in0=xatth[:, t0:t0 + TT4, :],
                        scalar1=maskbc[:, j:j + 1])
                    base = cc_in[0, j * 128:(j + 1) * 128, :]
                    dst = bass.AP(tensor=base.tensor,
                                  offset=base.offset + h * HD,
                                  ap=[[HC, 128], [N_CORES * 128 * HC, TT4],
                                      [1, HD]])
                    nc.sync.dma_start(out=dst, in_=stg)

    # =================== AllToAll (per-tt) + FFN ===================
    with tc.tile_pool(name="ffw", bufs=1) as ffw, \
         tc.tile_pool(name="ffa", bufs=3) as ffa, \
         tc.tile_pool(name="ffs", bufs=4) as ffs, \
         tc.tile_pool(name="w1p", bufs=4) as w1p, \
         tc.tile_pool(name="w2p", bufs=3) as w2p, \
         tc.tile_pool(name="psf", bufs=2, space="PSUM") as psfp, \
         tc.tile_pool(name="pso", bufs=1, space="PSUM") as psop, \
         tc.tile_pool(name="pstf", bufs=2, space="PSUM") as pstf:

        xtk_all = ffw.tile([128, TT4, D], F32)
        for tt in range(TT4):
            nc.sync.dma_start(out=xtk_all[:, tt, :],
                              in_=x_tok[tt * 128:(tt + 1) * 128, :])
        for tt in range(TT4):
            if fake_cc:
                nc.sync.dma_start(out=cc_out[tt, :, :], in_=cc_in[tt, :, :])
            else:
                nc.gpsimd.collective_compute(
                    "AllToAll", ALU.bypass,
                    replica_groups=[list(range(N_CORES))],
                    ins=[cc_in[tt, :, :].opt()],
                    outs=[cc_out[tt, :, :].opt()])

        g2bc = ffw.tile([128, D], F32)
        nc.sync.dma_start(out=g2bc, in_=g2_d[None, :].to_broadcast([128, D]))
        be2bc = ffw.tile([128, D], F32)
        nc.sync.dma_start(out=be2bc, in_=be2_d[None, :].to_broadcast([128, D]))
        bf2bc = ffw.tile([128, D], F32)
        nc.sync.dma_start(out=bf2bc, in_=bf2_d[None, :].to_broadcast([128, D]))
        bf1sb = ffw.tile([128, DFF // 128], F32)
        nc.sync.dma_start(out=bf1sb, in_=bf1_d.rearrange("(f p) -> p f", p=128))
        bf1h = ffw.tile([128, DFF // 128], F32)
        nc.scalar.activation(out=bf1h, in_=bf1sb, func=AF.Copy, scale=0.5)
        x1_all = ffw.tile([128, TT4, D], F32)
        xn1T = ffw.tile([128, D // 128, TOK], BF16)
        hT = ffw.tile([128, DFF // 128, TOK], BF16)

        cc_a = ffw.tile([128, TT4, D], F32)
        cc_b = ffw.tile([128, TT4, D], F32)
        for tt in range(TT4):
            for half, dstt in ((0, cc_a), (1, cc_b)):
                srcb = cc_out[tt, half * 4 * 128:(half * 4 + 4) * 128, :]
                srca = bass.AP(tensor=srcb.tensor, offset=srcb.offset,
                               ap=[[HC, 128], [128 * HC, 4], [1, HC]])
                nc.sync.dma_start(out=dstt[:, tt, :], in_=srca)
        for tt in range(TT4):
            xa = ffa.tile([128, D], F32, tag="xa")
            nc.vector.tensor_add(out=xa, in0=cc_a[:, tt, :], in1=cc_b[:, tt, :])
            nc.gpsimd.tensor_add(out=x1_all[:, tt, :], in0=xtk_all[:, tt, :],
                                 in1=xa)
            # LN2
            st = ffs.tile([128, 2, 6], F32, tag="st")
            for sg in range(2):
                nc.vector.bn_stats(out=st[:, sg, :],
                                   in_=x1_all[:, tt, sg * 512:(sg + 1) * 512])
            mv = ffs.tile([128, 2], F32, tag="mv")
            nc.vector.bn_aggr(out=mv, in_=st)
            rstd = ffs.tile([128, 1], F32, tag="rstd")
            nc.scalar.activation(out=rstd, in_=mv[:, 1:2], func=AF.Sqrt,
                                 bias=epsT, scale=1.0)
            nc.vector.reciprocal(out=rstd, in_=rstd)
            xn1 = ffa.tile([128, D], F32, tag="xn1")
            nc.vector.tensor_scalar(out=xn1, in0=x1_all[:, tt, :],
                                    scalar1=mv[:, 0:1], scalar2=rstd,
                                    op0=ALU.subtract, op1=ALU.mult)
            nc.vector.tensor_mul(out=xn1, in0=xn1, in1=g2bc)
            nc.gpsimd.tensor_add(out=xn1, in0=xn1, in1=be2bc)
            for dd in range(D // 128):
                pt = pstf.tile([128, 128], F32, tag="pt")
                nc.tensor.transpose(pt, xn1[:, dd * 128:(dd + 1) * 128], ident)
                nc.scalar.copy(out=xn1T[:, dd, tt * 128:(tt + 1) * 128], in_=pt)

        # h^T = gelu(W1^T @ xn1^T + bf1)
        for f in range(DFF // 128):
            w1f = w1p.tile([128, D // 128, 128], F32, tag="w1f")
            nc.sync.dma_start(
                out=w1f,
                in_=w1_d.rearrange("(dd p) ff -> p dd ff",
                                   p=128)[:, :, f * 128:(f + 1) * 128])
            w1b = w1p.tile([128, D // 128, 128], BF16, tag="w1b")
            nc.vector.tensor_copy(out=w1b, in_=w1f)
            ph = psfp.tile([128, TOK], F32, tag="ph")
            for dd in range(D // 128):
                nc.tensor.matmul(ph, w1b[:, dd, :], xn1T[:, dd, :],
                                 start=(dd == 0), stop=(dd == D // 128 - 1))
            # gelu (tanh approx), computed on y = x/2:
            #   gelu(x) = y*(1+tanh(y*(2*c0 + 8*c3*y^2))), c0=sqrt(2/pi), c3=0.044715*c0
            gy = ffa.tile([128, TOK], F32, tag="gy")
            nc.scalar.activation(out=gy, in_=ph, func=AF.Identity, scale=0.5,
                                 bias=bf1h[:, f:f + 1])
            gt = ffa.tile([128, TOK], F32, tag="gt")
            nc.scalar.activation(out=gt, in_=gy, func=AF.Square, scale=1.0)
            nc.vector.tensor_scalar(out=gt, in0=gt, scalar1=8 * 0.044715 * GC0,
                                    scalar2=2 * GC0, op0=ALU.mult, op1=ALU.add)
            nc.vector.tensor_mul(out=gt, in0=gt, in1=gy)
            nc.scalar.activation(out=gt, in_=gt, func=AF.Tanh, scale=1.0)
            nc.vector.scalar_tensor_tensor(out=hT[:, f, :], in0=gt, scalar=1.0,
                                           in1=gy, op0=ALU.add, op1=ALU.mult)

        # out = x1 + h @ W2 + bf2   (W2 streamed, bf16)
        for dh in range(D // 512):
            pos = [psop.tile([128, 512], F32, tag=f"po{tt}", name=f"po{tt}") for tt in range(TT4)]
            for f in range(DFF // 128):
                w2s = w2p.tile([128, 512], F32, tag="w2s")
                nc.sync.dma_start(out=w2s,
                                  in_=w2_d[f * 128:(f + 1) * 128,
                                           dh * 512:(dh + 1) * 512])
                w2b = w2p.tile([128, 512], BF16, tag="w2b")
                nc.vector.tensor_copy(out=w2b, in_=w2s)
                for tt in range(TT4):
                    nc.tensor.matmul(pos[tt], hT[:, f, tt * 128:(tt + 1) * 128],
                                     w2b, start=(f == 0),
                                     stop=(f == DFF // 128 - 1))
            for tt in range(TT4):
                o1 = ffa.tile([128, 512], F32, tag="o1")
                nc.vector.tensor_add(out=o1, in0=pos[tt],
                                     in1=x1_all[:, tt, dh * 512:(dh + 1) * 512])
                nc.vector.tensor_add(out=o1, in0=o1,
                                     in1=bf2bc[:, dh * 512:(dh + 1) * 512])
                nc.sync.dma_start(out=out_d[tt * 128:(tt + 1) * 128,
                                            dh * 512:(dh + 1) * 512], in_=o1)

    ctx.close()


# ======================= host-side driver =======================

def shard_inputs(inputs, S=S_FULL):
    """Build per-core in_maps from full inputs."""
    x = np.ascontiguousarray(inputs["x"], dtype=np.float32)
    TOK = S // 4
    in_maps = []
    for c in range(N_CORES):
        b = c // 4
        hg = c % 4
        hsl = slice(hg * NHL, (hg + 1) * NHL)            # global head indices
        csl = slice(hg * NHL * HD, (hg + 1) * NHL * HD)  # head cols in D
        rsl = slice(hg * TOK, (hg + 1) * TOK)            # FFN token rows
        m = {
            "x_full": x[b],
            "x_heads": x[b][:, csl],
            "x_tok": x[b][rsl, :],
            "wq": inputs["Wq"][hsl].reshape(NHL * HD, HD),
            "wk": inputs["Wk"][hsl].reshape(NHL * HD, HD),
            "wv": inputs["Wv"][hsl].reshape(NHL * HD, HD),
            "wo": inputs["Wo"][hsl].reshape(NHL * HD, HD),
            "omega": inputs["omega"][hsl],
            "g1h": inputs["g1"][csl],
            "be1h": inputs["be1"][csl],
            "g2": inputs["g2"], "be2": inputs["be2"],
            "w1": inputs["W1"], "bf1": inputs["bf1"],
            "w2": inputs["W2"], "bf2": inputs["bf2"],
            "gmask": np.array([1.0 if j // 4 == b else 0.0
                               for j in range(N_CORES)], dtype=np.float32),
        }
        in_maps.append({k: np.ascontiguousarray(v, dtype=np.float32)
                        for k, v in m.items()})
    return in_maps


def assemble_output(results, S=S_FULL):
    TOK = S // 4
    out = np.zeros((B, S, D), dtype=np.float32)
    for c in range(N_CORES):
        b, hg = c // 4, c % 4
        out[b, hg * TOK:(hg + 1) * TOK, :] = results[c]["out"]
    return out


_NC_CACHE = {}


def kernel(**inputs):
    from concourse.bass_utils import run_bass_kernel_spmd
    S = inputs["x"].shape[1]
    if S not in _NC_CACHE:
        _NC_CACHE[S] = build_nc(S)
    nc = _NC_CACHE[S]
    in_maps = shard_inputs(inputs, S)
    res = run_bass_kernel_spmd(nc, in_maps, core_ids=list(range(N_CORES)))
    return assemble_output(res.results, S)



# revision 27
# speedup vs baseline: 1.2813x; 1.0170x over previous
"""Trainium2 Bass kernel for MinimalResonanceLayer (8-core SPMD).

Sharding: core c handles batch b = c//4 and local heads [ (c%4)*4, (c%4)*4+4 ).
Each head's resonance recurrence runs fully on-core (E^T resident in SBUF,
bf16); the head-concat + FFN uses one 8-core AllToAll, with per-core
divergence encoded in an input mask so the program stays SPMD-uniform.

State is kept in rotated coordinates z~ = K*exp(-i*alpha)*z so the
Kuramoto-Sakaguchi rotation folds into the PSUM copy-out scalars.
"""
import math
import numpy as np

import concourse.bass as bass
import concourse.tile as tile
from concourse import bacc, mybir
from concourse.masks import make_identity

# ---- problem constants (hardcoded per contest contract) ----
B, S_FULL, D, H, HD = 2, 2048, 1024, 16, 64
DFF = 2 * D
MU, ALPHA, K_COUP, DT, STEPS, MIX = 1.0, 0.1, 3.0, 0.02, 5, 0.3
N_CORES = 8
NHL = 4  # heads per core

CA, SA = math.cos(ALPHA), math.sin(ALPHA)
C1V = K_COUP * CA            # c1
C2V = K_COUP * SA            # c2
CC1 = MU - K_COUP            # -2.0
R21 = C2V / C1V              # tan(alpha)
W1S = C1V - C2V * C2V / C1V  # pass-1 roa scale
W2S = -2.0 * C2V             # pass-1 rob scale
M2 = (1.0 - MIX) * CA / K_COUP
M3 = (1.0 - MIX) * SA / K_COUP
SCL = 1.0 / math.sqrt(HD)
INVK = 1.0 / K_COUP
GC0 = math.sqrt(2.0 / math.pi)

F32 = mybir.dt.float32
F32R = mybir.dt.float32r
BF16 = mybir.dt.bfloat16
ALU = mybir.AluOpType
AF = mybir.ActivationFunctionType


def r(ap):
    """bitcast fp32 AP -> float32r for full-rate PE streaming."""
    return ap.bitcast(F32R)


def build_nc(S=S_FULL, fake_cc=False):
    """Build the 8-core SPMD program. S = sequence length (per batch).

    fake_cc=True replaces the AllToAll with a local DMA so the program is
    collective-free (for TimelineSim cost modeling only).
    """
    nc = bacc.Bacc("TRN2", target_bir_lowering=False, debug=False,
                   num_devices=N_CORES)

    def din(name, shape):
        return nc.dram_tensor(name, shape, F32, kind="ExternalInput").ap()

    TOK = S // 4
    io = dict(
        x_full=din("x_full", [S, D]),
        x_heads=din("x_heads", [S, NHL * HD]),
        x_tok=din("x_tok", [TOK, D]),
        wq_d=din("wq", [NHL * HD, HD]),
        wk_d=din("wk", [NHL * HD, HD]),
        wv_d=din("wv", [NHL * HD, HD]),
        wo_d=din("wo", [NHL * HD, HD]),
        om_d=din("omega", [NHL, HD]),
        g1_d=din("g1h", [NHL * HD]),
        be1_d=din("be1h", [NHL * HD]),
        g2_d=din("g2", [D]),
        be2_d=din("be2", [D]),
        w1_d=nc.dram_tensor("w1b", [D, DFF], BF16,
                            kind="ExternalInput").ap(),
        bf1_d=din("bf1", [DFF]),
        w2_d=nc.dram_tensor("w2b", [DFF, D], BF16,
                            kind="ExternalInput").ap(),
        bf2_d=din("bf2", [D]),
        gm_d=din("gmask", [N_CORES]),
        out_d=nc.dram_tensor("out", [TOK, D], F32, kind="ExternalOutput").ap(),
    )

    with tile.TileContext(nc) as tc:
        _body(nc, tc, io, S, fake_cc)

    nc.compile()
    return nc


def _body(nc, tc, io, S, fake_cc=False):
    NMB = S // 128          # token blocks of 128
    NG = NMB // 4           # groups of 4 blocks (512 tokens)
    TOK = S // 4            # FFN tokens per core (= B*S/8)
    TT4 = TOK // 128        # FFN token tiles
    NSL = S // 512          # 512-wide column slices of S
    HC = NHL * HD           # 256 head cols per core

    x_full, x_heads, x_tok = io["x_full"], io["x_heads"], io["x_tok"]
    wq_d, wk_d, wv_d, wo_d = io["wq_d"], io["wk_d"], io["wv_d"], io["wo_d"]
    om_d, g1_d, be1_d = io["om_d"], io["g1_d"], io["be1_d"]
    g2_d, be2_d = io["g2_d"], io["be2_d"]
    w1_d, bf1_d, w2_d, bf2_d = io["w1_d"], io["bf1_d"], io["w2_d"], io["bf2_d"]
    gm_d, out_d = io["gm_d"], io["out_d"]

    from contextlib import ExitStack
    ctx = ExitStack()
    sing = ctx.enter_context(tc.tile_pool(name="sing", bufs=1))
    dram = ctx.enter_context(tc.tile_pool(name="dram", bufs=1, space="DRAM"))

    # ---- whole-kernel constants ----
    ident = sing.tile([128, 128], F32)
    make_identity(nc, ident)
    identb = sing.tile([128, 128], BF16)
    nc.vector.tensor_copy(out=identb, in_=ident)
    epsT = sing.tile([128, 1], F32)
    nc.vector.memset(epsT, 1e-5)
    maskbc = sing.tile([128, N_CORES], F32)
    nc.sync.dma_start(out=maskbc, in_=gm_d[None, :].to_broadcast([128, N_CORES]))
    # signed rotation coefficient row: [+R21 | -R21] (for s1 = R21v * wswap)
    R21v = sing.tile([128, 128], BF16)
    nc.vector.memset(R21v[:, 0:HD], R21)
    nc.vector.memset(R21v[:, HD:128], -R21)

    cc_in = dram.tile([TOK // 128, N_CORES * 128, HC], BF16)
    cc_out = dram.tile([TOK // 128, N_CORES * 128, HC], BF16)

    # =================== attention super-phase ===================
    with ExitStack() as actx:
        big = actx.enter_context(tc.tile_pool(name="big", bufs=1))
        with tc.tile_pool(name="gstg", bufs=1) as gstg:
            g1f = gstg.tile([128, HC], F32)
            nc.sync.dma_start(out=g1f, in_=g1_d[None, :].to_broadcast([128, HC]))
            be1f = gstg.tile([128, HC], F32)
            nc.sync.dma_start(out=be1f,
                              in_=be1_d[None, :].to_broadcast([128, HC]))
            g1bc = big.tile([128, HC], BF16)
            nc.vector.tensor_copy(out=g1bc, in_=g1f)
            be1bc = big.tile([128, HC], BF16)
            nc.vector.tensor_copy(out=be1bc, in_=be1f)
        wq_sb = big.tile([64, NHL, HD], BF16)
        wk_sb = big.tile([64, NHL, HD], BF16)
        wv_sb = big.tile([64, NHL, HD], BF16)
        wo_bf = big.tile([64, NHL, HD], BF16)
        with tc.tile_pool(name="wstg", bufs=1) as wstg:
            wstage = wstg.tile([64, 4, NHL, HD], F32)
            for wi, wd in enumerate([wq_d, wk_d, wv_d, wo_d]):
                nc.sync.dma_start(out=wstage[:, wi, :, :],
                                  in_=wd.rearrange("(h p) e -> p h e", p=HD))
            nc.vector.tensor_copy(out=wq_sb, in_=wstage[:, 0, :, :])
            nc.vector.tensor_copy(out=wk_sb, in_=wstage[:, 1, :, :])
            nc.vector.tensor_copy(out=wv_sb, in_=wstage[:, 2, :, :])
            nc.vector.tensor_copy(out=wo_bf, in_=wstage[:, 3, :, :])

        xnh = big.tile([128, NMB, HC], BF16)
        X = big.tile([128, NMB, 128], BF16)     # state [XA | XB], bf16
        Xp = big.tile([128, NMB, 128], BF16)    # Heun predictor
        tsum = big.tile([128, NMB, 128], BF16)  # Xp + X (for corrector)
        vb1 = big.tile([128, NMB, 128], BF16)   # [v | ones] pass-1 rhs
        attnv = big.tile([128, NMB, HD], BF16)  # A @ v
        rz2dt = big.tile([128, NMB, HD], BF16)   # DT*c1/Z (bcast to halves)
        om_st = big.tile([128, HD], F32)
        omdt = big.tile([128, 128], BF16)       # [-DT*omega | +DT*omega]
        nc.vector.memset(vb1[:, :, HD:128], 1.0)

        def swap_ap(t, gs):
            """halves-swapped view of t[:, gs, :]: [...,[XB|XA],...]"""
            base = t[:, gs, :]
            return bass.AP(tensor=base.tensor, offset=base.offset + HD,
                           ap=[base.ap[0], base.ap[1], [-HD, 2], [1, HD]])

        def bc2_ap(t):
            """[128, g, 64] -> [128, g, 2, 64] broadcast of the half dim"""
            return bass.AP(tensor=t.tensor, offset=t.offset,
                           ap=[t.ap[0], t.ap[1], [0, 2], [1, HD]])

        def row_ap(t, g=4):
            """[128, 128] const row -> [128, g, 128] group-broadcast"""
            return bass.AP(tensor=t.tensor, offset=t.offset,
                           ap=[t.ap[0], [0, g], [1, 128]])

        # ---------------- LN1 ----------------
        with tc.tile_pool(name="ln", bufs=4) as ln, \
             tc.tile_pool(name="lns", bufs=6) as lns:
            for t in range(NMB):
                xt = ln.tile([128, D], F32, tag="xt")
                nc.sync.dma_start(out=xt, in_=x_full[t * 128:(t + 1) * 128, :])
                xh = ln.tile([128, HC], F32, tag="xh")
                nc.sync.dma_start(out=xh, in_=x_heads[t * 128:(t + 1) * 128, :])
                st = lns.tile([128, 2, 6], F32, tag="st")
                for sg in range(2):
                    nc.vector.bn_stats(out=st[:, sg, :],
                                       in_=xt[:, sg * 512:(sg + 1) * 512])
                mv = lns.tile([128, 2], F32, tag="mv")
                nc.vector.bn_aggr(out=mv, in_=st)
                rstd = lns.tile([128, 1], F32, tag="rstd")
                nc.scalar.activation(out=rstd, in_=mv[:, 1:2], func=AF.Sqrt,
                                     bias=epsT, scale=1.0)
                nc.vector.reciprocal(out=rstd, in_=rstd)
                nb = lns.tile([128, 1], F32, tag="nb")
                nc.vector.tensor_scalar(out=nb, in0=mv[:, 0:1], scalar1=rstd,
                                        scalar2=-1.0, op0=ALU.mult, op1=ALU.mult)
                xs = lns.tile([128, HC], F32, tag="xs")
                nc.gpsimd.tensor_scalar(out=xs, in0=xh, scalar1=rstd,
                                        scalar2=nb, op0=ALU.mult, op1=ALU.add)
                nc.vector.tensor_mul(out=xs, in0=xs, in1=g1bc)
                nc.gpsimd.tensor_add(out=xnh[:, t, :], in0=xs, in1=be1bc)

        # ---------------- per-head resonance ----------------
        with tc.tile_pool(name="xhTp", bufs=2) as xhTp, \
             tc.tile_pool(name="etp", bufs=2) as etp, \
             tc.tile_pool(name="qkp", bufs=2) as qkp, \
             tc.tile_pool(name="pmisc", bufs=2, space="PSUM") as pmisc, \
             tc.tile_pool(name="pssc", bufs=2, space="PSUM") as psscp, \
             tc.tile_pool(name="psg", bufs=4, space="PSUM") as psgp, \
             tc.tile_pool(name="scr", bufs=2) as scr, \
             tc.tile_pool(name="mts", bufs=2) as mts:

            def emit_prep(h, ET, qT, kT, xhT):
                """Per-head prep as thunks: xhT transposes, q/k proj,
                scores+exp, then v-proj (v-proj last: WAR on vb1 must
                land after the previous head's pass-1 reads)."""
                th = []

                def omth():
                    src = bass.AP(tensor=om_d.tensor,
                                  offset=om_d.offset + h * HD,
                                  ap=[[0, 128], [1, HD]])
                    nc.sync.dma_start(out=om_st, in_=src)
                    nc.vector.tensor_scalar_mul(out=omdt[:, 0:HD], in0=om_st,
                                                scalar1=-DT)
                    nc.vector.tensor_scalar_mul(out=omdt[:, HD:128], in0=om_st,
                                                scalar1=DT)
                th.append(omth)

                def xhTth(t):
                    pt = pmisc.tile([64, 128], BF16, tag="pm")
                    nc.tensor.transpose(pt, xnh[:, t, h * HD:(h + 1) * HD],
                                        identb)
                    nc.scalar.copy(out=xhT[:, t * 128:(t + 1) * 128], in_=pt)
                for t in range(NMB):
                    th.append(lambda t=t: xhTth(t))

                def projth(sl):
                    pq = pmisc.tile([64, 512], F32, tag="pm")
                    nc.tensor.matmul(pq, wq_sb[:, h, :],
                                     xhT[:, sl * 512:(sl + 1) * 512],
                                     start=True, stop=True)
                    nc.scalar.copy(out=qT[:, sl * 512:(sl + 1) * 512], in_=pq)
                    pk = pmisc.tile([64, 512], F32, tag="pm")
                    nc.tensor.matmul(pk, wk_sb[:, h, :],
                                     xhT[:, sl * 512:(sl + 1) * 512],
                                     start=True, stop=True)
                    nc.scalar.copy(out=kT[:, sl * 512:(sl + 1) * 512], in_=pk)
                for sl in range(NSL):
                    th.append(lambda sl=sl: projth(sl))

                def scoreth(k, sl):
                    c0 = sl * 512
                    ps = psscp.tile([128, 512], F32, tag="ps")
                    nc.tensor.matmul(ps, kT[:, k * 128:(k + 1) * 128],
                                     qT[:, c0:c0 + 512],
                                     start=True, stop=True)
                    nc.scalar.activation(out=ET[:, k, c0:c0 + 512],
                                         in_=ps, func=AF.Exp, scale=SCL)
                for sl in range(NSL):
                    for k in range(NMB):
                        th.append(lambda k=k, sl=sl: scoreth(k, sl))

                def vth(t):
                    pv = pmisc.tile([128, HD], F32, tag="pm")
                    nc.tensor.matmul(pv, xhT[:, t * 128:(t + 1) * 128],
                                     wv_sb[:, h, :], start=True, stop=True)
                    nc.scalar.copy(out=vb1[:, t, 0:HD], in_=pv)
                for t in range(NMB):
                    th.append(lambda t=t: vth(t))
                return th

            def hbufs(h):
                return (etp.tile([128, NMB, S], BF16, tag="ET",
                                 name=f"ET_{h}"),
                        qkp.tile([64, S], BF16, tag="qT", name=f"qT_{h}"),
                        qkp.tile([64, S], BF16, tag="kT", name=f"kT_{h}"),
                        xhTp.tile([64, S], BF16, tag="xhT", name=f"xhT_{h}"))

            hbuf = {0: hbufs(0)}
            for f in emit_prep(0, *hbuf[0]):
                f()
            for h in range(NHL):
                ET, qT, kT, xhT = hbuf[h]
                if h + 1 < NHL:
                    hbuf[h + 1] = hbufs(h + 1)
                    nextq = emit_prep(h + 1, *hbuf[h + 1])
                else:
                    nextq = []
                # v-proj thunks (the last 16) must land after pass-1 of head
                # h finishes reading vb1; draining starts at pass 2 so the
                # in-order PE queue never stalls on the WAR.
                nslots = (2 * STEPS - 1) * NG
                per_slot = max(1, -(-len(nextq) // nslots))

                # --- 10 Heun passes (all-bf16 chain, DT folded in) ---
                for p in range(1, 2 * STEPS + 1):
                    odd = (p % 2 == 1)
                    rhs = vb1 if p == 1 else (X if odd else Xp)
                    xin = X if odd else Xp

                    for g in range(NG):
                        pg = psgp.tile([128, 4, 128], F32, tag="pg")
                        for ml in range(4):
                            mb = g * 4 + ml
                            for k in range(NMB):
                                nc.tensor.matmul(pg[:, ml, :],
                                                 ET[:, k, mb * 128:(mb + 1) * 128],
                                                 rhs[:, k, :],
                                                 start=(k == 0), stop=(k == NMB - 1))
                        gs = slice(g * 4, g * 4 + 4)
                        w = scr.tile([128, 4, 128], BF16, tag="w")
                        nc.scalar.copy(out=w, in_=pg)
                        if p == 1:
                            # psum cols [HD:128] hold Z (replicated);
                            # rz2dt = DT*c1/Z, attnv = (A@v) = pA/Z
                            rcp = scr.tile([128, 4], F32, tag="rcp")
                            nc.vector.reciprocal(out=rcp, in_=pg[:, :, HD:HD + 1])
                            rcpb = scr.tile([128, 4], BF16, tag="rcpb")
                            nc.vector.tensor_copy(out=rcpb, in_=rcp)
                            rb64b = bass.AP(tensor=rcpb.tensor, offset=rcpb.offset,
                                            ap=[rcpb.ap[0], [1, 4], [0, HD]])
                            nc.vector.tensor_scalar_mul(out=rz2dt[:, gs, :],
                                                        in0=rb64b,
                                                        scalar1=DT * C1V)
                            rb64 = bass.AP(tensor=rcpb.tensor, offset=rcpb.offset,
                                           ap=[rcpb.ap[0], [1, 4], [0, HD]])
                            nc.vector.tensor_mul(out=attnv[:, gs, :],
                                                 in0=w[:, :, 0:HD], in1=rb64)
                            # state init X0 = [c1*v | -c2*v]
                            nc.vector.tensor_scalar_mul(out=X[:, gs, 0:HD],
                                                        in0=vb1[:, gs, 0:HD],
                                                        scalar1=C1V)
                            nc.vector.tensor_scalar_mul(out=X[:, gs, HD:128],
                                                        in0=vb1[:, gs, 0:HD],
                                                        scalar1=-C2V)
                            # rotated coupling via W1S/W2S (w holds E@v only);
                            # write cols [HD:] first, then scale [0:HD] in place
                            nc.vector.tensor_scalar_mul(out=w[:, :, HD:128],
                                                        in0=w[:, :, 0:HD],
                                                        scalar1=W2S)
                            nc.vector.tensor_scalar_mul(out=w[:, :, 0:HD],
                                                        in0=w[:, :, 0:HD],
                                                        scalar1=W1S)
                        else:
                            wsw = bass.AP(tensor=w.tensor, offset=w.offset + HD,
                                          ap=[w.ap[0], w.ap[1], [-HD, 2], [1, HD]])
                            s1 = scr.tile([128, 4, 128], BF16, tag="s1")
                            nc.vector.tensor_mul(out=s1, in0=wsw, in1=row_ap(R21v))
                            nc.vector.tensor_add(out=w, in0=w, in1=s1)
                        ro = scr.tile([128, 4, 128], BF16, tag="ro")
                        rzb = rz2dt[:, gs, :]
                        rzb = bass.AP(tensor=rzb.tensor, offset=rzb.offset,
                                      ap=[rzb.ap[0], rzb.ap[1], [0, 2], [1, HD]])
                        nc.vector.tensor_mul(out=ro, in0=w, in1=rzb)
                        # elementwise drift, DT-scaled: dd = DT*f_local + ro
                        sq = scr.tile([128, 4, 128], BF16, tag="sq")
                        nc.scalar.activation(out=sq, in_=xin[:, gs, :],
                                             func=AF.Square, scale=1.0)
                        r2h = scr.tile([128, 4, HD], BF16, tag="r2h")
                        nc.vector.tensor_add(out=r2h, in0=sq[:, :, 0:HD],
                                             in1=sq[:, :, HD:128])
                        mtl = scr.tile([128, 4, HD], BF16, tag="mtl")
                        nc.vector.tensor_scalar(out=mtl, in0=r2h,
                                                scalar1=-DT * INVK * INVK,
                                                scalar2=DT * CC1,
                                                op0=ALU.mult, op1=ALU.add)
                        u = scr.tile([128, 4, 128], BF16, tag="u")
                        nc.vector.tensor_mul(out=u, in0=bc2_ap(mtl),
                                             in1=xin[:, gs, :])
                        cross = scr.tile([128, 4, 128], BF16, tag="cross")
                        nc.gpsimd.tensor_mul(out=cross, in0=row_ap(omdt),
                                             in1=swap_ap(xin, gs))
                        nc.vector.tensor_add(out=u, in0=u, in1=cross)
                        dd = u
                        nc.vector.tensor_add(out=dd, in0=u, in1=ro)
                        if odd:
                            nc.vector.tensor_add(out=Xp[:, gs, :],
                                                 in0=X[:, gs, :], in1=dd)
                            nc.gpsimd.tensor_add(out=tsum[:, gs, :],
                                                 in0=Xp[:, gs, :], in1=X[:, gs, :])
                        else:
                            # X' = 0.5*(Xp + X + dd2)
                            nc.vector.tensor_add(out=dd, in0=tsum[:, gs, :],
                                                 in1=dd)
                            nc.vector.tensor_scalar_mul(out=X[:, gs, :], in0=dd,
                                                        scalar1=0.5)
                        if p >= 2:
                            for _ in range(min(per_slot, len(nextq))):
                                nextq.pop(0)()
                for f in nextq:
                    f()
                nextq = []

                # --- readout: mixed -> @Wo -> xattn cols ---
                nc.vector.tensor_scalar_mul(out=attnv, in0=attnv, scalar1=MIX)
                nc.vector.scalar_tensor_tensor(out=attnv, in0=X[:, :, 0:HD],
                                               scalar=M2, in1=attnv,
                                               op0=ALU.mult, op1=ALU.add)
                nc.vector.scalar_tensor_tensor(out=attnv, in0=X[:, :, HD:128],
                                               scalar=-M3, in1=attnv,
                                               op0=ALU.mult, op1=ALU.add)
                mixv = attnv
                xatth = mts.tile([128, NMB, HD], BF16, tag="xatth",
                                 name=f"xatth_{h}")
                for t in range(NMB):
                    pt = pmisc.tile([64, 128], BF16, tag="pm")
                    nc.tensor.transpose(pt, mixv[:, t, :], identb)
                    mt = mts.tile([64, 128], BF16, tag="mt")
                    nc.scalar.copy(out=mt, in_=pt)
                    po = pmisc.tile([128, HD], F32, tag="pm")
                    nc.tensor.matmul(po, mt, wo_bf[:, h, :],
                                     start=True, stop=True)
                    nc.scalar.copy(out=xatth[:, t, :], in_=po)

                # --- stage this head's slice of cc_in (masked) ---
                for j in range(N_CORES):
                    t0 = (j % 4) * TT4
                    stg = mts.tile([128, TT4, HD], BF16, tag="stg",
                                   name=f"stg_{h}_{j}")
                    nc.vector.tensor_scalar_mul(
                        out=stg,
                        in0=xatth[:, t0:t0 + TT4, :],
                        scalar1=maskbc[:, j:j + 1])
                    base = cc_in[0, j * 128:(j + 1) * 128, :]
                    dst = bass.AP(tensor=base.tensor,
                                  offset=base.offset + h * HD,
                                  ap=[[HC, 128], [N_CORES * 128 * HC, TT4],
                                      [1, HD]])
                    nc.sync.dma_start(out=dst, in_=stg)

    # =================== AllToAll (per-tt) + FFN ===================
    with tc.tile_pool(name="ffw", bufs=1) as ffw, \
         tc.tile_pool(name="ffa", bufs=3) as ffa, \
         tc.tile_pool(name="ffs", bufs=4) as ffs, \
         tc.tile_pool(name="w1p", bufs=4) as w1p, \
         tc.tile_pool(name="w2p", bufs=3) as w2p, \
         tc.tile_pool(name="psf", bufs=2, space="PSUM") as psfp, \
         tc.tile_pool(name="pso", bufs=1, space="PSUM") as psop, \
         tc.tile_pool(name="pstf", bufs=2, space="PSUM") as pstf:

        xtk_all = ffw.tile([128, TT4, D], F32)
        for tt in range(TT4):
            nc.sync.dma_start(out=xtk_all[:, tt, :],
                              in_=x_tok[tt * 128:(tt + 1) * 128, :])
        for tt in range(TT4):
            if fake_cc:
                nc.sync.dma_start(out=cc_out[tt, :, :], in_=cc_in[tt, :, :])
            else:
                nc.gpsimd.collective_compute(
                    "AllToAll", ALU.bypass,
                    replica_groups=[list(range(N_CORES))],
                    ins=[cc_in[tt, :, :].opt()],
                    outs=[cc_out[tt, :, :].opt()])

        g2bc = ffw.tile([128, D], F32)
        nc.sync.dma_start(out=g2bc, in_=g2_d[None, :].to_broadcast([128, D]))
        be2bc = ffw.tile([128, D], F32)
        nc.sync.dma_start(out=be2bc, in_=be2_d[None, :].to_broadcast([128, D]))
        bf2bc = ffw.tile([128, D], F32)
        nc.sync.dma_start(out=bf2bc, in_=bf2_d[None, :].to_broadcast([128, D]))
        bf1sb = ffw.tile([128, DFF // 128], F32)
        nc.sync.dma_start(out=bf1sb, in_=bf1_d.rearrange("(f p) -> p f", p=128))
        bf1h = ffw.tile([128, DFF // 128], F32)
        nc.scalar.activation(out=bf1h, in_=bf1sb, func=AF.Copy, scale=0.5)
        x1_all = ffw.tile([128, TT4, D], F32)
        xn1T = ffw.tile([128, D // 128, TOK], BF16)
        hT = ffw.tile([128, DFF // 128, TOK], BF16)

        cc_a = ffw.tile([128, TT4, D], BF16)
        cc_b = ffw.tile([128, TT4, D], BF16)
        for tt in range(TT4):
            for half, dstt in ((0, cc_a), (1, cc_b)):
                srcb = cc_out[tt, half * 4 * 128:(half * 4 + 4) * 128, :]
                srca = bass.AP(tensor=srcb.tensor, offset=srcb.offset,
                               ap=[[HC, 128], [128 * HC, 4], [1, HC]])
                nc.sync.dma_start(out=dstt[:, tt, :], in_=srca)
        for tt in range(TT4):
            xa = ffa.tile([128, D], BF16, tag="xa")
            nc.vector.tensor_add(out=xa, in0=cc_a[:, tt, :], in1=cc_b[:, tt, :])
            nc.gpsimd.tensor_add(out=x1_all[:, tt, :], in0=xtk_all[:, tt, :],
                                 in1=xa)
            # LN2
            st = ffs.tile([128, 2, 6], F32, tag="st")
            for sg in range(2):
                nc.vector.bn_stats(out=st[:, sg, :],
                                   in_=x1_all[:, tt, sg * 512:(sg + 1) * 512])
            mv = ffs.tile([128, 2], F32, tag="mv")
            nc.vector.bn_aggr(out=mv, in_=st)
            rstd = ffs.tile([128, 1], F32, tag="rstd")
            nc.scalar.activation(out=rstd, in_=mv[:, 1:2], func=AF.Sqrt,
                                 bias=epsT, scale=1.0)
            nc.vector.reciprocal(out=rstd, in_=rstd)
            xn1 = ffa.tile([128, D], F32, tag="xn1")
            nc.vector.tensor_scalar(out=xn1, in0=x1_all[:, tt, :],
                                    scalar1=mv[:, 0:1], scalar2=rstd,
                                    op0=ALU.subtract, op1=ALU.mult)
            nc.vector.tensor_mul(out=xn1, in0=xn1, in1=g2bc)
            nc.gpsimd.tensor_add(out=xn1, in0=xn1, in1=be2bc)
            for dd in range(D // 128):
                pt = pstf.tile([128, 128], F32, tag="pt")
                nc.tensor.transpose(pt, xn1[:, dd * 128:(dd + 1) * 128], ident)
                nc.scalar.copy(out=xn1T[:, dd, tt * 128:(tt + 1) * 128], in_=pt)

        # h^T = gelu(W1^T @ xn1^T + bf1)
        for f in range(DFF // 128):
            w1b = w1p.tile([128, D // 128, 128], BF16, tag="w1b")
            nc.sync.dma_start(
                out=w1b,
                in_=w1_d.rearrange("(dd p) ff -> p dd ff",
                                   p=128)[:, :, f * 128:(f + 1) * 128])
            ph = psfp.tile([128, TOK], F32, tag="ph")
            for dd in range(D // 128):
                nc.tensor.matmul(ph, w1b[:, dd, :], xn1T[:, dd, :],
                                 start=(dd == 0), stop=(dd == D // 128 - 1))
            # gelu (tanh approx), computed on y = x/2:
            #   gelu(x) = y*(1+tanh(y*(2*c0 + 8*c3*y^2))), c0=sqrt(2/pi), c3=0.044715*c0
            gy = ffa.tile([128, TOK], F32, tag="gy")
            nc.scalar.activation(out=gy, in_=ph, func=AF.Identity, scale=0.5,
                                 bias=bf1h[:, f:f + 1])
            gt = ffa.tile([128, TOK], F32, tag="gt")
            nc.scalar.activation(out=gt, in_=gy, func=AF.Square, scale=1.0)
            nc.vector.tensor_scalar(out=gt, in0=gt, scalar1=8 * 0.044715 * GC0,
                                    scalar2=2 * GC0, op0=ALU.mult, op1=ALU.add)
            nc.vector.tensor_mul(out=gt, in0=gt, in1=gy)
            nc.scalar.activation(out=gt, in_=gt, func=AF.Tanh, scale=1.0)
            nc.vector.scalar_tensor_tensor(out=hT[:, f, :], in0=gt, scalar=1.0,
                                           in1=gy, op0=ALU.add, op1=ALU.mult)

        # out = x1 + h @ W2 + bf2   (W2 streamed, bf16)
        for dh in range(D // 512):
            pos = [psop.tile([128, 512], F32, tag=f"po{tt}", name=f"po{tt}") for tt in range(TT4)]
            for f in range(DFF // 128):
                w2b = w2p.tile([128, 512], BF16, tag="w2b")
                nc.sync.dma_start(out=w2b,
                                  in_=w2_d[f * 128:(f + 1) * 128,
                                           dh * 512:(dh + 1) * 512])
                for tt in range(TT4):
                    nc.tensor.matmul(pos[tt], hT[:, f, tt * 128:(tt + 1) * 128],
                                     w2b, start=(f == 0),
                                     stop=(f == DFF // 128 - 1))
            for tt in range(TT4):
                o1 = ffa.tile([128, 512], F32, tag="o1")
                nc.vector.tensor_add(out=o1, in0=pos[tt],
                                     in1=x1_all[:, tt, dh * 512:(dh + 1) * 512])
                nc.vector.tensor_add(out=o1, in0=o1,
                                     in1=bf2bc[:, dh * 512:(dh + 1) * 512])
                nc.sync.dma_start(out=out_d[tt * 128:(tt + 1) * 128,
                                            dh * 512:(dh + 1) * 512], in_=o1)

    ctx.close()


# ======================= host-side driver =======================

def shard_inputs(inputs, S=S_FULL):
    """Build per-core in_maps from full inputs."""
    import ml_dtypes
    x = np.ascontiguousarray(inputs["x"], dtype=np.float32)
    w1b = np.ascontiguousarray(
        np.asarray(inputs["W1"], np.float32).astype(ml_dtypes.bfloat16))
    w2b = np.ascontiguousarray(
        np.asarray(inputs["W2"], np.float32).astype(ml_dtypes.bfloat16))
    TOK = S // 4
    in_maps = []
    for c in range(N_CORES):
        b = c // 4
        hg = c % 4
        hsl = slice(hg * NHL, (hg + 1) * NHL)            # global head indices
        csl = slice(hg * NHL * HD, (hg + 1) * NHL * HD)  # head cols in D
        rsl = slice(hg * TOK, (hg + 1) * TOK)            # FFN token rows
        m = {
            "x_full": x[b],
            "x_heads": x[b][:, csl],
            "x_tok": x[b][rsl, :],
            "wq": inputs["Wq"][hsl].reshape(NHL * HD, HD),
            "wk": inputs["Wk"][hsl].reshape(NHL * HD, HD),
            "wv": inputs["Wv"][hsl].reshape(NHL * HD, HD),
            "wo": inputs["Wo"][hsl].reshape(NHL * HD, HD),
            "omega": inputs["omega"][hsl],
            "g1h": inputs["g1"][csl],
            "be1h": inputs["be1"][csl],
            "g2": inputs["g2"], "be2": inputs["be2"],
            "w1b": w1b, "bf1": inputs["bf1"],
            "w2b": w2b, "bf2": inputs["bf2"],
            "gmask": np.array([1.0 if j // 4 == b else 0.0
                               for j in range(N_CORES)], dtype=np.float32),
        }
        in_maps.append({k: (v if k in ("w1b", "w2b") else
                            np.ascontiguousarray(v, dtype=np.float32))
                        for k, v in m.items()})
    return in_maps


def assemble_output(results, S=S_FULL):
    TOK = S // 4
    out = np.zeros((B, S, D), dtype=np.float32)
    for c in range(N_CORES):
        b, hg = c // 4, c % 4
        out[b, hg * TOK:(hg + 1) * TOK, :] = results[c]["out"]
    return out


_NC_CACHE = {}


def kernel(**inputs):
    from concourse.bass_utils import run_bass_kernel_spmd
    S = inputs["x"].shape[1]
    if S not in _NC_CACHE:
        _NC_CACHE[S] = build_nc(S)
    nc = _NC_CACHE[S]
    in_maps = shard_inputs(inputs, S)
    res = run_bass_kernel_spmd(nc, in_maps, core_ids=list(range(N_CORES)))
    return assemble_output(res.results, S)



# revision 38
# speedup vs baseline: 1.4187x; 1.1072x over previous
"""Trainium2 Bass kernel for MinimalResonanceLayer (8-core SPMD).

Sharding: core c handles batch b = c//4 and local heads [ (c%4)*4, (c%4)*4+4 ).
Each head's resonance recurrence runs fully on-core (E^T resident in SBUF,
bf16); the head-concat + FFN uses one 8-core AllToAll, with per-core
divergence encoded in an input mask so the program stays SPMD-uniform.

State is kept in rotated coordinates z~ = K*exp(-i*alpha)*z so the
Kuramoto-Sakaguchi rotation folds into the PSUM copy-out scalars.
"""
import math
import numpy as np

import concourse.bass as bass
import concourse.tile as tile
from concourse import bacc, mybir
from concourse.masks import make_identity

# ---- problem constants (hardcoded per contest contract) ----
B, S_FULL, D, H, HD = 2, 2048, 1024, 16, 64
DFF = 2 * D
MU, ALPHA, K_COUP, DT, STEPS, MIX = 1.0, 0.1, 3.0, 0.02, 5, 0.3
N_CORES = 8
NHL = 4  # heads per core

CA, SA = math.cos(ALPHA), math.sin(ALPHA)
C1V = K_COUP * CA            # c1
C2V = K_COUP * SA            # c2
CC1 = MU - K_COUP            # -2.0
R21 = C2V / C1V              # tan(alpha)
W1S = C1V - C2V * C2V / C1V  # pass-1 roa scale
W2S = -2.0 * C2V             # pass-1 rob scale
M2 = (1.0 - MIX) * CA / K_COUP
M3 = (1.0 - MIX) * SA / K_COUP
SCL = 1.0 / math.sqrt(HD)
INVK = 1.0 / K_COUP
GC0 = math.sqrt(2.0 / math.pi)

F32 = mybir.dt.float32
F32R = mybir.dt.float32r
BF16 = mybir.dt.bfloat16
FP8 = mybir.dt.float8e4
DRPM = mybir.MatmulPerfMode.DoubleRow
EXPB = -3.7  # exp offset: e^(8.97+eps-3.7) < 240 = fp8e4 max finite (cancels via Z)
ALU = mybir.AluOpType
AF = mybir.ActivationFunctionType


def r(ap):
    """bitcast fp32 AP -> float32r for full-rate PE streaming."""
    return ap.bitcast(F32R)


def build_nc(S=S_FULL, fake_cc=False):
    """Build the 8-core SPMD program. S = sequence length (per batch).

    fake_cc=True replaces the AllToAll with a local DMA so the program is
    collective-free (for TimelineSim cost modeling only).
    """
    nc = bacc.Bacc("TRN2", target_bir_lowering=False, debug=False,
                   num_devices=N_CORES)

    def din(name, shape):
        return nc.dram_tensor(name, shape, F32, kind="ExternalInput").ap()

    TOK = S // 4
    io = dict(
        x_full=din("x_full", [S, D]),
        x_heads=din("x_heads", [S, NHL * HD]),
        x_tok=din("x_tok", [TOK, D]),
        wq_d=din("wq", [NHL * HD, HD]),
        wk_d=din("wk", [NHL * HD, HD]),
        wv_d=din("wv", [NHL * HD, HD]),
        wo_d=din("wo", [NHL * HD, HD]),
        om_d=din("omega", [NHL, HD]),
        g1_d=din("g1h", [NHL * HD]),
        be1_d=din("be1h", [NHL * HD]),
        g2_d=din("g2", [D]),
        be2_d=din("be2", [D]),
        w1_d=nc.dram_tensor("w1b", [D, DFF], BF16,
                            kind="ExternalInput").ap(),
        bf1_d=din("bf1", [DFF]),
        w2_d=nc.dram_tensor("w2b", [DFF, D], BF16,
                            kind="ExternalInput").ap(),
        bf2_d=din("bf2", [D]),
        gm_d=din("gmask", [N_CORES]),
        out_d=nc.dram_tensor("out", [TOK, D], F32, kind="ExternalOutput").ap(),
    )

    with tile.TileContext(nc) as tc:
        _body(nc, tc, io, S, fake_cc)

    nc.compile()
    return nc


def _body(nc, tc, io, S, fake_cc=False):
    NMB = S // 128          # token blocks of 128
    NG = NMB // 4           # groups of 4 blocks (512 tokens)
    TOK = S // 4            # FFN tokens per core (= B*S/8)
    TT4 = TOK // 128        # FFN token tiles
    NSL = S // 512          # 512-wide column slices of S
    HC = NHL * HD           # 256 head cols per core

    x_full, x_heads, x_tok = io["x_full"], io["x_heads"], io["x_tok"]
    wq_d, wk_d, wv_d, wo_d = io["wq_d"], io["wk_d"], io["wv_d"], io["wo_d"]
    om_d, g1_d, be1_d = io["om_d"], io["g1_d"], io["be1_d"]
    g2_d, be2_d = io["g2_d"], io["be2_d"]
    w1_d, bf1_d, w2_d, bf2_d = io["w1_d"], io["bf1_d"], io["w2_d"], io["bf2_d"]
    gm_d, out_d = io["gm_d"], io["out_d"]

    from contextlib import ExitStack
    ctx = ExitStack()
    sing = ctx.enter_context(tc.tile_pool(name="sing", bufs=1))
    dram = ctx.enter_context(tc.tile_pool(name="dram", bufs=1, space="DRAM"))

    # ---- whole-kernel constants ----
    ident = sing.tile([128, 128], F32)
    make_identity(nc, ident)
    identb = sing.tile([128, 128], BF16)
    nc.vector.tensor_copy(out=identb, in_=ident)
    epsT = sing.tile([128, 1], F32)
    nc.vector.memset(epsT, 1e-5)
    expbT = sing.tile([128, 1], F32)
    nc.vector.memset(expbT, EXPB)
    maskbc = sing.tile([128, N_CORES], F32)
    nc.sync.dma_start(out=maskbc, in_=gm_d[None, :].to_broadcast([128, N_CORES]))
    # signed rotation coefficient row: [+R21 | -R21] (for s1 = R21v * wswap)
    R21v = sing.tile([128, 128], BF16)
    nc.vector.memset(R21v[:, 0:HD], R21)
    nc.vector.memset(R21v[:, HD:128], -R21)

    xtk_all = sing.tile([128, TT4, D], F32)
    w1sb = sing.tile([128, DFF // 128, D // 128, 128], BF16)

    def issue_ffn_prefetch():
        for tt in range(TT4):
            nc.sync.dma_start(out=xtk_all[:, tt, :],
                              in_=x_tok[tt * 128:(tt + 1) * 128, :])
        for f in range(DFF // 128):
            nc.sync.dma_start(
                out=w1sb[:, f, :, :],
                in_=w1_d.rearrange("(dd p) ff -> p dd ff",
                                   p=128)[:, :, f * 128:(f + 1) * 128])
    cc_in = dram.tile([TOK // 128, N_CORES * 128, HC], BF16)
    cc_out = dram.tile([TOK // 128, N_CORES * 128, HC], BF16)

    # =================== attention super-phase ===================
    with ExitStack() as actx:
        big = actx.enter_context(tc.tile_pool(name="big", bufs=1))
        with tc.tile_pool(name="gstg", bufs=1) as gstg:
            g1f = gstg.tile([128, HC], F32)
            nc.sync.dma_start(out=g1f, in_=g1_d[None, :].to_broadcast([128, HC]))
            be1f = gstg.tile([128, HC], F32)
            nc.sync.dma_start(out=be1f,
                              in_=be1_d[None, :].to_broadcast([128, HC]))
            g1bc = big.tile([128, HC], BF16)
            nc.vector.tensor_copy(out=g1bc, in_=g1f)
            be1bc = big.tile([128, HC], BF16)
            nc.vector.tensor_copy(out=be1bc, in_=be1f)
        wq_sb = big.tile([64, NHL, HD], BF16)
        wk_sb = big.tile([64, NHL, HD], BF16)
        wv_sb = big.tile([64, NHL, HD], BF16)
        wo_bf = big.tile([64, NHL, HD], BF16)
        with tc.tile_pool(name="wstg", bufs=1) as wstg:
            wstage = wstg.tile([64, 4, NHL, HD], F32)
            for wi, wd in enumerate([wq_d, wk_d, wv_d, wo_d]):
                nc.sync.dma_start(out=wstage[:, wi, :, :],
                                  in_=wd.rearrange("(h p) e -> p h e", p=HD))
            nc.vector.tensor_copy(out=wq_sb, in_=wstage[:, 0, :, :])
            nc.vector.tensor_copy(out=wk_sb, in_=wstage[:, 1, :, :])
            nc.vector.tensor_copy(out=wv_sb, in_=wstage[:, 2, :, :])
            nc.vector.tensor_copy(out=wo_bf, in_=wstage[:, 3, :, :])

        xnh = big.tile([128, NMB, HC], BF16)
        X = big.tile([128, NMB, 128], BF16)     # state [XA | XB], bf16
        X8 = big.tile([128, NMB, 128], FP8)     # fp8(rot(X)) matmul rhs
        Xp8 = big.tile([128, NMB, 128], FP8)    # fp8(rot(Xp))
        vb18 = big.tile([128, NMB, 128], FP8)   # fp8 [v | ones] pass-1 rhs
        Xp = big.tile([128, NMB, 128], BF16)    # Heun predictor
        tsum = big.tile([128, NMB, 128], BF16)  # Xp + X (for corrector)
        vb1 = big.tile([128, NMB, 128], BF16)   # [v | ones] pass-1 rhs
        attnv = big.tile([128, NMB, HD], BF16)  # A @ v
        rz2dt = big.tile([128, NMB, HD], BF16)   # DT*c1/Z (bcast to halves)
        om_st = big.tile([128, HD], F32)
        omdt = big.tile([128, 128], BF16)       # [-DT*omega | +DT*omega]
        nc.vector.memset(vb1[:, :, HD:128], 1.0)
        nc.vector.memset(vb18[:, :, HD:128], 1.0)

        def swap_ap(t, gs):
            """halves-swapped view of t[:, gs, :]: [...,[XB|XA],...]"""
            base = t[:, gs, :]
            return bass.AP(tensor=base.tensor, offset=base.offset + HD,
                           ap=[base.ap[0], base.ap[1], [-HD, 2], [1, HD]])

        def bc2_ap(t):
            """[128, g, 64] -> [128, g, 2, 64] broadcast of the half dim"""
            return bass.AP(tensor=t.tensor, offset=t.offset,
                           ap=[t.ap[0], t.ap[1], [0, 2], [1, HD]])

        def row_ap(t, g=4):
            """[128, 128] const row -> [128, g, 128] group-broadcast"""
            return bass.AP(tensor=t.tensor, offset=t.offset,
                           ap=[t.ap[0], [0, g], [1, 128]])

        # ---------------- LN1 ----------------
        with tc.tile_pool(name="ln", bufs=4) as ln, \
             tc.tile_pool(name="lns", bufs=6) as lns:
            for t in range(NMB):
                xt = ln.tile([128, D], F32, tag="xt")
                nc.sync.dma_start(out=xt, in_=x_full[t * 128:(t + 1) * 128, :])
                xh = ln.tile([128, HC], F32, tag="xh")
                nc.sync.dma_start(out=xh, in_=x_heads[t * 128:(t + 1) * 128, :])
                st = lns.tile([128, 2, 6], F32, tag="st")
                for sg in range(2):
                    nc.vector.bn_stats(out=st[:, sg, :],
                                       in_=xt[:, sg * 512:(sg + 1) * 512])
                mv = lns.tile([128, 2], F32, tag="mv")
                nc.vector.bn_aggr(out=mv, in_=st)
                rstd = lns.tile([128, 1], F32, tag="rstd")
                nc.scalar.activation(out=rstd, in_=mv[:, 1:2], func=AF.Sqrt,
                                     bias=epsT, scale=1.0)
                nc.vector.reciprocal(out=rstd, in_=rstd)
                nb = lns.tile([128, 1], F32, tag="nb")
                nc.vector.tensor_scalar(out=nb, in0=mv[:, 0:1], scalar1=rstd,
                                        scalar2=-1.0, op0=ALU.mult, op1=ALU.mult)
                xs = lns.tile([128, HC], F32, tag="xs")
                nc.gpsimd.tensor_scalar(out=xs, in0=xh, scalar1=rstd,
                                        scalar2=nb, op0=ALU.mult, op1=ALU.add)
                nc.vector.tensor_mul(out=xs, in0=xs, in1=g1bc)
                nc.gpsimd.tensor_add(out=xnh[:, t, :], in0=xs, in1=be1bc)

        # ---------------- per-head resonance ----------------
        with tc.tile_pool(name="xhTp", bufs=2) as xhTp, \
             tc.tile_pool(name="etp", bufs=2) as etp, \
             tc.tile_pool(name="qkp", bufs=2) as qkp, \
             tc.tile_pool(name="pmisc", bufs=2, space="PSUM") as pmisc, \
             tc.tile_pool(name="pssc", bufs=2, space="PSUM") as psscp, \
             tc.tile_pool(name="psg", bufs=4, space="PSUM") as psgp, \
             tc.tile_pool(name="scr", bufs=2) as scr, \
             tc.tile_pool(name="mts", bufs=2) as mts:

            def emit_prep(h, ET, qT, kT, xhT):
                """Per-head prep as thunks: xhT transposes, q/k proj,
                scores+exp, then v-proj (v-proj last: WAR on vb1 must
                land after the previous head's pass-1 reads)."""
                th = []

                def omth():
                    src = bass.AP(tensor=om_d.tensor,
                                  offset=om_d.offset + h * HD,
                                  ap=[[0, 128], [1, HD]])
                    nc.sync.dma_start(out=om_st, in_=src)
                    nc.vector.tensor_scalar_mul(out=omdt[:, 0:HD], in0=om_st,
                                                scalar1=-DT)
                    nc.vector.tensor_scalar_mul(out=omdt[:, HD:128], in0=om_st,
                                                scalar1=DT)
                th.append(omth)

                def xhTth(t):
                    pt = pmisc.tile([64, 128], BF16, tag="pm")
                    nc.tensor.transpose(pt, xnh[:, t, h * HD:(h + 1) * HD],
                                        identb)
                    nc.scalar.copy(out=xhT[:, t * 128:(t + 1) * 128], in_=pt)
                for t in range(NMB):
                    th.append(lambda t=t: xhTth(t))

                def projth(sl):
                    pq = pmisc.tile([64, 512], F32, tag="pm")
                    nc.tensor.matmul(pq, wq_sb[:, h, :],
                                     xhT[:, sl * 512:(sl + 1) * 512],
                                     start=True, stop=True)
                    nc.scalar.copy(out=qT[:, sl * 512:(sl + 1) * 512], in_=pq)
                    pk = pmisc.tile([64, 512], F32, tag="pm")
                    nc.tensor.matmul(pk, wk_sb[:, h, :],
                                     xhT[:, sl * 512:(sl + 1) * 512],
                                     start=True, stop=True)
                    nc.scalar.copy(out=kT[:, sl * 512:(sl + 1) * 512], in_=pk)
                for sl in range(NSL):
                    th.append(lambda sl=sl: projth(sl))

                def scoreth(k, sl):
                    c0 = sl * 512
                    ps = psscp.tile([128, 512], F32, tag="ps")
                    nc.tensor.matmul(ps, kT[:, k * 128:(k + 1) * 128],
                                     qT[:, c0:c0 + 512],
                                     start=True, stop=True)
                    nc.scalar.activation(out=ET[:, k, c0:c0 + 512],
                                         in_=ps, func=AF.Exp, scale=SCL,
                                         bias=expbT)
                for sl in range(NSL):
                    for k in range(NMB):
                        th.append(lambda k=k, sl=sl: scoreth(k, sl))

                def vth(t):
                    pv = pmisc.tile([128, HD], F32, tag="pm")
                    nc.tensor.matmul(pv, xhT[:, t * 128:(t + 1) * 128],
                                     wv_sb[:, h, :], start=True, stop=True)
                    nc.scalar.copy(out=vb1[:, t, 0:HD], in_=pv)
                    nc.scalar.copy(out=vb18[:, t, 0:HD], in_=pv)
                for t in range(NMB):
                    th.append(lambda t=t: vth(t))
                return th

            def hbufs(h):
                return (etp.tile([128, NMB, S], FP8, tag="ET",
                                 name=f"ET_{h}"),
                        qkp.tile([64, S], BF16, tag="qT", name=f"qT_{h}"),
                        qkp.tile([64, S], BF16, tag="kT", name=f"kT_{h}"),
                        xhTp.tile([64, S], BF16, tag="xhT", name=f"xhT_{h}"))

            issue_ffn_prefetch()
            hbuf = {0: hbufs(0)}
            for f in emit_prep(0, *hbuf[0]):
                f()
            for h in range(NHL):
                ET, qT, kT, xhT = hbuf[h]
                if h + 1 < NHL:
                    hbuf[h + 1] = hbufs(h + 1)
                    nextq = emit_prep(h + 1, *hbuf[h + 1])
                else:
                    nextq = []
                # v-proj thunks (the last 16) must land after pass-1 of head
                # h finishes reading vb1; draining starts at pass 2 so the
                # in-order PE queue never stalls on the WAR.
                nslots = (2 * STEPS - 1) * NG
                per_slot = max(1, -(-len(nextq) // nslots))

                # --- 10 Heun passes: fp8 DoubleRow matmuls, bf16 chain ---
                for p in range(1, 2 * STEPS + 1):
                    odd = (p % 2 == 1)
                    rhs8 = vb18 if p == 1 else (X8 if odd else Xp8)
                    xin = X if odd else Xp

                    for g in range(NG):
                        pg = psgp.tile([128, 4, 128], F32, tag="pg")
                        for ml in range(4):
                            mb = g * 4 + ml
                            for kp in range(NMB // 2):
                                nc.tensor.matmul(
                                    pg[:, ml, :],
                                    ET[:, 2 * kp:2 * kp + 2,
                                       mb * 128:(mb + 1) * 128],
                                    rhs8[:, 2 * kp:2 * kp + 2, :],
                                    start=(kp == 0), stop=(kp == NMB // 2 - 1),
                                    perf_mode=DRPM)
                        gs = slice(g * 4, g * 4 + 4)
                        ro = scr.tile([128, 4, 128], BF16, tag="ro")
                        if p == 1:
                            # psum = [E8@v | Z8rep]; rz2dt = DT*c1/Z8
                            w = scr.tile([128, 4, 128], BF16, tag="w")
                            nc.scalar.copy(out=w, in_=pg)
                            rcp = scr.tile([128, 4], F32, tag="rcp")
                            nc.vector.reciprocal(out=rcp, in_=pg[:, :, HD:HD + 1])
                            rcpb = scr.tile([128, 4], BF16, tag="rcpb")
                            nc.vector.tensor_copy(out=rcpb, in_=rcp)
                            rb64b = bass.AP(tensor=rcpb.tensor, offset=rcpb.offset,
                                            ap=[rcpb.ap[0], [1, 4], [0, HD]])
                            nc.vector.tensor_scalar_mul(out=rz2dt[:, gs, :],
                                                        in0=rb64b,
                                                        scalar1=DT * C1V)
                            rb64 = bass.AP(tensor=rcpb.tensor, offset=rcpb.offset,
                                           ap=[rcpb.ap[0], [1, 4], [0, HD]])
                            nc.vector.tensor_mul(out=attnv[:, gs, :],
                                                 in0=w[:, :, 0:HD], in1=rb64)
                            # state init X0 = [c1*v | -c2*v]
                            nc.vector.tensor_scalar_mul(out=X[:, gs, 0:HD],
                                                        in0=vb1[:, gs, 0:HD],
                                                        scalar1=C1V)
                            nc.vector.tensor_scalar_mul(out=X[:, gs, HD:128],
                                                        in0=vb1[:, gs, 0:HD],
                                                        scalar1=-C2V)
                            # rotated coupling via W1S/W2S, then * rz
                            nc.vector.tensor_scalar_mul(out=w[:, :, HD:128],
                                                        in0=w[:, :, 0:HD],
                                                        scalar1=W2S)
                            nc.vector.tensor_scalar_mul(out=w[:, :, 0:HD],
                                                        in0=w[:, :, 0:HD],
                                                        scalar1=W1S)
                            rzb = rz2dt[:, gs, :]
                            rzb = bass.AP(tensor=rzb.tensor, offset=rzb.offset,
                                          ap=[rzb.ap[0], rzb.ap[1], [0, 2],
                                              [1, HD]])
                            nc.vector.tensor_mul(out=ro, in0=w, in1=rzb)
                        else:
                            # rhs was pre-rotated: psum IS the rotated coupling
                            w = scr.tile([128, 4, 128], BF16, tag="w")
                            nc.scalar.copy(out=w, in_=pg)
                            rzb = rz2dt[:, gs, :]
                            rzb = bass.AP(tensor=rzb.tensor, offset=rzb.offset,
                                          ap=[rzb.ap[0], rzb.ap[1], [0, 2],
                                              [1, HD]])
                            nc.vector.tensor_mul(out=ro, in0=w, in1=rzb)
                        # elementwise drift, DT-scaled: dd = DT*f_local + ro
                        sq = scr.tile([128, 4, 128], BF16, tag="sq")
                        if h >= 2:
                            nc.scalar.activation(out=sq, in_=xin[:, gs, :],
                                                 func=AF.Square, scale=1.0)
                        else:
                            nc.gpsimd.tensor_mul(out=sq, in0=xin[:, gs, :],
                                                 in1=xin[:, gs, :])
                        r2h = scr.tile([128, 4, HD], BF16, tag="r2h")
                        nc.vector.tensor_add(out=r2h, in0=sq[:, :, 0:HD],
                                             in1=sq[:, :, HD:128])
                        mtl = scr.tile([128, 4, HD], BF16, tag="mtl")
                        nc.vector.tensor_scalar(out=mtl, in0=r2h,
                                                scalar1=-DT * INVK * INVK,
                                                scalar2=DT * CC1,
                                                op0=ALU.mult, op1=ALU.add)
                        u = scr.tile([128, 4, 128], BF16, tag="u")
                        nc.vector.tensor_mul(out=u, in0=bc2_ap(mtl),
                                             in1=xin[:, gs, :])
                        cross = scr.tile([128, 4, 128], BF16, tag="cross")
                        nc.gpsimd.tensor_mul(out=cross, in0=row_ap(omdt),
                                             in1=swap_ap(xin, gs))
                        nc.vector.tensor_add(out=u, in0=u, in1=cross)
                        dd = u
                        nc.vector.tensor_add(out=dd, in0=u, in1=ro)
                        ry = scr.tile([128, 4, 128], BF16, tag="ry")
                        if odd:
                            nc.vector.tensor_add(out=Xp[:, gs, :],
                                                 in0=X[:, gs, :], in1=dd)
                            nc.vector.tensor_add(out=tsum[:, gs, :],
                                                 in0=Xp[:, gs, :], in1=X[:, gs, :])
                            if p < 2 * STEPS:
                                # Xp8 = fp8(rot(Xp))
                                nc.vector.tensor_mul(out=ry, in0=row_ap(R21v),
                                                     in1=swap_ap(Xp, gs))
                                nc.vector.tensor_add(out=ry, in0=ry,
                                                     in1=Xp[:, gs, :])
                                if g == NG - 1:
                                    nc.vector.tensor_copy(out=Xp8[:, gs, :],
                                                          in_=ry)
                                else:
                                    nc.scalar.copy(out=Xp8[:, gs, :], in_=ry)
                        else:
                            # X' = 0.5*(Xp + X + dd2)
                            nc.vector.tensor_add(out=dd, in0=tsum[:, gs, :],
                                                 in1=dd)
                            nc.vector.tensor_scalar_mul(out=X[:, gs, :], in0=dd,
                                                        scalar1=0.5)
                            if p < 2 * STEPS:
                                # X8 = fp8(rot(X))
                                nc.gpsimd.tensor_mul(out=ry, in0=row_ap(R21v),
                                                     in1=swap_ap(X, gs))
                                nc.vector.tensor_add(out=ry, in0=ry,
                                                     in1=X[:, gs, :])
                                if g == NG - 1:
                                    nc.vector.tensor_copy(out=X8[:, gs, :],
                                                          in_=ry)
                                else:
                                    nc.scalar.copy(out=X8[:, gs, :], in_=ry)
                        if p >= 2 and g < NG - 1:
                            for _ in range(min(per_slot, len(nextq))):
                                nextq.pop(0)()
                for f in nextq:
                    f()
                nextq = []

                # --- readout: mixed -> @Wo -> xattn cols ---
                nc.vector.tensor_scalar_mul(out=attnv, in0=attnv, scalar1=MIX)
                nc.vector.scalar_tensor_tensor(out=attnv, in0=X[:, :, 0:HD],
                                               scalar=M2, in1=attnv,
                                               op0=ALU.mult, op1=ALU.add)
                nc.vector.scalar_tensor_tensor(out=attnv, in0=X[:, :, HD:128],
                                               scalar=-M3, in1=attnv,
                                               op0=ALU.mult, op1=ALU.add)
                mixv = attnv
                xatth = mts.tile([128, NMB, HD], BF16, tag="xatth",
                                 name=f"xatth_{h}")
                for t in range(NMB):
                    pt = pmisc.tile([64, 128], BF16, tag="pm")
                    nc.tensor.transpose(pt, mixv[:, t, :], identb)
                    mt = mts.tile([64, 128], BF16, tag="mt")
                    nc.scalar.copy(out=mt, in_=pt)
                    po = pmisc.tile([128, HD], F32, tag="pm")
                    nc.tensor.matmul(po, mt, wo_bf[:, h, :],
                                     start=True, stop=True)
                    nc.scalar.copy(out=xatth[:, t, :], in_=po)

                # --- stage this head's slice of cc_in (masked) ---
                for j in range(N_CORES):
                    t0 = (j % 4) * TT4
                    stg = mts.tile([128, TT4, HD], BF16, tag="stg",
                                   name=f"stg_{h}_{j}")
                    nc.vector.tensor_scalar_mul(
                        out=stg,
                        in0=xatth[:, t0:t0 + TT4, :],
                        scalar1=maskbc[:, j:j + 1])
                    base = cc_in[0, j * 128:(j + 1) * 128, :]
                    dst = bass.AP(tensor=base.tensor,
                                  offset=base.offset + h * HD,
                                  ap=[[HC, 128], [N_CORES * 128 * HC, TT4],
                                      [1, HD]])
                    nc.sync.dma_start(out=dst, in_=stg)

    # =================== AllToAll (per-tt) + FFN ===================
    with tc.tile_pool(name="ffw", bufs=1) as ffw, \
         tc.tile_pool(name="ffa", bufs=3) as ffa, \
         tc.tile_pool(name="ffs", bufs=4) as ffs, \
         tc.tile_pool(name="w1p", bufs=4) as w1p, \
         tc.tile_pool(name="w2p", bufs=3) as w2p, \
         tc.tile_pool(name="psf", bufs=2, space="PSUM") as psfp, \
         tc.tile_pool(name="pso", bufs=1, space="PSUM") as psop, \
         tc.tile_pool(name="pstf", bufs=2, space="PSUM") as pstf:

        for tt in range(TT4):
            if fake_cc:
                nc.sync.dma_start(out=cc_out[tt, :, :], in_=cc_in[tt, :, :])
            else:
                nc.gpsimd.collective_compute(
                    "AllToAll", ALU.bypass,
                    replica_groups=[list(range(N_CORES))],
                    ins=[cc_in[tt, :, :].opt()],
                    outs=[cc_out[tt, :, :].opt()])

        g2bc = ffw.tile([128, D], F32)
        nc.sync.dma_start(out=g2bc, in_=g2_d[None, :].to_broadcast([128, D]))
        be2bc = ffw.tile([128, D], F32)
        nc.sync.dma_start(out=be2bc, in_=be2_d[None, :].to_broadcast([128, D]))
        bf2bc = ffw.tile([128, D], F32)
        nc.sync.dma_start(out=bf2bc, in_=bf2_d[None, :].to_broadcast([128, D]))
        bf1sb = ffw.tile([128, DFF // 128], F32)
        nc.sync.dma_start(out=bf1sb, in_=bf1_d.rearrange("(f p) -> p f", p=128))
        bf1h = ffw.tile([128, DFF // 128], F32)
        nc.scalar.activation(out=bf1h, in_=bf1sb, func=AF.Copy, scale=0.5)
        x1_all = ffw.tile([128, TT4, D], F32)
        xn1T = ffw.tile([128, D // 128, TOK], BF16)
        hT = ffw.tile([128, DFF // 128, TOK], BF16)

        cc_a = ffw.tile([128, TT4, D], BF16)
        cc_b = ffw.tile([128, TT4, D], BF16)
        for tt in range(TT4):
            for half, dstt in ((0, cc_a), (1, cc_b)):
                srcb = cc_out[tt, half * 4 * 128:(half * 4 + 4) * 128, :]
                srca = bass.AP(tensor=srcb.tensor, offset=srcb.offset,
                               ap=[[HC, 128], [128 * HC, 4], [1, HC]])
                nc.sync.dma_start(out=dstt[:, tt, :], in_=srca)
        for tt in range(TT4):
            xa = ffa.tile([128, D], BF16, tag="xa")
            nc.vector.tensor_add(out=xa, in0=cc_a[:, tt, :], in1=cc_b[:, tt, :])
            nc.gpsimd.tensor_add(out=x1_all[:, tt, :], in0=xtk_all[:, tt, :],
                                 in1=xa)
            # LN2
            st = ffs.tile([128, 2, 6], F32, tag="st")
            for sg in range(2):
                nc.vector.bn_stats(out=st[:, sg, :],
                                   in_=x1_all[:, tt, sg * 512:(sg + 1) * 512])
            mv = ffs.tile([128, 2], F32, tag="mv")
            nc.vector.bn_aggr(out=mv, in_=st)
            rstd = ffs.tile([128, 1], F32, tag="rstd")
            nc.scalar.activation(out=rstd, in_=mv[:, 1:2], func=AF.Sqrt,
                                 bias=epsT, scale=1.0)
            nc.vector.reciprocal(out=rstd, in_=rstd)
            xn1 = ffa.tile([128, D], F32, tag="xn1")
            nc.vector.tensor_scalar(out=xn1, in0=x1_all[:, tt, :],
                                    scalar1=mv[:, 0:1], scalar2=rstd,
                                    op0=ALU.subtract, op1=ALU.mult)
            nc.vector.tensor_mul(out=xn1, in0=xn1, in1=g2bc)
            nc.gpsimd.tensor_add(out=xn1, in0=xn1, in1=be2bc)
            for dd in range(D // 128):
                pt = pstf.tile([128, 128], F32, tag="pt")
                nc.tensor.transpose(pt, xn1[:, dd * 128:(dd + 1) * 128], ident)
                nc.scalar.copy(out=xn1T[:, dd, tt * 128:(tt + 1) * 128], in_=pt)

        # h^T = gelu(W1^T @ xn1^T + bf1)
        for f in range(DFF // 128):
            ph = psfp.tile([128, TOK], F32, tag="ph")
            for dd in range(D // 128):
                nc.tensor.matmul(ph, w1sb[:, f, dd, :], xn1T[:, dd, :],
                                 start=(dd == 0), stop=(dd == D // 128 - 1))
            # gelu (tanh approx), computed on y = x/2:
            #   gelu(x) = y*(1+tanh(y*(2*c0 + 8*c3*y^2))), c0=sqrt(2/pi), c3=0.044715*c0
            gy = ffa.tile([128, TOK], F32, tag="gy")
            nc.scalar.activation(out=gy, in_=ph, func=AF.Identity, scale=0.5,
                                 bias=bf1h[:, f:f + 1])
            gt = ffa.tile([128, TOK], F32, tag="gt")
            nc.scalar.activation(out=gt, in_=gy, func=AF.Square, scale=1.0)
            nc.vector.tensor_scalar(out=gt, in0=gt, scalar1=8 * 0.044715 * GC0,
                                    scalar2=2 * GC0, op0=ALU.mult, op1=ALU.add)
            nc.vector.tensor_mul(out=gt, in0=gt, in1=gy)
            nc.scalar.activation(out=gt, in_=gt, func=AF.Tanh, scale=1.0)
            nc.vector.scalar_tensor_tensor(out=hT[:, f, :], in0=gt, scalar=1.0,
                                           in1=gy, op0=ALU.add, op1=ALU.mult)

        # out = x1 + h @ W2 + bf2   (W2 streamed, bf16)
        for dh in range(D // 512):
            pos = [psop.tile([128, 512], F32, tag=f"po{tt}", name=f"po{tt}") for tt in range(TT4)]
            for f in range(DFF // 128):
                w2b = w2p.tile([128, 512], BF16, tag="w2b")
                nc.sync.dma_start(out=w2b,
                                  in_=w2_d[f * 128:(f + 1) * 128,
                                           dh * 512:(dh + 1) * 512])
                for tt in range(TT4):
                    nc.tensor.matmul(pos[tt], hT[:, f, tt * 128:(tt + 1) * 128],
                                     w2b, start=(f == 0),
                                     stop=(f == DFF // 128 - 1))
            for tt in range(TT4):
                o1 = ffa.tile([128, 512], F32, tag="o1")
                nc.vector.tensor_add(out=o1, in0=pos[tt],
                                     in1=x1_all[:, tt, dh * 512:(dh + 1) * 512])
                nc.vector.tensor_add(out=o1, in0=o1,
                                     in1=bf2bc[:, dh * 512:(dh + 1) * 512])
                nc.sync.dma_start(out=out_d[tt * 128:(tt + 1) * 128,
                                            dh * 512:(dh + 1) * 512], in_=o1)

    ctx.close()


# ======================= host-side driver =======================

def shard_inputs(inputs, S=S_FULL):
    """Build per-core in_maps from full inputs."""
    import ml_dtypes
    x = np.ascontiguousarray(inputs["x"], dtype=np.float32)
    w1b = np.ascontiguousarray(
        np.asarray(inputs["W1"], np.float32).astype(ml_dtypes.bfloat16))
    w2b = np.ascontiguousarray(
        np.asarray(inputs["W2"], np.float32).astype(ml_dtypes.bfloat16))
    TOK = S // 4
    in_maps = []
    for c in range(N_CORES):
        b = c // 4
        hg = c % 4
        hsl = slice(hg * NHL, (hg + 1) * NHL)            # global head indices
        csl = slice(hg * NHL * HD, (hg + 1) * NHL * HD)  # head cols in D
        rsl = slice(hg * TOK, (hg + 1) * TOK)            # FFN token rows
        m = {
            "x_full": x[b],
            "x_heads": x[b][:, csl],
            "x_tok": x[b][rsl, :],
            "wq": inputs["Wq"][hsl].reshape(NHL * HD, HD),
            "wk": inputs["Wk"][hsl].reshape(NHL * HD, HD),
            "wv": inputs["Wv"][hsl].reshape(NHL * HD, HD),
            "wo": inputs["Wo"][hsl].reshape(NHL * HD, HD),
            "omega": inputs["omega"][hsl],
            "g1h": inputs["g1"][csl],
            "be1h": inputs["be1"][csl],
            "g2": inputs["g2"], "be2": inputs["be2"],
            "w1b": w1b, "bf1": inputs["bf1"],
            "w2b": w2b, "bf2": inputs["bf2"],
            "gmask": np.array([1.0 if j // 4 == b else 0.0
                               for j in range(N_CORES)], dtype=np.float32),
        }
        in_maps.append({k: (v if k in ("w1b", "w2b") else
                            np.ascontiguousarray(v, dtype=np.float32))
                        for k, v in m.items()})
    return in_maps


def assemble_output(results, S=S_FULL):
    TOK = S // 4
    out = np.zeros((B, S, D), dtype=np.float32)
    for c in range(N_CORES):
        b, hg = c // 4, c % 4
        out[b, hg * TOK:(hg + 1) * TOK, :] = results[c]["out"]
    return out


_NC_CACHE = {}


def kernel(**inputs):
    from concourse.bass_utils import run_bass_kernel_spmd
    S = inputs["x"].shape[1]
    if S not in _NC_CACHE:
        _NC_CACHE[S] = build_nc(S)
    nc = _NC_CACHE[S]
    in_maps = shard_inputs(inputs, S)
    res = run_bass_kernel_spmd(nc, in_maps, core_ids=list(range(N_CORES)))
    return assemble_output(res.results, S)



# revision 45
# speedup vs baseline: 1.4492x; 1.0215x over previous
"""Trainium2 Bass kernel for MinimalResonanceLayer (8-core SPMD).

Sharding: core c handles batch b = c//4 and local heads [ (c%4)*4, (c%4)*4+4 ).
Each head's resonance recurrence runs fully on-core (E^T resident in SBUF,
bf16); the head-concat + FFN uses one 8-core AllToAll, with per-core
divergence encoded in an input mask so the program stays SPMD-uniform.

State is kept in rotated coordinates z~ = K*exp(-i*alpha)*z so the
Kuramoto-Sakaguchi rotation folds into the PSUM copy-out scalars.
"""
import math
import numpy as np

import concourse.bass as bass
import concourse.tile as tile
from concourse import bacc, mybir
from concourse.masks import make_identity

# ---- problem constants (hardcoded per contest contract) ----
B, S_FULL, D, H, HD = 2, 2048, 1024, 16, 64
DFF = 2 * D
MU, ALPHA, K_COUP, DT, STEPS, MIX = 1.0, 0.1, 3.0, 0.02, 5, 0.3
N_CORES = 8
NHL = 4  # heads per core

CA, SA = math.cos(ALPHA), math.sin(ALPHA)
C1V = K_COUP * CA            # c1
C2V = K_COUP * SA            # c2
CC1 = MU - K_COUP            # -2.0
R21 = C2V / C1V              # tan(alpha)
W1S = C1V - C2V * C2V / C1V  # pass-1 roa scale
W2S = -2.0 * C2V             # pass-1 rob scale
M2 = (1.0 - MIX) * CA / K_COUP
M3 = (1.0 - MIX) * SA / K_COUP
SCL = 1.0 / math.sqrt(HD)
INVK = 1.0 / K_COUP
GC0 = math.sqrt(2.0 / math.pi)

F32 = mybir.dt.float32
F32R = mybir.dt.float32r
BF16 = mybir.dt.bfloat16
FP8 = mybir.dt.float8e4
DRPM = mybir.MatmulPerfMode.DoubleRow
EXPB = -3.7  # exp offset: e^(8.97+eps-3.7) < 240 = fp8e4 max finite (cancels via Z)
ALU = mybir.AluOpType
AF = mybir.ActivationFunctionType


def r(ap):
    """bitcast fp32 AP -> float32r for full-rate PE streaming."""
    return ap.bitcast(F32R)


def build_nc(S=S_FULL, fake_cc=False):
    """Build the 8-core SPMD program. S = sequence length (per batch).

    fake_cc=True replaces the AllToAll with a local DMA so the program is
    collective-free (for TimelineSim cost modeling only).
    """
    nc = bacc.Bacc("TRN2", target_bir_lowering=False, debug=False,
                   num_devices=N_CORES)

    def din(name, shape):
        return nc.dram_tensor(name, shape, F32, kind="ExternalInput").ap()

    TOK = S // 4
    io = dict(
        x_full=din("x_full", [S, D]),
        x_heads=din("x_heads", [S, NHL * HD]),
        x_tok=din("x_tok", [TOK, D]),
        wq_d=din("wq", [NHL * HD, HD]),
        wk_d=din("wk", [NHL * HD, HD]),
        wv_d=din("wv", [NHL * HD, HD]),
        wo_d=din("wo", [NHL * HD, HD]),
        om_d=din("omega", [NHL, HD]),
        g1_d=din("g1h", [NHL * HD]),
        be1_d=din("be1h", [NHL * HD]),
        g2_d=din("g2", [D]),
        be2_d=din("be2", [D]),
        w1_d=nc.dram_tensor("w1b", [D, DFF], BF16,
                            kind="ExternalInput").ap(),
        bf1_d=din("bf1", [DFF]),
        w2_d=nc.dram_tensor("w2b", [DFF, D], BF16,
                            kind="ExternalInput").ap(),
        bf2_d=din("bf2", [D]),
        gm_d=din("gmask", [N_CORES]),
        out_d=nc.dram_tensor("out", [TOK, D], F32, kind="ExternalOutput").ap(),
    )

    with tile.TileContext(nc) as tc:
        _body(nc, tc, io, S, fake_cc)

    nc.compile()
    return nc


def _body(nc, tc, io, S, fake_cc=False):
    NMB = S // 128          # token blocks of 128
    NG = NMB // 4           # groups of 4 blocks (512 tokens)
    TOK = S // 4            # FFN tokens per core (= B*S/8)
    TT4 = TOK // 128        # FFN token tiles
    NSL = S // 512          # 512-wide column slices of S
    HC = NHL * HD           # 256 head cols per core

    x_full, x_heads, x_tok = io["x_full"], io["x_heads"], io["x_tok"]
    wq_d, wk_d, wv_d, wo_d = io["wq_d"], io["wk_d"], io["wv_d"], io["wo_d"]
    om_d, g1_d, be1_d = io["om_d"], io["g1_d"], io["be1_d"]
    g2_d, be2_d = io["g2_d"], io["be2_d"]
    w1_d, bf1_d, w2_d, bf2_d = io["w1_d"], io["bf1_d"], io["w2_d"], io["bf2_d"]
    gm_d, out_d = io["gm_d"], io["out_d"]

    from contextlib import ExitStack
    ctx = ExitStack()
    sing = ctx.enter_context(tc.tile_pool(name="sing", bufs=1))
    dram = ctx.enter_context(tc.tile_pool(name="dram", bufs=1, space="DRAM"))

    # ---- whole-kernel constants ----
    ident = sing.tile([128, 128], F32)
    make_identity(nc, ident)
    identb = sing.tile([128, 128], BF16)
    nc.vector.tensor_copy(out=identb, in_=ident)
    epsT = sing.tile([128, 1], F32)
    nc.vector.memset(epsT, 1e-5)
    expbT = sing.tile([128, 1], F32)
    nc.vector.memset(expbT, EXPB)
    maskbc = sing.tile([128, N_CORES], F32)
    nc.sync.dma_start(out=maskbc, in_=gm_d[None, :].to_broadcast([128, N_CORES]))
    # signed rotation coefficient row: [+R21 | -R21] (for s1 = R21v * wswap)
    R21v = sing.tile([128, 128], BF16)
    nc.vector.memset(R21v[:, 0:HD], R21)
    nc.vector.memset(R21v[:, HD:128], -R21)

    xtk_all = sing.tile([128, TT4, D], F32)
    w1sb = sing.tile([128, DFF // 128, D // 128, 128], BF16)

    def issue_ffn_prefetch():
        for tt in range(TT4):
            nc.sync.dma_start(out=xtk_all[:, tt, :],
                              in_=x_tok[tt * 128:(tt + 1) * 128, :])
        for f in range(DFF // 128):
            nc.sync.dma_start(
                out=w1sb[:, f, :, :],
                in_=w1_d.rearrange("(dd p) ff -> p dd ff",
                                   p=128)[:, :, f * 128:(f + 1) * 128])
    cc_in = dram.tile([TOK // 128, N_CORES * 128, HC], BF16)
    cc_out = dram.tile([TOK // 128, N_CORES * 128, HC], BF16)

    # =================== attention super-phase ===================
    with ExitStack() as actx:
        big = actx.enter_context(tc.tile_pool(name="big", bufs=1))
        with tc.tile_pool(name="gstg", bufs=1) as gstg:
            g1f = gstg.tile([128, HC], F32)
            nc.sync.dma_start(out=g1f, in_=g1_d[None, :].to_broadcast([128, HC]))
            be1f = gstg.tile([128, HC], F32)
            nc.sync.dma_start(out=be1f,
                              in_=be1_d[None, :].to_broadcast([128, HC]))
            g1bc = big.tile([128, HC], BF16)
            nc.vector.tensor_copy(out=g1bc, in_=g1f)
            be1bc = big.tile([128, HC], BF16)
            nc.vector.tensor_copy(out=be1bc, in_=be1f)
        wq_sb = big.tile([64, NHL, HD], BF16)
        wk_sb = big.tile([64, NHL, HD], BF16)
        wv_sb = big.tile([64, NHL, HD], BF16)
        wo_bf = big.tile([128, NHL, HD], BF16)
        with tc.tile_pool(name="wstg", bufs=1) as wstg:
            wstage = wstg.tile([64, 4, NHL, HD], F32)
            for wi, wd in enumerate([wq_d, wk_d, wv_d, wo_d]):
                nc.sync.dma_start(out=wstage[:, wi, :, :],
                                  in_=wd.rearrange("(h p) e -> p h e", p=HD))
            nc.vector.tensor_copy(out=wq_sb, in_=wstage[:, 0, :, :])
            nc.vector.tensor_copy(out=wk_sb, in_=wstage[:, 1, :, :])
            nc.vector.tensor_copy(out=wv_sb, in_=wstage[:, 2, :, :])
            nc.vector.tensor_copy(out=wo_bf[0:64, :, :],
                                  in_=wstage[:, 3, :, :])
            nc.vector.tensor_copy(out=wo_bf[64:128, :, :],
                                  in_=wstage[:, 3, :, :])

        xnh = big.tile([128, NMB, HC], BF16)
        X = big.tile([128, NMB, 128], BF16)     # state [XA | XB], bf16
        X8 = big.tile([128, NMB, 128], FP8)     # fp8(rot(X)) matmul rhs
        Xp8 = big.tile([128, NMB, 128], FP8)    # fp8(rot(Xp))
        vb18 = big.tile([128, NMB, 128], FP8)   # fp8 [v | ones] pass-1 rhs
        Xp = big.tile([128, NMB, 128], BF16)    # Heun predictor
        tsum = big.tile([128, NMB, 128], BF16)  # Xp + X (for corrector)
        vb1 = big.tile([128, NMB, 128], BF16)   # [v | ones] pass-1 rhs
        attnv = big.tile([128, NMB, HD], BF16)  # A @ v
        rz2dt = big.tile([128, NMB, HD], BF16)   # DT*c1/Z (bcast to halves)
        om_st = big.tile([128, HD], F32)
        omdt = big.tile([128, 128], BF16)       # [-DT*omega | +DT*omega]
        nc.vector.memset(vb1[:, :, HD:128], 1.0)
        nc.vector.memset(vb18[:, :, HD:128], 1.0)

        def swap_ap(t, gs):
            """halves-swapped view of t[:, gs, :]: [...,[XB|XA],...]"""
            base = t[:, gs, :]
            return bass.AP(tensor=base.tensor, offset=base.offset + HD,
                           ap=[base.ap[0], base.ap[1], [-HD, 2], [1, HD]])

        def bc2_ap(t):
            """[128, g, 64] -> [128, g, 2, 64] broadcast of the half dim"""
            return bass.AP(tensor=t.tensor, offset=t.offset,
                           ap=[t.ap[0], t.ap[1], [0, 2], [1, HD]])

        def row_ap(t, g=4):
            """[128, 128] const row -> [128, g, 128] group-broadcast"""
            return bass.AP(tensor=t.tensor, offset=t.offset,
                           ap=[t.ap[0], [0, g], [1, 128]])

        # ---------------- LN1 ----------------
        with tc.tile_pool(name="ln", bufs=4) as ln, \
             tc.tile_pool(name="lns", bufs=6) as lns:
            for t in range(NMB):
                xt = ln.tile([128, D], F32, tag="xt")
                nc.sync.dma_start(out=xt, in_=x_full[t * 128:(t + 1) * 128, :])
                xh = ln.tile([128, HC], F32, tag="xh")
                nc.sync.dma_start(out=xh, in_=x_heads[t * 128:(t + 1) * 128, :])
                st = lns.tile([128, 2, 6], F32, tag="st")
                for sg in range(2):
                    nc.vector.bn_stats(out=st[:, sg, :],
                                       in_=xt[:, sg * 512:(sg + 1) * 512])
                mv = lns.tile([128, 2], F32, tag="mv")
                nc.vector.bn_aggr(out=mv, in_=st)
                rstd = lns.tile([128, 1], F32, tag="rstd")
                nc.scalar.activation(out=rstd, in_=mv[:, 1:2], func=AF.Sqrt,
                                     bias=epsT, scale=1.0)
                nc.vector.reciprocal(out=rstd, in_=rstd)
                nb = lns.tile([128, 1], F32, tag="nb")
                nc.vector.tensor_scalar(out=nb, in0=mv[:, 0:1], scalar1=rstd,
                                        scalar2=-1.0, op0=ALU.mult, op1=ALU.mult)
                xs = lns.tile([128, HC], F32, tag="xs")
                nc.gpsimd.tensor_scalar(out=xs, in0=xh, scalar1=rstd,
                                        scalar2=nb, op0=ALU.mult, op1=ALU.add)
                nc.vector.tensor_mul(out=xs, in0=xs, in1=g1bc)
                nc.gpsimd.tensor_add(out=xnh[:, t, :], in0=xs, in1=be1bc)

        # ---------------- per-head resonance ----------------
        with tc.tile_pool(name="xhTp", bufs=2) as xhTp, \
             tc.tile_pool(name="etp", bufs=2) as etp, \
             tc.tile_pool(name="qkp", bufs=2) as qkp, \
             tc.tile_pool(name="pmisc", bufs=2, space="PSUM") as pmisc, \
             tc.tile_pool(name="pssc", bufs=2, space="PSUM") as psscp, \
             tc.tile_pool(name="psg", bufs=2, space="PSUM") as psgp, \
             tc.tile_pool(name="scr", bufs=2) as scr, \
             tc.tile_pool(name="mts", bufs=2) as mts:

            def emit_prep(h, ET, qT, kT, xhT):
                """Per-head prep as thunks: xhT transposes, q/k proj,
                scores+exp, then v-proj (v-proj last: WAR on vb1 must
                land after the previous head's pass-1 reads)."""
                th = []

                def omth():
                    src = bass.AP(tensor=om_d.tensor,
                                  offset=om_d.offset + h * HD,
                                  ap=[[0, 128], [1, HD]])
                    nc.sync.dma_start(out=om_st, in_=src)
                    nc.vector.tensor_scalar_mul(out=omdt[:, 0:HD], in0=om_st,
                                                scalar1=-DT)
                    nc.vector.tensor_scalar_mul(out=omdt[:, HD:128], in0=om_st,
                                                scalar1=DT)
                th.append(omth)

                def xhTth(t):
                    pt = pmisc.tile([64, 128], BF16, tag="pm")
                    nc.tensor.transpose(pt, xnh[:, t, h * HD:(h + 1) * HD],
                                        identb)
                    nc.scalar.copy(out=xhT[:, t * 128:(t + 1) * 128], in_=pt)
                for t in range(NMB):
                    th.append(lambda t=t: xhTth(t))

                def projth(sl):
                    pq = pmisc.tile([64, 512], F32, tag="pm")
                    nc.tensor.matmul(pq, wq_sb[:, h, :],
                                     xhT[:, sl * 512:(sl + 1) * 512],
                                     start=True, stop=True)
                    nc.scalar.copy(out=qT[:, sl * 512:(sl + 1) * 512], in_=pq)
                    pk = pmisc.tile([64, 512], F32, tag="pm")
                    nc.tensor.matmul(pk, wk_sb[:, h, :],
                                     xhT[:, sl * 512:(sl + 1) * 512],
                                     start=True, stop=True)
                    nc.scalar.copy(out=kT[:, sl * 512:(sl + 1) * 512], in_=pk)
                for sl in range(NSL):
                    th.append(lambda sl=sl: projth(sl))

                def scoreth(k, sl2):
                    c0 = sl2 * 1024
                    ps = psscp.tile([128, 1024], F32, tag="ps")
                    for j in range(2):
                        nc.tensor.matmul(ps[:, j * 512:(j + 1) * 512],
                                         kT[:, k * 128:(k + 1) * 128],
                                         qT[:, c0 + j * 512:c0 + (j + 1) * 512],
                                         start=True, stop=True)
                    nc.scalar.activation(out=ET[:, k, c0:c0 + 1024],
                                         in_=ps, func=AF.Exp, scale=SCL,
                                         bias=expbT)
                for sl2 in range(NSL // 2):
                    for k in range(NMB):
                        th.append(lambda k=k, sl2=sl2: scoreth(k, sl2))

                def vth(t):
                    pv = pmisc.tile([128, HD], F32, tag="pm")
                    nc.tensor.matmul(pv, xhT[:, t * 128:(t + 1) * 128],
                                     wv_sb[:, h, :], start=True, stop=True)
                    nc.scalar.copy(out=vb1[:, t, 0:HD], in_=pv)
                    nc.scalar.copy(out=vb18[:, t, 0:HD], in_=pv)
                for t in range(NMB):
                    th.append(lambda t=t: vth(t))
                return th

            def hbufs(h):
                return (etp.tile([128, NMB, S], FP8, tag="ET",
                                 name=f"ET_{h}"),
                        qkp.tile([64, S], BF16, tag="qT", name=f"qT_{h}"),
                        qkp.tile([64, S], BF16, tag="kT", name=f"kT_{h}"),
                        xhTp.tile([64, S], BF16, tag="xhT", name=f"xhT_{h}"))

            hbuf = {0: hbufs(0)}
            for f in emit_prep(0, *hbuf[0]):
                f()
            for h in range(NHL):
                ET, qT, kT, xhT = hbuf[h]
                if h + 1 < NHL:
                    hbuf[h + 1] = hbufs(h + 1)
                    nextq = emit_prep(h + 1, *hbuf[h + 1])
                else:
                    nextq = []
                if h == 0:
                    issue_ffn_prefetch()
                # v-proj thunks (the last 16) must land after pass-1 of head
                # h finishes reading vb1; draining starts at pass 2 so the
                # in-order PE queue never stalls on the WAR.
                nslots = (2 * STEPS - 1) * NG
                per_slot = max(1, -(-len(nextq) // nslots))

                # --- 10 Heun passes: fp8 DoubleRow matmuls, bf16 chain ---
                for p in range(1, 2 * STEPS + 1):
                    odd = (p % 2 == 1)
                    rhs8 = vb18 if p == 1 else (X8 if odd else Xp8)
                    xin = X if odd else Xp

                    for g in range(NG):
                        pg = psgp.tile([128, 4, 128], F32, tag="pg")
                        for ml in range(4):
                            mb = g * 4 + ml
                            for kp in range(NMB // 2):
                                nc.tensor.matmul(
                                    pg[:, ml, :],
                                    ET[:, 2 * kp:2 * kp + 2,
                                       mb * 128:(mb + 1) * 128],
                                    rhs8[:, 2 * kp:2 * kp + 2, :],
                                    start=(kp == 0), stop=(kp == NMB // 2 - 1),
                                    perf_mode=DRPM)
                        gs = slice(g * 4, g * 4 + 4)
                        ro = scr.tile([128, 4, 128], BF16, tag="ro")
                        if p == 1:
                            # psum = [E8@v | Z8rep]; rz2dt = DT*c1/Z8
                            w = scr.tile([128, 4, 128], BF16, tag="w")
                            nc.scalar.copy(out=w, in_=pg)
                            rcp = scr.tile([128, 4], F32, tag="rcp")
                            nc.vector.reciprocal(out=rcp, in_=pg[:, :, HD:HD + 1])
                            rcpb = scr.tile([128, 4], BF16, tag="rcpb")
                            nc.vector.tensor_copy(out=rcpb, in_=rcp)
                            rb64b = bass.AP(tensor=rcpb.tensor, offset=rcpb.offset,
                                            ap=[rcpb.ap[0], [1, 4], [0, HD]])
                            nc.vector.tensor_scalar_mul(out=rz2dt[:, gs, :],
                                                        in0=rb64b,
                                                        scalar1=DT * C1V)
                            rb64 = bass.AP(tensor=rcpb.tensor, offset=rcpb.offset,
                                           ap=[rcpb.ap[0], [1, 4], [0, HD]])
                            nc.vector.tensor_mul(out=attnv[:, gs, :],
                                                 in0=w[:, :, 0:HD], in1=rb64)
                            # state init X0 = [c1*v | -c2*v]
                            nc.vector.tensor_scalar_mul(out=X[:, gs, 0:HD],
                                                        in0=vb1[:, gs, 0:HD],
                                                        scalar1=C1V)
                            nc.vector.tensor_scalar_mul(out=X[:, gs, HD:128],
                                                        in0=vb1[:, gs, 0:HD],
                                                        scalar1=-C2V)
                            # rotated coupling via W1S/W2S, then * rz
                            nc.vector.tensor_scalar_mul(out=w[:, :, HD:128],
                                                        in0=w[:, :, 0:HD],
                                                        scalar1=W2S)
                            nc.vector.tensor_scalar_mul(out=w[:, :, 0:HD],
                                                        in0=w[:, :, 0:HD],
                                                        scalar1=W1S)
                            rzb = rz2dt[:, gs, :]
                            rzb = bass.AP(tensor=rzb.tensor, offset=rzb.offset,
                                          ap=[rzb.ap[0], rzb.ap[1], [0, 2],
                                              [1, HD]])
                            nc.vector.tensor_mul(out=ro, in0=w, in1=rzb)
                        else:
                            # rhs was pre-rotated: psum IS the rotated coupling
                            rzb = rz2dt[:, gs, :]
                            rzb = bass.AP(tensor=rzb.tensor, offset=rzb.offset,
                                          ap=[rzb.ap[0], rzb.ap[1], [0, 2],
                                              [1, HD]])
                            if g == NG - 1:
                                # barrier group: single fused PSUM read on DVE
                                nc.vector.tensor_mul(out=ro, in0=pg, in1=rzb)
                            else:
                                w = scr.tile([128, 4, 128], BF16, tag="w")
                                nc.scalar.copy(out=w, in_=pg)
                                nc.vector.tensor_mul(out=ro, in0=w, in1=rzb)
                        # elementwise drift, DT-scaled: dd = DT*f_local + ro
                        sq = scr.tile([128, 4, 128], BF16, tag="sq")
                        if h >= 2:
                            nc.scalar.activation(out=sq, in_=xin[:, gs, :],
                                                 func=AF.Square, scale=1.0)
                        else:
                            nc.gpsimd.tensor_mul(out=sq, in0=xin[:, gs, :],
                                                 in1=xin[:, gs, :])
                        r2h = scr.tile([128, 4, HD], BF16, tag="r2h")
                        nc.vector.tensor_add(out=r2h, in0=sq[:, :, 0:HD],
                                             in1=sq[:, :, HD:128])
                        mtl = scr.tile([128, 4, HD], BF16, tag="mtl")
                        nc.vector.tensor_scalar(out=mtl, in0=r2h,
                                                scalar1=-DT * INVK * INVK,
                                                scalar2=DT * CC1,
                                                op0=ALU.mult, op1=ALU.add)
                        u = scr.tile([128, 4, 128], BF16, tag="u")
                        nc.vector.tensor_mul(out=u, in0=bc2_ap(mtl),
                                             in1=xin[:, gs, :])
                        cross = scr.tile([128, 4, 128], BF16, tag="cross")
                        nc.gpsimd.tensor_mul(out=cross, in0=row_ap(omdt),
                                             in1=swap_ap(xin, gs))
                        nc.vector.tensor_add(out=u, in0=u, in1=cross)
                        dd = u
                        nc.vector.tensor_add(out=dd, in0=u, in1=ro)
                        ry = scr.tile([128, 4, 128], BF16, tag="ry")
                        if odd:
                            nc.vector.tensor_add(out=Xp[:, gs, :],
                                                 in0=X[:, gs, :], in1=dd)
                            nc.vector.tensor_add(out=tsum[:, gs, :],
                                                 in0=Xp[:, gs, :], in1=X[:, gs, :])
                            if p < 2 * STEPS:
                                # Xp8 = fp8(rot(Xp))
                                nc.vector.tensor_mul(out=ry, in0=row_ap(R21v),
                                                     in1=swap_ap(Xp, gs))
                                nc.vector.tensor_add(out=ry, in0=ry,
                                                     in1=Xp[:, gs, :])
                                if g == NG - 1:
                                    nc.vector.tensor_copy(out=Xp8[:, gs, :],
                                                          in_=ry)
                                else:
                                    nc.scalar.copy(out=Xp8[:, gs, :], in_=ry)
                        else:
                            # X' = 0.5*(Xp + X + dd2)
                            nc.vector.tensor_add(out=dd, in0=tsum[:, gs, :],
                                                 in1=dd)
                            nc.vector.tensor_scalar_mul(out=X[:, gs, :], in0=dd,
                                                        scalar1=0.5)
                            if p < 2 * STEPS:
                                # X8 = fp8(rot(X))
                                nc.gpsimd.tensor_mul(out=ry, in0=row_ap(R21v),
                                                     in1=swap_ap(X, gs))
                                nc.vector.tensor_add(out=ry, in0=ry,
                                                     in1=X[:, gs, :])
                                if g == NG - 1:
                                    nc.vector.tensor_copy(out=X8[:, gs, :],
                                                          in_=ry)
                                else:
                                    nc.scalar.copy(out=X8[:, gs, :], in_=ry)
                        if p >= 2 and g < NG - 1:
                            for _ in range(min(per_slot, len(nextq))):
                                nextq.pop(0)()
                for f in nextq:
                    f()
                nextq = []

                # --- readout: mixed -> @Wo -> xattn cols ---
                nc.vector.tensor_scalar_mul(out=attnv, in0=attnv, scalar1=MIX)
                nc.vector.scalar_tensor_tensor(out=attnv, in0=X[:, :, 0:HD],
                                               scalar=M2, in1=attnv,
                                               op0=ALU.mult, op1=ALU.add)
                nc.vector.scalar_tensor_tensor(out=attnv, in0=X[:, :, HD:128],
                                               scalar=-M3, in1=attnv,
                                               op0=ALU.mult, op1=ALU.add)
                mixv = attnv
                xatth = mts.tile([128, NMB, HD], BF16, tag="xatth",
                                 name=f"xatth_{h}")
                for t2 in range(NMB // 2):
                    pt = pmisc.tile([128, 128], BF16, tag="pm")
                    nc.tensor.transpose(pt, mixv[:, 2 * t2:2 * t2 + 2, :],
                                        identb)
                    mt = mts.tile([128, 128], BF16, tag="mt")
                    nc.scalar.copy(out=mt, in_=pt)
                    for j in range(2):
                        po = pmisc.tile([128, HD], F32, tag="pm")
                        nc.tensor.matmul(po, mt[j * 64:(j + 1) * 64, :],
                                         wo_bf[j * 64:(j + 1) * 64, h, :],
                                         start=True, stop=True)
                        nc.scalar.copy(out=xatth[:, 2 * t2 + j, :], in_=po)

                # --- stage this head's slice of cc_in (masked) ---
                for j in range(N_CORES):
                    t0 = (j % 4) * TT4
                    stg = mts.tile([128, TT4, HD], BF16, tag="stg",
                                   name=f"stg_{h}_{j}")
                    nc.vector.tensor_scalar_mul(
                        out=stg,
                        in0=xatth[:, t0:t0 + TT4, :],
                        scalar1=maskbc[:, j:j + 1])
                    base = cc_in[0, j * 128:(j + 1) * 128, :]
                    dst = bass.AP(tensor=base.tensor,
                                  offset=base.offset + h * HD,
                                  ap=[[HC, 128], [N_CORES * 128 * HC, TT4],
                                      [1, HD]])
                    nc.sync.dma_start(out=dst, in_=stg)

    # =================== AllToAll (per-tt) + FFN ===================
    with tc.tile_pool(name="ffw", bufs=1) as ffw, \
         tc.tile_pool(name="ffa", bufs=3) as ffa, \
         tc.tile_pool(name="ffs", bufs=4) as ffs, \
         tc.tile_pool(name="w1p", bufs=4) as w1p, \
         tc.tile_pool(name="w2p", bufs=8) as w2p, \
         tc.tile_pool(name="psf", bufs=2, space="PSUM") as psfp, \
         tc.tile_pool(name="pso", bufs=1, space="PSUM") as psop, \
         tc.tile_pool(name="pstf", bufs=2, space="PSUM") as pstf:

        for tt in range(TT4):
            if fake_cc:
                nc.sync.dma_start(out=cc_out[tt, :, :], in_=cc_in[tt, :, :])
            else:
                nc.gpsimd.collective_compute(
                    "AllToAll", ALU.bypass,
                    replica_groups=[list(range(N_CORES))],
                    ins=[cc_in[tt, :, :].opt()],
                    outs=[cc_out[tt, :, :].opt()])

        g2bc = ffw.tile([128, D], F32)
        nc.sync.dma_start(out=g2bc, in_=g2_d[None, :].to_broadcast([128, D]))
        be2bc = ffw.tile([128, D], F32)
        nc.sync.dma_start(out=be2bc, in_=be2_d[None, :].to_broadcast([128, D]))
        bf2bc = ffw.tile([128, D], F32)
        nc.sync.dma_start(out=bf2bc, in_=bf2_d[None, :].to_broadcast([128, D]))
        bf1sb = ffw.tile([128, DFF // 128], F32)
        nc.sync.dma_start(out=bf1sb, in_=bf1_d.rearrange("(f p) -> p f", p=128))
        bf1h = ffw.tile([128, DFF // 128], F32)
        nc.scalar.activation(out=bf1h, in_=bf1sb, func=AF.Copy, scale=0.5)
        x1_all = ffw.tile([128, TT4, D], F32)
        xn1T = ffw.tile([128, D // 128, TOK], BF16)
        hT = ffw.tile([128, DFF // 128, TOK], BF16)

        cc_a = ffw.tile([128, TT4, D], BF16)
        cc_b = ffw.tile([128, TT4, D], BF16)
        for tt in range(TT4):
            for half, dstt in ((0, cc_a), (1, cc_b)):
                srcb = cc_out[tt, half * 4 * 128:(half * 4 + 4) * 128, :]
                srca = bass.AP(tensor=srcb.tensor, offset=srcb.offset,
                               ap=[[HC, 128], [128 * HC, 4], [1, HC]])
                nc.sync.dma_start(out=dstt[:, tt, :], in_=srca)
        for tt in range(TT4):
            xa = ffa.tile([128, D], BF16, tag="xa")
            nc.vector.tensor_add(out=xa, in0=cc_a[:, tt, :], in1=cc_b[:, tt, :])
            nc.gpsimd.tensor_add(out=x1_all[:, tt, :], in0=xtk_all[:, tt, :],
                                 in1=xa)
            # LN2
            st = ffs.tile([128, 2, 6], F32, tag="st")
            for sg in range(2):
                nc.vector.bn_stats(out=st[:, sg, :],
                                   in_=x1_all[:, tt, sg * 512:(sg + 1) * 512])
            mv = ffs.tile([128, 2], F32, tag="mv")
            nc.vector.bn_aggr(out=mv, in_=st)
            rstd = ffs.tile([128, 1], F32, tag="rstd")
            nc.scalar.activation(out=rstd, in_=mv[:, 1:2], func=AF.Sqrt,
                                 bias=epsT, scale=1.0)
            nc.vector.reciprocal(out=rstd, in_=rstd)
            xn1 = ffa.tile([128, D], F32, tag="xn1")
            nc.vector.tensor_scalar(out=xn1, in0=x1_all[:, tt, :],
                                    scalar1=mv[:, 0:1], scalar2=rstd,
                                    op0=ALU.subtract, op1=ALU.mult)
            nc.vector.tensor_mul(out=xn1, in0=xn1, in1=g2bc)
            nc.gpsimd.tensor_add(out=xn1, in0=xn1, in1=be2bc)
            for dd in range(D // 128):
                pt = pstf.tile([128, 128], F32, tag="pt")
                nc.tensor.transpose(pt, xn1[:, dd * 128:(dd + 1) * 128], ident)
                nc.scalar.copy(out=xn1T[:, dd, tt * 128:(tt + 1) * 128], in_=pt)

        # h^T = gelu(W1^T @ xn1^T + bf1)
        for f in range(DFF // 128):
            ph = psfp.tile([128, TOK], F32, tag="ph")
            for dd in range(D // 128):
                nc.tensor.matmul(ph, w1sb[:, f, dd, :], xn1T[:, dd, :],
                                 start=(dd == 0), stop=(dd == D // 128 - 1))
            # gelu (tanh approx), computed on y = x/2:
            #   gelu(x) = y*(1+tanh(y*(2*c0 + 8*c3*y^2))), c0=sqrt(2/pi), c3=0.044715*c0
            gy = ffa.tile([128, TOK], F32, tag="gy")
            nc.scalar.activation(out=gy, in_=ph, func=AF.Identity, scale=0.5,
                                 bias=bf1h[:, f:f + 1])
            gt = ffa.tile([128, TOK], F32, tag="gt")
            nc.scalar.activation(out=gt, in_=gy, func=AF.Square, scale=1.0)
            nc.vector.tensor_scalar(out=gt, in0=gt, scalar1=8 * 0.044715 * GC0,
                                    scalar2=2 * GC0, op0=ALU.mult, op1=ALU.add)
            nc.vector.tensor_mul(out=gt, in0=gt, in1=gy)
            nc.scalar.activation(out=gt, in_=gt, func=AF.Tanh, scale=1.0)
            nc.vector.scalar_tensor_tensor(out=hT[:, f, :], in0=gt, scalar=1.0,
                                           in1=gy, op0=ALU.add, op1=ALU.mult)

        # out = x1 + h @ W2 + bf2   (W2 streamed, bf16)
        for dh in range(D // 512):
            pos = [psop.tile([128, 512], F32, tag=f"po{tt}", name=f"po{tt}") for tt in range(TT4)]
            for f in range(DFF // 128):
                w2b = w2p.tile([128, 512], BF16, tag="w2b")
                nc.sync.dma_start(out=w2b,
                                  in_=w2_d[f * 128:(f + 1) * 128,
                                           dh * 512:(dh + 1) * 512])
                for tt in range(TT4):
                    nc.tensor.matmul(pos[tt], hT[:, f, tt * 128:(tt + 1) * 128],
                                     w2b, start=(f == 0),
                                     stop=(f == DFF // 128 - 1))
            for tt in range(TT4):
                o1 = ffa.tile([128, 512], F32, tag="o1")
                nc.vector.tensor_add(out=o1, in0=pos[tt],
                                     in1=x1_all[:, tt, dh * 512:(dh + 1) * 512])
                nc.vector.tensor_add(out=o1, in0=o1,
                                     in1=bf2bc[:, dh * 512:(dh + 1) * 512])
                nc.sync.dma_start(out=out_d[tt * 128:(tt + 1) * 128,
                                            dh * 512:(dh + 1) * 512], in_=o1)

    ctx.close()


# ======================= host-side driver =======================

def shard_inputs(inputs, S=S_FULL):
    """Build per-core in_maps from full inputs."""
    import ml_dtypes
    x = np.ascontiguousarray(inputs["x"], dtype=np.float32)
    w1b = np.ascontiguousarray(
        np.asarray(inputs["W1"], np.float32).astype(ml_dtypes.bfloat16))
    w2b = np.ascontiguousarray(
        np.asarray(inputs["W2"], np.float32).astype(ml_dtypes.bfloat16))
    TOK = S // 4
    in_maps = []
    for c in range(N_CORES):
        b = c // 4
        hg = c % 4
        hsl = slice(hg * NHL, (hg + 1) * NHL)            # global head indices
        csl = slice(hg * NHL * HD, (hg + 1) * NHL * HD)  # head cols in D
        rsl = slice(hg * TOK, (hg + 1) * TOK)            # FFN token rows
        m = {
            "x_full": x[b],
            "x_heads": x[b][:, csl],
            "x_tok": x[b][rsl, :],
            "wq": inputs["Wq"][hsl].reshape(NHL * HD, HD),
            "wk": inputs["Wk"][hsl].reshape(NHL * HD, HD),
            "wv": inputs["Wv"][hsl].reshape(NHL * HD, HD),
            "wo": inputs["Wo"][hsl].reshape(NHL * HD, HD),
            "omega": inputs["omega"][hsl],
            "g1h": inputs["g1"][csl],
            "be1h": inputs["be1"][csl],
            "g2": inputs["g2"], "be2": inputs["be2"],
            "w1b": w1b, "bf1": inputs["bf1"],
            "w2b": w2b, "bf2": inputs["bf2"],
            "gmask": np.array([1.0 if j // 4 == b else 0.0
                               for j in range(N_CORES)], dtype=np.float32),
        }
        in_maps.append({k: (v if k in ("w1b", "w2b") else
                            np.ascontiguousarray(v, dtype=np.float32))
                        for k, v in m.items()})
    return in_maps


def assemble_output(results, S=S_FULL):
    TOK = S // 4
    out = np.zeros((B, S, D), dtype=np.float32)
    for c in range(N_CORES):
        b, hg = c // 4, c % 4
        out[b, hg * TOK:(hg + 1) * TOK, :] = results[c]["out"]
    return out


_NC_CACHE = {}


def kernel(**inputs):
    from concourse.bass_utils import run_bass_kernel_spmd
    S = inputs["x"].shape[1]
    if S not in _NC_CACHE:
        _NC_CACHE[S] = build_nc(S)
    nc = _NC_CACHE[S]
    in_maps = shard_inputs(inputs, S)
    res = run_bass_kernel_spmd(nc, in_maps, core_ids=list(range(N_CORES)))
    return assemble_output(res.results, S)



# revision 56
# speedup vs baseline: 1.4593x; 1.0070x over previous
"""Trainium2 Bass kernel for MinimalResonanceLayer (8-core SPMD).

Sharding: core c handles batch b = c//4 and local heads [ (c%4)*4, (c%4)*4+4 ).
Each head's resonance recurrence runs fully on-core (E^T resident in SBUF,
bf16); the head-concat + FFN uses one 8-core AllToAll, with per-core
divergence encoded in an input mask so the program stays SPMD-uniform.

State is kept in rotated coordinates z~ = K*exp(-i*alpha)*z so the
Kuramoto-Sakaguchi rotation folds into the PSUM copy-out scalars.
"""
import math
import numpy as np

import concourse.bass as bass
import concourse.tile as tile
from concourse import bacc, mybir
from concourse.masks import make_identity

# ---- problem constants (hardcoded per contest contract) ----
B, S_FULL, D, H, HD = 2, 2048, 1024, 16, 64
DFF = 2 * D
MU, ALPHA, K_COUP, DT, STEPS, MIX = 1.0, 0.1, 3.0, 0.02, 5, 0.3
N_CORES = 8
NHL = 4  # heads per core

CA, SA = math.cos(ALPHA), math.sin(ALPHA)
C1V = K_COUP * CA            # c1
C2V = K_COUP * SA            # c2
CC1 = MU - K_COUP            # -2.0
R21 = C2V / C1V              # tan(alpha)
W1S = C1V - C2V * C2V / C1V  # pass-1 roa scale
W2S = -2.0 * C2V             # pass-1 rob scale
M2 = (1.0 - MIX) * CA / K_COUP
M3 = (1.0 - MIX) * SA / K_COUP
SCL = 1.0 / math.sqrt(HD)
INVK = 1.0 / K_COUP
GC0 = math.sqrt(2.0 / math.pi)

F32 = mybir.dt.float32
F32R = mybir.dt.float32r
BF16 = mybir.dt.bfloat16
FP8 = mybir.dt.float8e4
DRPM = mybir.MatmulPerfMode.DoubleRow
EXPB = -3.7  # exp offset: e^(8.97+eps-3.7) < 240 = fp8e4 max finite (cancels via Z)
ALU = mybir.AluOpType
AF = mybir.ActivationFunctionType


def r(ap):
    """bitcast fp32 AP -> float32r for full-rate PE streaming."""
    return ap.bitcast(F32R)


def build_nc(S=S_FULL, fake_cc=False):
    """Build the 8-core SPMD program. S = sequence length (per batch).

    fake_cc=True replaces the AllToAll with a local DMA so the program is
    collective-free (for TimelineSim cost modeling only).
    """
    nc = bacc.Bacc("TRN2", target_bir_lowering=False, debug=False,
                   num_devices=N_CORES)

    def din(name, shape):
        return nc.dram_tensor(name, shape, F32, kind="ExternalInput").ap()

    TOK = S // 4
    io = dict(
        x_full=din("x_full", [S, D]),
        x_heads=din("x_heads", [S, NHL * HD]),
        x_tok=din("x_tok", [TOK, D]),
        wq_d=din("wq", [NHL * HD, HD]),
        wk_d=din("wk", [NHL * HD, HD]),
        wv_d=din("wv", [NHL * HD, HD]),
        wo_d=din("wo", [NHL * HD, HD]),
        om_d=din("omega", [NHL, HD]),
        g1_d=din("g1h", [NHL * HD]),
        be1_d=din("be1h", [NHL * HD]),
        g2_d=din("g2", [D]),
        be2_d=din("be2", [D]),
        w1_d=nc.dram_tensor("w1b", [D, DFF], BF16,
                            kind="ExternalInput").ap(),
        bf1_d=din("bf1", [DFF]),
        w2_d=nc.dram_tensor("w2b", [DFF, D], BF16,
                            kind="ExternalInput").ap(),
        bf2_d=din("bf2", [D]),
        gm_d=din("gmask", [N_CORES]),
        out_d=nc.dram_tensor("out", [TOK, D], F32, kind="ExternalOutput").ap(),
    )

    with tile.TileContext(nc) as tc:
        _body(nc, tc, io, S, fake_cc)

    nc.compile()
    return nc


def _body(nc, tc, io, S, fake_cc=False):
    NMB = S // 128          # token blocks of 128
    NG = NMB // 4           # groups of 4 blocks (512 tokens)
    TOK = S // 4            # FFN tokens per core (= B*S/8)
    TT4 = TOK // 128        # FFN token tiles
    NSL = S // 512          # 512-wide column slices of S
    HC = NHL * HD           # 256 head cols per core

    x_full, x_heads, x_tok = io["x_full"], io["x_heads"], io["x_tok"]
    wq_d, wk_d, wv_d, wo_d = io["wq_d"], io["wk_d"], io["wv_d"], io["wo_d"]
    om_d, g1_d, be1_d = io["om_d"], io["g1_d"], io["be1_d"]
    g2_d, be2_d = io["g2_d"], io["be2_d"]
    w1_d, bf1_d, w2_d, bf2_d = io["w1_d"], io["bf1_d"], io["w2_d"], io["bf2_d"]
    gm_d, out_d = io["gm_d"], io["out_d"]

    from contextlib import ExitStack
    ctx = ExitStack()
    sing = ctx.enter_context(tc.tile_pool(name="sing", bufs=1))
    dram = ctx.enter_context(tc.tile_pool(name="dram", bufs=1, space="DRAM"))

    # ---- whole-kernel constants ----
    ident = sing.tile([128, 128], F32)
    make_identity(nc, ident)
    identb = sing.tile([128, 128], BF16)
    nc.vector.tensor_copy(out=identb, in_=ident)
    epsT = sing.tile([128, 1], F32)
    nc.vector.memset(epsT, 1e-5)
    expbT = sing.tile([128, 1], F32)
    nc.vector.memset(expbT, EXPB)
    maskbc = sing.tile([128, N_CORES], F32)
    nc.sync.dma_start(out=maskbc, in_=gm_d[None, :].to_broadcast([128, N_CORES]))
    # signed rotation coefficient row: [+R21 | -R21] (for s1 = R21v * wswap)
    R21v = sing.tile([128, 128], BF16)
    nc.vector.memset(R21v[:, 0:HD], R21)
    nc.vector.memset(R21v[:, HD:128], -R21)

    xtk_all = sing.tile([128, TT4, D], F32)
    w1sb = sing.tile([128, DFF // 128, D // 128, 128], BF16)

    def issue_ffn_prefetch():
        for tt in range(TT4):
            nc.sync.dma_start(out=xtk_all[:, tt, :],
                              in_=x_tok[tt * 128:(tt + 1) * 128, :])
        for f in range(DFF // 128):
            nc.sync.dma_start(
                out=w1sb[:, f, :, :],
                in_=w1_d.rearrange("(dd p) ff -> p dd ff",
                                   p=128)[:, :, f * 128:(f + 1) * 128])
    cc_in = dram.tile([TOK // 128, N_CORES * 128, HC], BF16)
    cc_out = dram.tile([TOK // 128, N_CORES * 128, HC], BF16)

    # =================== attention super-phase ===================
    with ExitStack() as actx:
        big = actx.enter_context(tc.tile_pool(name="big", bufs=1))
        with tc.tile_pool(name="gstg", bufs=1) as gstg:
            g1f = gstg.tile([128, HC], F32)
            nc.sync.dma_start(out=g1f, in_=g1_d[None, :].to_broadcast([128, HC]))
            be1f = gstg.tile([128, HC], F32)
            nc.sync.dma_start(out=be1f,
                              in_=be1_d[None, :].to_broadcast([128, HC]))
            g1bc = big.tile([128, HC], BF16)
            nc.vector.tensor_copy(out=g1bc, in_=g1f)
            be1bc = big.tile([128, HC], BF16)
            nc.vector.tensor_copy(out=be1bc, in_=be1f)
        wq_sb = big.tile([64, NHL, HD], BF16)
        wk_sb = big.tile([64, NHL, HD], BF16)
        wv_sb = big.tile([64, NHL, HD], BF16)
        wo_bf = big.tile([128, NHL, HD], BF16)
        with tc.tile_pool(name="wstg", bufs=1) as wstg:
            wstage = wstg.tile([64, 4, NHL, HD], F32)
            for wi, wd in enumerate([wq_d, wk_d, wv_d, wo_d]):
                nc.sync.dma_start(out=wstage[:, wi, :, :],
                                  in_=wd.rearrange("(h p) e -> p h e", p=HD))
            nc.vector.tensor_copy(out=wq_sb, in_=wstage[:, 0, :, :])
            nc.vector.tensor_copy(out=wk_sb, in_=wstage[:, 1, :, :])
            nc.vector.tensor_copy(out=wv_sb, in_=wstage[:, 2, :, :])
            nc.vector.tensor_copy(out=wo_bf[0:64, :, :],
                                  in_=wstage[:, 3, :, :])
            nc.vector.tensor_copy(out=wo_bf[64:128, :, :],
                                  in_=wstage[:, 3, :, :])

        xnh = big.tile([128, NMB, HC], BF16)
        X = big.tile([128, NMB, 128], BF16)     # state [XA | XB], bf16
        X8 = big.tile([128, NMB, 128], FP8)     # fp8(rot(X)) matmul rhs
        Xp8 = big.tile([128, NMB, 128], FP8)    # fp8(rot(Xp))
        vb18 = big.tile([128, NMB, 128], FP8)   # fp8 [v | ones] pass-1 rhs
        Xp = big.tile([128, NMB, 128], BF16)    # Heun predictor
        tsum = big.tile([128, NMB, 128], BF16)  # Xp + X (for corrector)
        vb1 = big.tile([128, NMB, 128], BF16)   # [v | ones] pass-1 rhs
        attnv = big.tile([128, NMB, HD], BF16)  # A @ v
        rz2dt = big.tile([128, NMB, HD], BF16)   # DT*c1/Z (bcast to halves)
        om_st = big.tile([128, HD], F32)
        omdt = big.tile([128, 128], BF16)       # [-DT*omega | +DT*omega]
        nc.vector.memset(vb1[:, :, HD:128], 1.0)
        nc.vector.memset(vb18[:, :, HD:128], 1.0)

        def swap_ap(t, gs):
            """halves-swapped view of t[:, gs, :]: [...,[XB|XA],...]"""
            base = t[:, gs, :]
            return bass.AP(tensor=base.tensor, offset=base.offset + HD,
                           ap=[base.ap[0], base.ap[1], [-HD, 2], [1, HD]])

        def bc2_ap(t):
            """[128, g, 64] -> [128, g, 2, 64] broadcast of the half dim"""
            return bass.AP(tensor=t.tensor, offset=t.offset,
                           ap=[t.ap[0], t.ap[1], [0, 2], [1, HD]])

        def row_ap(t, g=4):
            """[128, 128] const row -> [128, g, 128] group-broadcast"""
            return bass.AP(tensor=t.tensor, offset=t.offset,
                           ap=[t.ap[0], [0, g], [1, 128]])

        # ---------------- LN1 ----------------
        with tc.tile_pool(name="ln", bufs=4) as ln, \
             tc.tile_pool(name="lns", bufs=6) as lns:
            for t in range(NMB):
                xt = ln.tile([128, D], F32, tag="xt")
                nc.sync.dma_start(out=xt, in_=x_full[t * 128:(t + 1) * 128, :])
                xh = ln.tile([128, HC], F32, tag="xh")
                nc.sync.dma_start(out=xh, in_=x_heads[t * 128:(t + 1) * 128, :])
                st = lns.tile([128, 2, 6], F32, tag="st")
                for sg in range(2):
                    nc.vector.bn_stats(out=st[:, sg, :],
                                       in_=xt[:, sg * 512:(sg + 1) * 512])
                mv = lns.tile([128, 2], F32, tag="mv")
                nc.vector.bn_aggr(out=mv, in_=st)
                rstd = lns.tile([128, 1], F32, tag="rstd")
                nc.scalar.activation(out=rstd, in_=mv[:, 1:2], func=AF.Sqrt,
                                     bias=epsT, scale=1.0)
                nc.vector.reciprocal(out=rstd, in_=rstd)
                nb = lns.tile([128, 1], F32, tag="nb")
                nc.vector.tensor_scalar(out=nb, in0=mv[:, 0:1], scalar1=rstd,
                                        scalar2=-1.0, op0=ALU.mult, op1=ALU.mult)
                xs = lns.tile([128, HC], F32, tag="xs")
                nc.gpsimd.tensor_scalar(out=xs, in0=xh, scalar1=rstd,
                                        scalar2=nb, op0=ALU.mult, op1=ALU.add)
                nc.vector.tensor_mul(out=xs, in0=xs, in1=g1bc)
                nc.gpsimd.tensor_add(out=xnh[:, t, :], in0=xs, in1=be1bc)

        # ---------------- per-head resonance ----------------
        with tc.tile_pool(name="xhTp", bufs=2) as xhTp, \
             tc.tile_pool(name="etp", bufs=2) as etp, \
             tc.tile_pool(name="qkp", bufs=2) as qkp, \
             tc.tile_pool(name="pmisc", bufs=2, space="PSUM") as pmisc, \
             tc.tile_pool(name="pssc", bufs=2, space="PSUM") as psscp, \
             tc.tile_pool(name="psg", bufs=2, space="PSUM") as psgp, \
             tc.tile_pool(name="scr", bufs=2) as scr, \
             tc.tile_pool(name="mts", bufs=3) as mts:

            def emit_prep(h, ET, qT, kT, xhT):
                """Per-head prep as thunks: xhT transposes, q/k proj,
                scores+exp, then v-proj (v-proj last: WAR on vb1 must
                land after the previous head's pass-1 reads)."""
                th = []

                def omth():
                    src = bass.AP(tensor=om_d.tensor,
                                  offset=om_d.offset + h * HD,
                                  ap=[[0, 128], [1, HD]])
                    nc.sync.dma_start(out=om_st, in_=src)
                    nc.vector.tensor_scalar_mul(out=omdt[:, 0:HD], in0=om_st,
                                                scalar1=-DT)
                    nc.vector.tensor_scalar_mul(out=omdt[:, HD:128], in0=om_st,
                                                scalar1=DT)
                th.append(omth)

                def xhTth(t):
                    pt = pmisc.tile([64, 128], BF16, tag="pm")
                    nc.tensor.transpose(pt, xnh[:, t, h * HD:(h + 1) * HD],
                                        identb)
                    nc.scalar.copy(out=xhT[:, t * 128:(t + 1) * 128], in_=pt)
                for t in range(NMB):
                    th.append(lambda t=t: xhTth(t))

                def projth(sl):
                    pq = pmisc.tile([64, 512], F32, tag="pm")
                    nc.tensor.matmul(pq, wq_sb[:, h, :],
                                     xhT[:, sl * 512:(sl + 1) * 512],
                                     start=True, stop=True)
                    nc.scalar.copy(out=qT[:, sl * 512:(sl + 1) * 512], in_=pq)
                    pk = pmisc.tile([64, 512], F32, tag="pm")
                    nc.tensor.matmul(pk, wk_sb[:, h, :],
                                     xhT[:, sl * 512:(sl + 1) * 512],
                                     start=True, stop=True)
                    nc.scalar.copy(out=kT[:, sl * 512:(sl + 1) * 512], in_=pk)
                for sl in range(NSL):
                    th.append(lambda sl=sl: projth(sl))

                def scoreth(k, sl2):
                    c0 = sl2 * 1024
                    ps = psscp.tile([128, 1024], F32, tag="ps")
                    for j in range(2):
                        nc.tensor.matmul(ps[:, j * 512:(j + 1) * 512],
                                         kT[:, k * 128:(k + 1) * 128],
                                         qT[:, c0 + j * 512:c0 + (j + 1) * 512],
                                         start=True, stop=True)
                    nc.scalar.activation(out=ET[:, k, c0:c0 + 1024],
                                         in_=ps, func=AF.Exp, scale=SCL,
                                         bias=expbT)
                for sl2 in range(NSL // 2):
                    for k in range(NMB):
                        th.append(lambda k=k, sl2=sl2: scoreth(k, sl2))

                def vth(t):
                    pv = pmisc.tile([128, HD], F32, tag="pm")
                    nc.tensor.matmul(pv, xhT[:, t * 128:(t + 1) * 128],
                                     wv_sb[:, h, :], start=True, stop=True)
                    nc.scalar.copy(out=vb1[:, t, 0:HD], in_=pv)
                    nc.scalar.copy(out=vb18[:, t, 0:HD], in_=pv)
                for t in range(NMB):
                    th.append(lambda t=t: vth(t))
                return th

            def hbufs(h):
                return (etp.tile([128, NMB, S], FP8, tag="ET",
                                 name=f"ET_{h}"),
                        qkp.tile([64, S], BF16, tag="qT", name=f"qT_{h}"),
                        qkp.tile([64, S], BF16, tag="kT", name=f"kT_{h}"),
                        xhTp.tile([64, S], BF16, tag="xhT", name=f"xhT_{h}"))

            hbuf = {0: hbufs(0)}
            for f in emit_prep(0, *hbuf[0]):
                f()
            for h in range(NHL):
                ET, qT, kT, xhT = hbuf[h]
                if h + 1 < NHL:
                    hbuf[h + 1] = hbufs(h + 1)
                    nextq = emit_prep(h + 1, *hbuf[h + 1])
                else:
                    nextq = []
                if h == 0:
                    issue_ffn_prefetch()
                # v-proj thunks (the last 16) must land after pass-1 of head
                # h finishes reading vb1; draining starts at pass 2 so the
                # in-order PE queue never stalls on the WAR.
                nslots = (2 * STEPS - 1) * NG
                per_slot = max(1, -(-len(nextq) // nslots))

                # --- 10 Heun passes: fp8 DoubleRow matmuls, bf16 chain ---
                for p in range(1, 2 * STEPS + 1):
                    odd = (p % 2 == 1)
                    rhs8 = vb18 if p == 1 else (X8 if odd else Xp8)
                    xin = X if odd else Xp

                    for g in range(NG):
                        pg = psgp.tile([128, 4, 128], F32, tag="pg")
                        for ml in range(4):
                            mb = g * 4 + ml
                            for kp in range(NMB // 2):
                                nc.tensor.matmul(
                                    pg[:, ml, :],
                                    ET[:, 2 * kp:2 * kp + 2,
                                       mb * 128:(mb + 1) * 128],
                                    rhs8[:, 2 * kp:2 * kp + 2, :],
                                    start=(kp == 0), stop=(kp == NMB // 2 - 1),
                                    perf_mode=DRPM)
                        gs = slice(g * 4, g * 4 + 4)
                        ro = scr.tile([128, 4, 128], BF16, tag="ro")
                        if p == 1:
                            # psum = [E8@v | Z8rep]; rz2dt = DT*c1/Z8
                            w = scr.tile([128, 4, 128], BF16, tag="w")
                            nc.scalar.copy(out=w, in_=pg)
                            rcp = scr.tile([128, 4], F32, tag="rcp")
                            nc.vector.reciprocal(out=rcp, in_=pg[:, :, HD:HD + 1])
                            rcpb = scr.tile([128, 4], BF16, tag="rcpb")
                            nc.vector.tensor_copy(out=rcpb, in_=rcp)
                            rb64b = bass.AP(tensor=rcpb.tensor, offset=rcpb.offset,
                                            ap=[rcpb.ap[0], [1, 4], [0, HD]])
                            nc.vector.tensor_scalar_mul(out=rz2dt[:, gs, :],
                                                        in0=rb64b,
                                                        scalar1=DT * C1V)
                            rb64 = bass.AP(tensor=rcpb.tensor, offset=rcpb.offset,
                                           ap=[rcpb.ap[0], [1, 4], [0, HD]])
                            nc.vector.tensor_mul(out=attnv[:, gs, :],
                                                 in0=w[:, :, 0:HD], in1=rb64)
                            # state init X0 = [c1*v | -c2*v]
                            nc.vector.tensor_scalar_mul(out=X[:, gs, 0:HD],
                                                        in0=vb1[:, gs, 0:HD],
                                                        scalar1=C1V)
                            nc.vector.tensor_scalar_mul(out=X[:, gs, HD:128],
                                                        in0=vb1[:, gs, 0:HD],
                                                        scalar1=-C2V)
                            # rotated coupling via W1S/W2S, then * rz
                            nc.vector.tensor_scalar_mul(out=w[:, :, HD:128],
                                                        in0=w[:, :, 0:HD],
                                                        scalar1=W2S)
                            nc.vector.tensor_scalar_mul(out=w[:, :, 0:HD],
                                                        in0=w[:, :, 0:HD],
                                                        scalar1=W1S)
                            rzb = rz2dt[:, gs, :]
                            rzb = bass.AP(tensor=rzb.tensor, offset=rzb.offset,
                                          ap=[rzb.ap[0], rzb.ap[1], [0, 2],
                                              [1, HD]])
                            nc.vector.tensor_mul(out=ro, in0=w, in1=rzb)
                        else:
                            # rhs was pre-rotated: psum IS the rotated coupling
                            rzb = rz2dt[:, gs, :]
                            rzb = bass.AP(tensor=rzb.tensor, offset=rzb.offset,
                                          ap=[rzb.ap[0], rzb.ap[1], [0, 2],
                                              [1, HD]])
                            if g == NG - 1:
                                # barrier group: single fused PSUM read on DVE
                                nc.vector.tensor_mul(out=ro, in0=pg, in1=rzb)
                            else:
                                w = scr.tile([128, 4, 128], BF16, tag="w")
                                nc.scalar.copy(out=w, in_=pg)
                                nc.vector.tensor_mul(out=ro, in0=w, in1=rzb)
                        # elementwise drift, DT-scaled: dd = DT*f_local + ro
                        sq = scr.tile([128, 4, 128], BF16, tag="sq")
                        if h >= 2:
                            nc.scalar.activation(out=sq, in_=xin[:, gs, :],
                                                 func=AF.Square, scale=1.0)
                        else:
                            nc.gpsimd.tensor_mul(out=sq, in0=xin[:, gs, :],
                                                 in1=xin[:, gs, :])
                        r2h = scr.tile([128, 4, HD], BF16, tag="r2h")
                        nc.vector.tensor_add(out=r2h, in0=sq[:, :, 0:HD],
                                             in1=sq[:, :, HD:128])
                        mtl = scr.tile([128, 4, HD], BF16, tag="mtl")
                        nc.vector.tensor_scalar(out=mtl, in0=r2h,
                                                scalar1=-DT * INVK * INVK,
                                                scalar2=DT * CC1,
                                                op0=ALU.mult, op1=ALU.add)
                        u = scr.tile([128, 4, 128], BF16, tag="u")
                        nc.vector.tensor_mul(out=u, in0=bc2_ap(mtl),
                                             in1=xin[:, gs, :])
                        cross = scr.tile([128, 4, 128], BF16, tag="cross")
                        nc.gpsimd.tensor_mul(out=cross, in0=row_ap(omdt),
                                             in1=swap_ap(xin, gs))
                        nc.vector.tensor_add(out=u, in0=u, in1=cross)
                        dd = u
                        nc.vector.tensor_add(out=dd, in0=u, in1=ro)
                        ry = scr.tile([128, 4, 128], BF16, tag="ry",
                                      name=f"ry_{h}_{p}_{g}")
                        if odd:
                            nc.vector.tensor_add(out=Xp[:, gs, :],
                                                 in0=X[:, gs, :], in1=dd)
                            nc.vector.tensor_add(out=tsum[:, gs, :],
                                                 in0=Xp[:, gs, :], in1=X[:, gs, :])
                            if p < 2 * STEPS:
                                # Xp8 = fp8(rot(Xp))
                                nc.vector.tensor_mul(out=ry, in0=row_ap(R21v),
                                                     in1=swap_ap(Xp, gs))
                                nc.vector.tensor_add(out=ry, in0=ry,
                                                     in1=Xp[:, gs, :])
                                if g == NG - 1:
                                    nc.vector.tensor_copy(out=Xp8[:, gs, :],
                                                          in_=ry)
                                else:
                                    nc.scalar.copy(out=Xp8[:, gs, :], in_=ry)
                        else:
                            # X' = 0.5*(Xp + X + dd2)
                            nc.vector.tensor_add(out=dd, in0=tsum[:, gs, :],
                                                 in1=dd)
                            nc.vector.tensor_scalar_mul(out=X[:, gs, :], in0=dd,
                                                        scalar1=0.5)
                            if p < 2 * STEPS:
                                # X8 = fp8(rot(X))
                                nc.gpsimd.tensor_mul(out=ry, in0=row_ap(R21v),
                                                     in1=swap_ap(X, gs))
                                nc.vector.tensor_add(out=ry, in0=ry,
                                                     in1=X[:, gs, :])
                                if g == NG - 1:
                                    nc.vector.tensor_copy(out=X8[:, gs, :],
                                                          in_=ry)
                                else:
                                    nc.scalar.copy(out=X8[:, gs, :], in_=ry)
                        if p >= 2 and g < NG - 1:
                            for _ in range(min(per_slot, len(nextq))):
                                nextq.pop(0)()
                for f in nextq:
                    f()
                nextq = []

                # --- readout: mixed -> @Wo -> xattn cols ---
                for gq in range(0, NMB, 4):
                    gqs = slice(gq, gq + 4)
                    nc.vector.tensor_scalar_mul(out=attnv[:, gqs, :],
                                                in0=attnv[:, gqs, :],
                                                scalar1=MIX)
                    nc.vector.scalar_tensor_tensor(out=attnv[:, gqs, :],
                                                   in0=X[:, gqs, 0:HD],
                                                   scalar=M2,
                                                   in1=attnv[:, gqs, :],
                                                   op0=ALU.mult, op1=ALU.add)
                    nc.vector.scalar_tensor_tensor(out=attnv[:, gqs, :],
                                                   in0=X[:, gqs, HD:128],
                                                   scalar=-M3,
                                                   in1=attnv[:, gqs, :],
                                                   op0=ALU.mult, op1=ALU.add)
                mixv = attnv
                xatth = mts.tile([128, NMB, HD], BF16, tag="xatth",
                                 name=f"xatth_{h}")
                for t2 in range(NMB // 2):
                    pt = pmisc.tile([128, 128], BF16, tag="pm")
                    nc.tensor.transpose(pt, mixv[:, 2 * t2:2 * t2 + 2, :],
                                        identb)
                    mt = mts.tile([128, 128], BF16, tag="mt")
                    nc.scalar.copy(out=mt, in_=pt)
                    for j in range(2):
                        po = pmisc.tile([128, HD], F32, tag="pm")
                        nc.tensor.matmul(po, mt[j * 64:(j + 1) * 64, :],
                                         wo_bf[j * 64:(j + 1) * 64, h, :],
                                         start=True, stop=True)
                        nc.scalar.copy(out=xatth[:, 2 * t2 + j, :], in_=po)

                # --- stage this head's slice of cc_in (masked) ---
                for j in range(N_CORES):
                    t0 = (j % 4) * TT4
                    stg = mts.tile([128, TT4, HD], BF16, tag="stg",
                                   name=f"stg_{h}_{j}")
                    nc.vector.tensor_scalar_mul(
                        out=stg,
                        in0=xatth[:, t0:t0 + TT4, :],
                        scalar1=maskbc[:, j:j + 1])
                    base = cc_in[0, j * 128:(j + 1) * 128, :]
                    dst = bass.AP(tensor=base.tensor,
                                  offset=base.offset + h * HD,
                                  ap=[[HC, 128], [N_CORES * 128 * HC, TT4],
                                      [1, HD]])
                    nc.sync.dma_start(out=dst, in_=stg)

    # =================== AllToAll (per-tt) + FFN ===================
    with tc.tile_pool(name="ffw", bufs=1) as ffw, \
         tc.tile_pool(name="ffa", bufs=4) as ffa, \
         tc.tile_pool(name="ffs", bufs=4) as ffs, \
         tc.tile_pool(name="w1p", bufs=4) as w1p, \
         tc.tile_pool(name="w2p", bufs=8) as w2p, \
         tc.tile_pool(name="psf", bufs=2, space="PSUM") as psfp, \
         tc.tile_pool(name="pso", bufs=1, space="PSUM") as psop, \
         tc.tile_pool(name="pstf", bufs=2, space="PSUM") as pstf:

        for tt in range(TT4):
            if fake_cc:
                nc.sync.dma_start(out=cc_out[tt, :, :], in_=cc_in[tt, :, :])
            else:
                nc.gpsimd.collective_compute(
                    "AllToAll", ALU.bypass,
                    replica_groups=[list(range(N_CORES))],
                    ins=[cc_in[tt, :, :].opt()],
                    outs=[cc_out[tt, :, :].opt()])

        g2bc = ffw.tile([128, D], F32)
        nc.sync.dma_start(out=g2bc, in_=g2_d[None, :].to_broadcast([128, D]))
        be2bc = ffw.tile([128, D], F32)
        nc.sync.dma_start(out=be2bc, in_=be2_d[None, :].to_broadcast([128, D]))
        bf2bc = ffw.tile([128, D], F32)
        nc.sync.dma_start(out=bf2bc, in_=bf2_d[None, :].to_broadcast([128, D]))
        bf1sb = ffw.tile([128, DFF // 128], F32)
        nc.sync.dma_start(out=bf1sb, in_=bf1_d.rearrange("(f p) -> p f", p=128))
        bf1h = ffw.tile([128, DFF // 128], F32)
        nc.scalar.activation(out=bf1h, in_=bf1sb, func=AF.Copy, scale=0.5)
        x1_all = ffw.tile([128, TT4, D], F32)
        xn1T = ffw.tile([128, D // 128, TOK], BF16)
        hT = ffw.tile([128, DFF // 128, TOK], BF16)

        cc_a = ffw.tile([128, TT4, D], BF16)
        cc_b = ffw.tile([128, TT4, D], BF16)
        for tt in range(TT4):
            for half, dstt in ((0, cc_a), (1, cc_b)):
                srcb = cc_out[tt, half * 4 * 128:(half * 4 + 4) * 128, :]
                srca = bass.AP(tensor=srcb.tensor, offset=srcb.offset,
                               ap=[[HC, 128], [128 * HC, 4], [1, HC]])
                nc.sync.dma_start(out=dstt[:, tt, :], in_=srca)
        for tt in range(TT4):
            xa = ffa.tile([128, D], BF16, tag="xa")
            nc.vector.tensor_add(out=xa, in0=cc_a[:, tt, :], in1=cc_b[:, tt, :])
            nc.gpsimd.tensor_add(out=x1_all[:, tt, :], in0=xtk_all[:, tt, :],
                                 in1=xa)
            # LN2
            st = ffs.tile([128, 2, 6], F32, tag="st")
            for sg in range(2):
                nc.vector.bn_stats(out=st[:, sg, :],
                                   in_=x1_all[:, tt, sg * 512:(sg + 1) * 512])
            mv = ffs.tile([128, 2], F32, tag="mv")
            nc.vector.bn_aggr(out=mv, in_=st)
            rstd = ffs.tile([128, 1], F32, tag="rstd")
            nc.scalar.activation(out=rstd, in_=mv[:, 1:2], func=AF.Sqrt,
                                 bias=epsT, scale=1.0)
            nc.vector.reciprocal(out=rstd, in_=rstd)
            xn1 = ffa.tile([128, D], F32, tag="xn1")
            nc.vector.tensor_scalar(out=xn1, in0=x1_all[:, tt, :],
                                    scalar1=mv[:, 0:1], scalar2=rstd,
                                    op0=ALU.subtract, op1=ALU.mult)
            nc.vector.tensor_mul(out=xn1, in0=xn1, in1=g2bc)
            nc.gpsimd.tensor_add(out=xn1, in0=xn1, in1=be2bc)
            for dd in range(D // 128):
                pt = pstf.tile([128, 128], F32, tag="pt")
                nc.tensor.transpose(pt, xn1[:, dd * 128:(dd + 1) * 128], ident)
                nc.scalar.copy(out=xn1T[:, dd, tt * 128:(tt + 1) * 128], in_=pt)

        # h^T = gelu(W1^T @ xn1^T + bf1)
        for f in range(DFF // 128):
            ph = psfp.tile([128, TOK], F32, tag="ph")
            for dd in range(D // 128):
                nc.tensor.matmul(ph, w1sb[:, f, dd, :], xn1T[:, dd, :],
                                 start=(dd == 0), stop=(dd == D // 128 - 1))
            # gelu (tanh approx), computed on y = x/2:
            #   gelu(x) = y*(1+tanh(y*(2*c0 + 8*c3*y^2))), c0=sqrt(2/pi), c3=0.044715*c0
            gy = ffa.tile([128, TOK], F32, tag="gy")
            nc.scalar.activation(out=gy, in_=ph, func=AF.Identity, scale=0.5,
                                 bias=bf1h[:, f:f + 1])
            gt = ffa.tile([128, TOK], F32, tag="gt")
            nc.scalar.activation(out=gt, in_=gy, func=AF.Square, scale=1.0)
            nc.vector.tensor_scalar(out=gt, in0=gt, scalar1=8 * 0.044715 * GC0,
                                    scalar2=2 * GC0, op0=ALU.mult, op1=ALU.add)
            nc.vector.tensor_mul(out=gt, in0=gt, in1=gy)
            nc.scalar.activation(out=gt, in_=gt, func=AF.Tanh, scale=1.0)
            nc.vector.scalar_tensor_tensor(out=hT[:, f, :], in0=gt, scalar=1.0,
                                           in1=gy, op0=ALU.add, op1=ALU.mult)

        # out = x1 + h @ W2 + bf2   (W2 streamed, bf16)
        for dh in range(D // 512):
            pos = [psop.tile([128, 512], F32, tag=f"po{tt}", name=f"po{tt}") for tt in range(TT4)]
            for f in range(DFF // 128):
                w2b = w2p.tile([128, 512], BF16, tag="w2b")
                nc.sync.dma_start(out=w2b,
                                  in_=w2_d[f * 128:(f + 1) * 128,
                                           dh * 512:(dh + 1) * 512])
                for tt in range(TT4):
                    nc.tensor.matmul(pos[tt], hT[:, f, tt * 128:(tt + 1) * 128],
                                     w2b, start=(f == 0),
                                     stop=(f == DFF // 128 - 1))
            for tt in range(TT4):
                o1 = ffa.tile([128, 512], F32, tag="o1")
                nc.vector.tensor_add(out=o1, in0=pos[tt],
                                     in1=x1_all[:, tt, dh * 512:(dh + 1) * 512])
                nc.vector.tensor_add(out=o1, in0=o1,
                                     in1=bf2bc[:, dh * 512:(dh + 1) * 512])
                nc.sync.dma_start(out=out_d[tt * 128:(tt + 1) * 128,
                                            dh * 512:(dh + 1) * 512], in_=o1)

    ctx.close()


# ======================= host-side driver =======================

def shard_inputs(inputs, S=S_FULL):
    """Build per-core in_maps from full inputs."""
    import ml_dtypes
    x = np.ascontiguousarray(inputs["x"], dtype=np.float32)
    w1b = np.ascontiguousarray(
        np.asarray(inputs["W1"], np.float32).astype(ml_dtypes.bfloat16))
    w2b = np.ascontiguousarray(
        np.asarray(inputs["W2"], np.float32).astype(ml_dtypes.bfloat16))
    TOK = S // 4
    in_maps = []
    for c in range(N_CORES):
        b = c // 4
        hg = c % 4
        hsl = slice(hg * NHL, (hg + 1) * NHL)            # global head indices
        csl = slice(hg * NHL * HD, (hg + 1) * NHL * HD)  # head cols in D
        rsl = slice(hg * TOK, (hg + 1) * TOK)            # FFN token rows
        m = {
            "x_full": x[b],
            "x_heads": x[b][:, csl],
            "x_tok": x[b][rsl, :],
            "wq": inputs["Wq"][hsl].reshape(NHL * HD, HD),
            "wk": inputs["Wk"][hsl].reshape(NHL * HD, HD),
            "wv": inputs["Wv"][hsl].reshape(NHL * HD, HD),
            "wo": inputs["Wo"][hsl].reshape(NHL * HD, HD),
            "omega": inputs["omega"][hsl],
            "g1h": inputs["g1"][csl],
            "be1h": inputs["be1"][csl],
            "g2": inputs["g2"], "be2": inputs["be2"],
            "w1b": w1b, "bf1": inputs["bf1"],
            "w2b": w2b, "bf2": inputs["bf2"],
            "gmask": np.array([1.0 if j // 4 == b else 0.0
                               for j in range(N_CORES)], dtype=np.float32),
        }
        in_maps.append({k: (v if k in ("w1b", "w2b") else
                            np.ascontiguousarray(v, dtype=np.float32))
                        for k, v in m.items()})
    return in_maps


def assemble_output(results, S=S_FULL):
    TOK = S // 4
    out = np.zeros((B, S, D), dtype=np.float32)
    for c in range(N_CORES):
        b, hg = c // 4, c % 4
        out[b, hg * TOK:(hg + 1) * TOK, :] = results[c]["out"]
    return out


_NC_CACHE = {}


def kernel(**inputs):
    from concourse.bass_utils import run_bass_kernel_spmd
    S = inputs["x"].shape[1]
    if S not in _NC_CACHE:
        _NC_CACHE[S] = build_nc(S)
    nc = _NC_CACHE[S]
    in_maps = shard_inputs(inputs, S)
    res = run_bass_kernel_spmd(nc, in_maps, core_ids=list(range(N_CORES)))
    return assemble_output(res.results, S)



# revision 70
# speedup vs baseline: 1.4822x; 1.0157x over previous
"""Trainium2 Bass kernel for MinimalResonanceLayer (8-core SPMD).

Sharding: core c handles batch b = c//4 and local heads [ (c%4)*4, (c%4)*4+4 ).
Each head's resonance recurrence runs fully on-core (E^T resident in SBUF,
bf16); the head-concat + FFN uses one 8-core AllToAll, with per-core
divergence encoded in an input mask so the program stays SPMD-uniform.

State is kept in rotated coordinates z~ = K*exp(-i*alpha)*z so the
Kuramoto-Sakaguchi rotation folds into the PSUM copy-out scalars.
"""
import math
import numpy as np

import concourse.bass as bass
import concourse.tile as tile
from concourse import bacc, mybir
from concourse.masks import make_identity

# ---- problem constants (hardcoded per contest contract) ----
B, S_FULL, D, H, HD = 2, 2048, 1024, 16, 64
DFF = 2 * D
MU, ALPHA, K_COUP, DT, STEPS, MIX = 1.0, 0.1, 3.0, 0.02, 5, 0.3
N_CORES = 8
NHL = 4  # heads per core

CA, SA = math.cos(ALPHA), math.sin(ALPHA)
C1V = K_COUP * CA            # c1
C2V = K_COUP * SA            # c2
CC1 = MU - K_COUP            # -2.0
R21 = C2V / C1V              # tan(alpha)
W1S = C1V - C2V * C2V / C1V  # pass-1 roa scale
W2S = -2.0 * C2V             # pass-1 rob scale
M2 = (1.0 - MIX) * CA / K_COUP
M3 = (1.0 - MIX) * SA / K_COUP
SCL = 1.0 / math.sqrt(HD)
INVK = 1.0 / K_COUP
GC0 = math.sqrt(2.0 / math.pi)

F32 = mybir.dt.float32
F32R = mybir.dt.float32r
BF16 = mybir.dt.bfloat16
FP8 = mybir.dt.float8e4
DRPM = mybir.MatmulPerfMode.DoubleRow
EXPB = -3.7  # exp offset: e^(8.97+eps-3.7) < 240 = fp8e4 max finite (cancels via Z)
ALU = mybir.AluOpType
AF = mybir.ActivationFunctionType


def r(ap):
    """bitcast fp32 AP -> float32r for full-rate PE streaming."""
    return ap.bitcast(F32R)


def build_nc(S=S_FULL, fake_cc=False):
    """Build the 8-core SPMD program. S = sequence length (per batch).

    fake_cc=True replaces the AllToAll with a local DMA so the program is
    collective-free (for TimelineSim cost modeling only).
    """
    nc = bacc.Bacc("TRN2", target_bir_lowering=False, debug=False,
                   num_devices=N_CORES)

    def din(name, shape):
        return nc.dram_tensor(name, shape, F32, kind="ExternalInput").ap()

    TOK = S // 4
    io = dict(
        x_full=nc.dram_tensor("x_full", [S, D], BF16,
                              kind="ExternalInput").ap(),
        x_heads=nc.dram_tensor("x_heads", [S, NHL * HD], BF16,
                               kind="ExternalInput").ap(),
        x_tok=din("x_tok", [TOK, D]),
        wq_d=din("wq", [NHL * HD, HD]),
        wk_d=din("wk", [NHL * HD, HD]),
        wv_d=din("wv", [NHL * HD, HD]),
        wo_d=din("wo", [NHL * HD, HD]),
        om_d=din("omega", [NHL, HD]),
        g1_d=din("g1h", [NHL * HD]),
        be1_d=din("be1h", [NHL * HD]),
        g2_d=din("g2", [D]),
        be2_d=din("be2", [D]),
        w1_d=nc.dram_tensor("w1b", [D, DFF], BF16,
                            kind="ExternalInput").ap(),
        bf1_d=din("bf1", [DFF]),
        w2_d=nc.dram_tensor("w2b", [DFF, D], BF16,
                            kind="ExternalInput").ap(),
        bf2_d=din("bf2", [D]),
        gm_d=din("gmask", [N_CORES]),
        out_d=nc.dram_tensor("out", [TOK, D], F32, kind="ExternalOutput").ap(),
    )

    with tile.TileContext(nc) as tc:
        _body(nc, tc, io, S, fake_cc)

    nc.compile()
    return nc


def _body(nc, tc, io, S, fake_cc=False):
    NMB = S // 128          # token blocks of 128
    NG = NMB // 4           # groups of 4 blocks (512 tokens)
    TOK = S // 4            # FFN tokens per core (= B*S/8)
    TT4 = TOK // 128        # FFN token tiles
    NSL = S // 512          # 512-wide column slices of S
    HC = NHL * HD           # 256 head cols per core

    x_full, x_heads, x_tok = io["x_full"], io["x_heads"], io["x_tok"]
    wq_d, wk_d, wv_d, wo_d = io["wq_d"], io["wk_d"], io["wv_d"], io["wo_d"]
    om_d, g1_d, be1_d = io["om_d"], io["g1_d"], io["be1_d"]
    g2_d, be2_d = io["g2_d"], io["be2_d"]
    w1_d, bf1_d, w2_d, bf2_d = io["w1_d"], io["bf1_d"], io["w2_d"], io["bf2_d"]
    gm_d, out_d = io["gm_d"], io["out_d"]

    from contextlib import ExitStack
    ctx = ExitStack()
    sing = ctx.enter_context(tc.tile_pool(name="sing", bufs=1))
    dram = ctx.enter_context(tc.tile_pool(name="dram", bufs=1, space="DRAM"))

    # ---- whole-kernel constants ----
    ident = sing.tile([128, 128], F32)
    make_identity(nc, ident)
    identb = sing.tile([128, 128], BF16)
    nc.vector.tensor_copy(out=identb, in_=ident)
    epsT = sing.tile([128, 1], F32)
    nc.vector.memset(epsT, 1e-5)
    expbT = sing.tile([128, 1], F32)
    nc.vector.memset(expbT, EXPB)
    dtcc1T = sing.tile([128, 1], F32)
    nc.vector.memset(dtcc1T, DT * CC1)
    maskbc = sing.tile([128, N_CORES], F32)
    nc.sync.dma_start(out=maskbc, in_=gm_d[None, :].to_broadcast([128, N_CORES]))
    # signed rotation coefficient row: [+R21 | -R21] (for s1 = R21v * wswap)
    R21v = sing.tile([128, 128], BF16)
    nc.vector.memset(R21v[:, 0:HD], R21)
    nc.vector.memset(R21v[:, HD:128], -R21)

    xtk_all = sing.tile([128, TT4, D], F32)
    w1sb = sing.tile([128, DFF // 128, D // 128, 128], BF16)

    def issue_ffn_prefetch():
        for tt in range(TT4):
            nc.sync.dma_start(out=xtk_all[:, tt, :],
                              in_=x_tok[tt * 128:(tt + 1) * 128, :])
        for f in range(DFF // 128):
            nc.sync.dma_start(
                out=w1sb[:, f, :, :],
                in_=w1_d.rearrange("(dd p) ff -> p dd ff",
                                   p=128)[:, :, f * 128:(f + 1) * 128])
    cc_in = dram.tile([TOK // 128, N_CORES * 128, HC], BF16)
    cc_out = dram.tile([TOK // 128, N_CORES * 128, HC], BF16)

    # =================== attention super-phase ===================
    with ExitStack() as actx:
        big = actx.enter_context(tc.tile_pool(name="big", bufs=1))
        with tc.tile_pool(name="gstg", bufs=1) as gstg:
            g1f = gstg.tile([128, HC], F32)
            nc.sync.dma_start(out=g1f, in_=g1_d[None, :].to_broadcast([128, HC]))
            be1f = gstg.tile([128, HC], F32)
            nc.sync.dma_start(out=be1f,
                              in_=be1_d[None, :].to_broadcast([128, HC]))
            g1bc = big.tile([128, HC], BF16)
            nc.vector.tensor_copy(out=g1bc, in_=g1f)
            be1bc = big.tile([128, HC], BF16)
            nc.vector.tensor_copy(out=be1bc, in_=be1f)
        wq_sb = big.tile([64, NHL, HD], BF16)
        wk_sb = big.tile([64, NHL, HD], BF16)
        wv_sb = big.tile([64, NHL, HD], BF16)
        wo_bf = big.tile([128, NHL, HD], BF16)
        with tc.tile_pool(name="wstg", bufs=1) as wstg:
            wstage = wstg.tile([64, 4, NHL, HD], F32)
            for wi, wd in enumerate([wq_d, wk_d, wv_d, wo_d]):
                nc.sync.dma_start(out=wstage[:, wi, :, :],
                                  in_=wd.rearrange("(h p) e -> p h e", p=HD))
            nc.vector.tensor_copy(out=wq_sb, in_=wstage[:, 0, :, :])
            nc.vector.tensor_copy(out=wk_sb, in_=wstage[:, 1, :, :])
            nc.vector.tensor_copy(out=wv_sb, in_=wstage[:, 2, :, :])
            nc.vector.tensor_copy(out=wo_bf[0:64, :, :],
                                  in_=wstage[:, 3, :, :])
            nc.vector.tensor_copy(out=wo_bf[64:128, :, :],
                                  in_=wstage[:, 3, :, :])

        xnh = big.tile([128, NMB, HC], BF16)
        X = big.tile([128, NMB, 128], BF16)     # state [XA | XB], bf16
        X8 = big.tile([128, NMB, 128], FP8)     # fp8(rot(X)) matmul rhs
        Xp8 = big.tile([128, NMB, 128], FP8)    # fp8(rot(Xp))
        vb18 = big.tile([128, NMB, 128], FP8)   # fp8 [v | ones] pass-1 rhs
        Xp = big.tile([128, NMB, 128], BF16)    # Heun predictor
        tsum = big.tile([128, NMB, 128], BF16)  # Xp + X (for corrector)
        vb1 = big.tile([128, NMB, 128], BF16)   # [v | ones] pass-1 rhs
        attnv = big.tile([128, NMB, HD], BF16)  # A @ v
        rz2dt = big.tile([128, NMB, HD], BF16)   # DT*c1/Z (bcast to halves)
        om_st = big.tile([128, HD], F32)
        omdt = big.tile([128, 128], BF16)       # [-DT*omega | +DT*omega]
        nc.vector.memset(vb1[:, :, HD:128], 1.0)
        nc.vector.memset(vb18[:, :, HD:128], 1.0)

        def swap_ap(t, gs):
            """halves-swapped view of t[:, gs, :]: [...,[XB|XA],...]"""
            base = t[:, gs, :]
            return bass.AP(tensor=base.tensor, offset=base.offset + HD,
                           ap=[base.ap[0], base.ap[1], [-HD, 2], [1, HD]])

        def bc2_ap(t):
            """[128, g, 64] -> [128, g, 2, 64] broadcast of the half dim"""
            return bass.AP(tensor=t.tensor, offset=t.offset,
                           ap=[t.ap[0], t.ap[1], [0, 2], [1, HD]])

        def row_ap(t, g=4):
            """[128, 128] const row -> [128, g, 128] group-broadcast"""
            return bass.AP(tensor=t.tensor, offset=t.offset,
                           ap=[t.ap[0], [0, g], [1, 128]])

        # ---------------- LN1 ----------------
        with tc.tile_pool(name="ln", bufs=4) as ln, \
             tc.tile_pool(name="lns", bufs=6) as lns:
            for t in range(NMB):
                xt = ln.tile([128, D], BF16, tag="xt")
                nc.sync.dma_start(out=xt, in_=x_full[t * 128:(t + 1) * 128, :])
                xh = ln.tile([128, HC], BF16, tag="xh")
                nc.sync.dma_start(out=xh, in_=x_heads[t * 128:(t + 1) * 128, :])
                st = lns.tile([128, 2, 6], F32, tag="st")
                for sg in range(2):
                    nc.vector.bn_stats(out=st[:, sg, :],
                                       in_=xt[:, sg * 512:(sg + 1) * 512])
                mv = lns.tile([128, 2], F32, tag="mv")
                nc.vector.bn_aggr(out=mv, in_=st)
                rstd = lns.tile([128, 1], F32, tag="rstd")
                nc.scalar.activation(out=rstd, in_=mv[:, 1:2], func=AF.Sqrt,
                                     bias=epsT, scale=1.0)
                nc.vector.reciprocal(out=rstd, in_=rstd)
                nb = lns.tile([128, 1], F32, tag="nb")
                nc.vector.tensor_scalar(out=nb, in0=mv[:, 0:1], scalar1=rstd,
                                        scalar2=-1.0, op0=ALU.mult, op1=ALU.mult)
                xs = lns.tile([128, HC], F32, tag="xs")
                nc.gpsimd.tensor_scalar(out=xs, in0=xh, scalar1=rstd,
                                        scalar2=nb, op0=ALU.mult, op1=ALU.add)
                nc.vector.tensor_mul(out=xs, in0=xs, in1=g1bc)
                nc.gpsimd.tensor_add(out=xnh[:, t, :], in0=xs, in1=be1bc)

        # ---------------- per-head resonance ----------------
        with tc.tile_pool(name="xhTp", bufs=2) as xhTp, \
             tc.tile_pool(name="etp", bufs=2) as etp, \
             tc.tile_pool(name="qkp", bufs=2) as qkp, \
             tc.tile_pool(name="pmisc", bufs=2, space="PSUM") as pmisc, \
             tc.tile_pool(name="pssc", bufs=2, space="PSUM") as psscp, \
             tc.tile_pool(name="psg", bufs=2, space="PSUM") as psgp, \
             tc.tile_pool(name="scr", bufs=2) as scr, \
             tc.tile_pool(name="mts", bufs=3) as mts:

            def emit_prep(h, ET, qT, kT, xhT):
                """Per-head prep as thunks: xhT transposes, q/k proj,
                scores+exp, then v-proj (v-proj last: WAR on vb1 must
                land after the previous head's pass-1 reads)."""
                th = []

                def omth():
                    src = bass.AP(tensor=om_d.tensor,
                                  offset=om_d.offset + h * HD,
                                  ap=[[0, 128], [1, HD]])
                    nc.sync.dma_start(out=om_st, in_=src)
                    nc.vector.tensor_scalar_mul(out=omdt[:, 0:HD], in0=om_st,
                                                scalar1=-DT)
                    nc.vector.tensor_scalar_mul(out=omdt[:, HD:128], in0=om_st,
                                                scalar1=DT)
                th.append(omth)

                def xhTth(t):
                    pt = pmisc.tile([64, 128], BF16, tag="pm")
                    nc.tensor.transpose(pt, xnh[:, t, h * HD:(h + 1) * HD],
                                        identb)
                    nc.scalar.copy(out=xhT[:, t * 128:(t + 1) * 128], in_=pt)
                for t in range(NMB):
                    th.append(lambda t=t: xhTth(t))

                def projth(sl):
                    pq = pmisc.tile([64, 512], F32, tag="pm")
                    nc.tensor.matmul(pq, wq_sb[:, h, :],
                                     xhT[:, sl * 512:(sl + 1) * 512],
                                     start=True, stop=True)
                    nc.scalar.copy(out=qT[:, sl * 512:(sl + 1) * 512], in_=pq)
                    pk = pmisc.tile([64, 512], F32, tag="pm")
                    nc.tensor.matmul(pk, wk_sb[:, h, :],
                                     xhT[:, sl * 512:(sl + 1) * 512],
                                     start=True, stop=True)
                    nc.scalar.copy(out=kT[:, sl * 512:(sl + 1) * 512], in_=pk)
                for sl in range(NSL):
                    th.append(lambda sl=sl: projth(sl))

                def scoreth(k, sl2):
                    c0 = sl2 * 1024
                    ps = psscp.tile([128, 1024], F32, tag="ps")
                    for j in range(2):
                        nc.tensor.matmul(ps[:, j * 512:(j + 1) * 512],
                                         kT[:, k * 128:(k + 1) * 128],
                                         qT[:, c0 + j * 512:c0 + (j + 1) * 512],
                                         start=True, stop=True)
                    nc.scalar.activation(out=ET[:, k, c0:c0 + 1024],
                                         in_=ps, func=AF.Exp, scale=SCL,
                                         bias=expbT)
                for sl2 in range(NSL // 2):
                    for k in range(NMB):
                        th.append(lambda k=k, sl2=sl2: scoreth(k, sl2))

                def vth(t):
                    pv = pmisc.tile([128, HD], F32, tag="pm")
                    nc.tensor.matmul(pv, xhT[:, t * 128:(t + 1) * 128],
                                     wv_sb[:, h, :], start=True, stop=True)
                    nc.scalar.copy(out=vb1[:, t, 0:HD], in_=pv)
                    nc.scalar.copy(out=vb18[:, t, 0:HD], in_=pv)
                for t in range(NMB):
                    th.append(lambda t=t: vth(t))
                return th

            def hbufs(h):
                return (etp.tile([128, NMB, S], FP8, tag="ET",
                                 name=f"ET_{h}"),
                        qkp.tile([64, S], BF16, tag="qT", name=f"qT_{h}"),
                        qkp.tile([64, S], BF16, tag="kT", name=f"kT_{h}"),
                        xhTp.tile([64, S], BF16, tag="xhT", name=f"xhT_{h}"))

            hbuf = {0: hbufs(0)}
            for f in emit_prep(0, *hbuf[0]):
                f()
            for h in range(NHL):
                ET, qT, kT, xhT = hbuf[h]
                if h + 1 < NHL:
                    hbuf[h + 1] = hbufs(h + 1)
                    nextq = emit_prep(h + 1, *hbuf[h + 1])
                else:
                    nextq = []
                if h == 0:
                    issue_ffn_prefetch()
                # v-proj thunks (the last 16) must land after pass-1 of head
                # h finishes reading vb1; draining starts at pass 2 so the
                # in-order PE queue never stalls on the WAR.
                nslots = (2 * STEPS - 1) * NG
                per_slot = max(1, -(-len(nextq) // nslots))

                # --- 10 Heun passes: fp8 DoubleRow matmuls, bf16 chain ---
                for p in range(1, 2 * STEPS + 1):
                    odd = (p % 2 == 1)
                    rhs8 = vb18 if p == 1 else (X8 if odd else Xp8)
                    xin = X if odd else Xp

                    for g in range(NG):
                        pg = psgp.tile([128, 4, 128], F32, tag="pg")
                        for ml in range(4):
                            mb = g * 4 + ml
                            for kp in range(NMB // 2):
                                nc.tensor.matmul(
                                    pg[:, ml, :],
                                    ET[:, 2 * kp:2 * kp + 2,
                                       mb * 128:(mb + 1) * 128],
                                    rhs8[:, 2 * kp:2 * kp + 2, :],
                                    start=(kp == 0), stop=(kp == NMB // 2 - 1),
                                    perf_mode=DRPM)
                        gs = slice(g * 4, g * 4 + 4)
                        ro = scr.tile([128, 4, 128], BF16, tag="ro")
                        if p == 1:
                            # psum = [E8@v | Z8rep]; rz2dt = DT*c1/Z8
                            w = scr.tile([128, 4, 128], BF16, tag="w")
                            nc.scalar.copy(out=w, in_=pg)
                            rcp = scr.tile([128, 4], F32, tag="rcp")
                            nc.vector.reciprocal(out=rcp, in_=pg[:, :, HD:HD + 1])
                            rcpb = scr.tile([128, 4], BF16, tag="rcpb")
                            nc.vector.tensor_copy(out=rcpb, in_=rcp)
                            rb64b = bass.AP(tensor=rcpb.tensor, offset=rcpb.offset,
                                            ap=[rcpb.ap[0], [1, 4], [0, HD]])
                            nc.vector.tensor_scalar_mul(out=rz2dt[:, gs, :],
                                                        in0=rb64b,
                                                        scalar1=DT * C1V)
                            rb64 = bass.AP(tensor=rcpb.tensor, offset=rcpb.offset,
                                           ap=[rcpb.ap[0], [1, 4], [0, HD]])
                            nc.vector.tensor_mul(out=attnv[:, gs, :],
                                                 in0=w[:, :, 0:HD], in1=rb64)
                            # state init X0 = [c1*v | -c2*v]
                            nc.vector.tensor_scalar_mul(out=X[:, gs, 0:HD],
                                                        in0=vb1[:, gs, 0:HD],
                                                        scalar1=C1V)
                            nc.vector.tensor_scalar_mul(out=X[:, gs, HD:128],
                                                        in0=vb1[:, gs, 0:HD],
                                                        scalar1=-C2V)
                            # rotated coupling via W1S/W2S, then * rz
                            nc.vector.tensor_scalar_mul(out=w[:, :, HD:128],
                                                        in0=w[:, :, 0:HD],
                                                        scalar1=W2S)
                            nc.vector.tensor_scalar_mul(out=w[:, :, 0:HD],
                                                        in0=w[:, :, 0:HD],
                                                        scalar1=W1S)
                            rzb = rz2dt[:, gs, :]
                            rzb = bass.AP(tensor=rzb.tensor, offset=rzb.offset,
                                          ap=[rzb.ap[0], rzb.ap[1], [0, 2],
                                              [1, HD]])
                            nc.vector.tensor_mul(out=ro, in0=w, in1=rzb)
                        else:
                            # rhs was pre-rotated: psum IS the rotated coupling
                            rzb = rz2dt[:, gs, :]
                            rzb = bass.AP(tensor=rzb.tensor, offset=rzb.offset,
                                          ap=[rzb.ap[0], rzb.ap[1], [0, 2],
                                              [1, HD]])
                            if g == NG - 1:
                                # barrier group: single fused PSUM read on DVE
                                nc.vector.tensor_mul(out=ro, in0=pg, in1=rzb)
                            else:
                                w = scr.tile([128, 4, 128], BF16, tag="w")
                                nc.scalar.copy(out=w, in_=pg)
                                nc.vector.tensor_mul(out=ro, in0=w, in1=rzb)
                        # elementwise drift, DT-scaled: dd = DT*f_local + ro
                        sq = scr.tile([128, 4, 128], BF16, tag="sq")
                        if h >= 2:
                            nc.scalar.activation(out=sq, in_=xin[:, gs, :],
                                                 func=AF.Square, scale=1.0)
                        else:
                            nc.gpsimd.tensor_mul(out=sq, in0=xin[:, gs, :],
                                                 in1=xin[:, gs, :])
                        r2h = scr.tile([128, 4, HD], BF16, tag="r2h")
                        nc.vector.tensor_add(out=r2h, in0=sq[:, :, 0:HD],
                                             in1=sq[:, :, HD:128])
                        mtl = scr.tile([128, 4, HD], BF16, tag="mtl")
                        nc.vector.tensor_scalar(out=mtl, in0=r2h,
                                                scalar1=-DT * INVK * INVK,
                                                scalar2=DT * CC1,
                                                op0=ALU.mult, op1=ALU.add)
                        u = scr.tile([128, 4, 128], BF16, tag="u")
                        nc.vector.tensor_mul(out=u, in0=bc2_ap(mtl),
                                             in1=xin[:, gs, :])
                        cross = scr.tile([128, 4, 128], BF16, tag="cross")
                        nc.gpsimd.tensor_mul(out=cross, in0=row_ap(omdt),
                                             in1=swap_ap(xin, gs))
                        nc.vector.tensor_add(out=u, in0=u, in1=cross)
                        dd = u
                        nc.vector.tensor_add(out=dd, in0=u, in1=ro)
                        ry = scr.tile([128, 4, 128], BF16, tag="ry",
                                      name=f"ry_{h}_{p}_{g}")
                        if odd:
                            nc.vector.tensor_add(out=Xp[:, gs, :],
                                                 in0=X[:, gs, :], in1=dd)
                            nc.gpsimd.tensor_add(out=tsum[:, gs, :],
                                                 in0=Xp[:, gs, :], in1=X[:, gs, :])
                            if p < 2 * STEPS:
                                # Xp8 = fp8(rot(Xp))
                                nc.vector.tensor_mul(out=ry, in0=row_ap(R21v),
                                                     in1=swap_ap(Xp, gs))
                                nc.vector.tensor_add(out=ry, in0=ry,
                                                     in1=Xp[:, gs, :])
                                if g == NG - 1:
                                    nc.vector.tensor_copy(out=Xp8[:, gs, :],
                                                          in_=ry)
                                else:
                                    nc.scalar.copy(out=Xp8[:, gs, :], in_=ry)
                        else:
                            # X' = 0.5*(Xp + X + dd2)
                            nc.vector.tensor_add(out=dd, in0=tsum[:, gs, :],
                                                 in1=dd)
                            nc.vector.tensor_scalar_mul(out=X[:, gs, :], in0=dd,
                                                        scalar1=0.5)
                            if p < 2 * STEPS:
                                # X8 = fp8(rot(X))
                                nc.gpsimd.tensor_mul(out=ry, in0=row_ap(R21v),
                                                     in1=swap_ap(X, gs))
                                nc.vector.tensor_add(out=ry, in0=ry,
                                                     in1=X[:, gs, :])
                                if g == NG - 1:
                                    nc.vector.tensor_copy(out=X8[:, gs, :],
                                                          in_=ry)
                                else:
                                    nc.scalar.copy(out=X8[:, gs, :], in_=ry)
                        if p >= 2 and g < NG - 1:
                            for _ in range(min(per_slot, len(nextq))):
                                nextq.pop(0)()
                for f in nextq:
                    f()
                nextq = []

                # --- readout: mixed -> @Wo -> xattn cols ---
                for gq in range(0, NMB, 4):
                    gqs = slice(gq, gq + 4)
                    nc.vector.tensor_scalar_mul(out=attnv[:, gqs, :],
                                                in0=attnv[:, gqs, :],
                                                scalar1=MIX)
                    nc.vector.scalar_tensor_tensor(out=attnv[:, gqs, :],
                                                   in0=X[:, gqs, 0:HD],
                                                   scalar=M2,
                                                   in1=attnv[:, gqs, :],
                                                   op0=ALU.mult, op1=ALU.add)
                    nc.vector.scalar_tensor_tensor(out=attnv[:, gqs, :],
                                                   in0=X[:, gqs, HD:128],
                                                   scalar=-M3,
                                                   in1=attnv[:, gqs, :],
                                                   op0=ALU.mult, op1=ALU.add)
                mixv = attnv
                xatth = mts.tile([128, NMB, HD], BF16, tag="xatth",
                                 name=f"xatth_{h}")
                for t2 in range(NMB // 2):
                    pt = pmisc.tile([128, 128], BF16, tag="pm")
                    nc.tensor.transpose(pt, mixv[:, 2 * t2:2 * t2 + 2, :],
                                        identb)
                    mt = mts.tile([128, 128], BF16, tag="mt")
                    nc.scalar.copy(out=mt, in_=pt)
                    for j in range(2):
                        po = pmisc.tile([128, HD], F32, tag="pm")
                        nc.tensor.matmul(po, mt[j * 64:(j + 1) * 64, :],
                                         wo_bf[j * 64:(j + 1) * 64, h, :],
                                         start=True, stop=True)
                        nc.scalar.copy(out=xatth[:, 2 * t2 + j, :], in_=po)

                # --- stage this head's slice of cc_in (masked) ---
                for j in range(N_CORES):
                    t0 = (j % 4) * TT4
                    stg = mts.tile([128, TT4, HD], BF16, tag="stg",
                                   name=f"stg_{h}_{j}")
                    nc.vector.tensor_scalar_mul(
                        out=stg,
                        in0=xatth[:, t0:t0 + TT4, :],
                        scalar1=maskbc[:, j:j + 1])
                    base = cc_in[0, j * 128:(j + 1) * 128, :]
                    dst = bass.AP(tensor=base.tensor,
                                  offset=base.offset + h * HD,
                                  ap=[[HC, 128], [N_CORES * 128 * HC, TT4],
                                      [1, HD]])
                    nc.sync.dma_start(out=dst, in_=stg)

    # =================== AllToAll (per-tt) + FFN ===================
    with tc.tile_pool(name="ffw", bufs=1) as ffw, \
         tc.tile_pool(name="ffa", bufs=4) as ffa, \
         tc.tile_pool(name="ffs", bufs=4) as ffs, \
         tc.tile_pool(name="w1p", bufs=4) as w1p, \
         tc.tile_pool(name="w2p", bufs=8) as w2p, \
         tc.tile_pool(name="psf", bufs=2, space="PSUM") as psfp, \
         tc.tile_pool(name="pso", bufs=1, space="PSUM") as psop, \
         tc.tile_pool(name="pstf", bufs=2, space="PSUM") as pstf:

        for tt in range(TT4):
            if fake_cc:
                nc.sync.dma_start(out=cc_out[tt, :, :], in_=cc_in[tt, :, :])
            else:
                nc.gpsimd.collective_compute(
                    "AllToAll", ALU.bypass,
                    replica_groups=[list(range(N_CORES))],
                    ins=[cc_in[tt, :, :].opt()],
                    outs=[cc_out[tt, :, :].opt()])

        g2bc = ffw.tile([128, D], F32)
        nc.sync.dma_start(out=g2bc, in_=g2_d[None, :].to_broadcast([128, D]))
        be2bc = ffw.tile([128, D], F32)
        nc.sync.dma_start(out=be2bc, in_=be2_d[None, :].to_broadcast([128, D]))
        bf2bc = ffw.tile([128, D], F32)
        nc.sync.dma_start(out=bf2bc, in_=bf2_d[None, :].to_broadcast([128, D]))
        bf1sb = ffw.tile([128, DFF // 128], F32)
        nc.sync.dma_start(out=bf1sb, in_=bf1_d.rearrange("(f p) -> p f", p=128))
        bf1h = ffw.tile([128, DFF // 128], F32)
        nc.scalar.activation(out=bf1h, in_=bf1sb, func=AF.Copy, scale=0.5)
        x1_all = ffw.tile([128, TT4, D], F32)
        xn1T = ffw.tile([128, D // 128, TOK], BF16)
        hT = ffw.tile([128, DFF // 128, TOK], BF16)

        cc_a = ffw.tile([128, TT4, D], BF16)
        cc_b = ffw.tile([128, TT4, D], BF16)
        for tt in range(TT4):
            for half, dstt in ((0, cc_a), (1, cc_b)):
                srcb = cc_out[tt, half * 4 * 128:(half * 4 + 4) * 128, :]
                srca = bass.AP(tensor=srcb.tensor, offset=srcb.offset,
                               ap=[[HC, 128], [128 * HC, 4], [1, HC]])
                nc.sync.dma_start(out=dstt[:, tt, :], in_=srca)
        for tt in range(TT4):
            xa = ffa.tile([128, D], BF16, tag="xa")
            nc.vector.tensor_add(out=xa, in0=cc_a[:, tt, :], in1=cc_b[:, tt, :])
            nc.gpsimd.tensor_add(out=x1_all[:, tt, :], in0=xtk_all[:, tt, :],
                                 in1=xa)
            # LN2
            st = ffs.tile([128, 2, 6], F32, tag="st")
            for sg in range(2):
                nc.vector.bn_stats(out=st[:, sg, :],
                                   in_=x1_all[:, tt, sg * 512:(sg + 1) * 512])
            mv = ffs.tile([128, 2], F32, tag="mv")
            nc.vector.bn_aggr(out=mv, in_=st)
            rstd = ffs.tile([128, 1], F32, tag="rstd")
            nc.scalar.activation(out=rstd, in_=mv[:, 1:2], func=AF.Sqrt,
                                 bias=epsT, scale=1.0)
            nc.vector.reciprocal(out=rstd, in_=rstd)
            xn1 = ffa.tile([128, D], F32, tag="xn1")
            nc.vector.tensor_scalar(out=xn1, in0=x1_all[:, tt, :],
                                    scalar1=mv[:, 0:1], scalar2=rstd,
                                    op0=ALU.subtract, op1=ALU.mult)
            nc.vector.tensor_mul(out=xn1, in0=xn1, in1=g2bc)
            nc.gpsimd.tensor_add(out=xn1, in0=xn1, in1=be2bc)
            for dd in range(D // 128):
                pt = pstf.tile([128, 128], F32, tag="pt")
                nc.tensor.transpose(pt, xn1[:, dd * 128:(dd + 1) * 128], ident)
                nc.scalar.copy(out=xn1T[:, dd, tt * 128:(tt + 1) * 128], in_=pt)

        # h^T = gelu(W1^T @ xn1^T + bf1)
        for f in range(DFF // 128):
            ph = psfp.tile([128, TOK], F32, tag="ph")
            for dd in range(D // 128):
                nc.tensor.matmul(ph, w1sb[:, f, dd, :], xn1T[:, dd, :],
                                 start=(dd == 0), stop=(dd == D // 128 - 1))
            # gelu (tanh approx), computed on y = x/2:
            #   gelu(x) = y*(1+tanh(y*(2*c0 + 8*c3*y^2))), c0=sqrt(2/pi), c3=0.044715*c0
            gy = ffa.tile([128, TOK], F32, tag="gy")
            nc.scalar.activation(out=gy, in_=ph, func=AF.Identity, scale=0.5,
                                 bias=bf1h[:, f:f + 1])
            gt = ffa.tile([128, TOK], F32, tag="gt")
            nc.scalar.activation(out=gt, in_=gy, func=AF.Square, scale=1.0)
            nc.vector.tensor_scalar(out=gt, in0=gt, scalar1=8 * 0.044715 * GC0,
                                    scalar2=2 * GC0, op0=ALU.mult, op1=ALU.add)
            nc.vector.tensor_mul(out=gt, in0=gt, in1=gy)
            nc.scalar.activation(out=gt, in_=gt, func=AF.Tanh, scale=1.0)
            nc.vector.scalar_tensor_tensor(out=hT[:, f, :], in0=gt, scalar=1.0,
                                           in1=gy, op0=ALU.add, op1=ALU.mult)

        # out = x1 + h @ W2 + bf2   (W2 streamed, bf16)
        for dh in range(D // 512):
            pos = [psop.tile([128, 512], F32, tag=f"po{tt}", name=f"po{tt}") for tt in range(TT4)]
            for f in range(DFF // 128):
                w2b = w2p.tile([128, 512], BF16, tag="w2b")
                nc.sync.dma_start(out=w2b,
                                  in_=w2_d[f * 128:(f + 1) * 128,
                                           dh * 512:(dh + 1) * 512])
                for tt in range(TT4):
                    nc.tensor.matmul(pos[tt], hT[:, f, tt * 128:(tt + 1) * 128],
                                     w2b, start=(f == 0),
                                     stop=(f == DFF // 128 - 1))
            for tt in range(TT4):
                o1 = ffa.tile([128, 512], F32, tag="o1")
                nc.vector.tensor_add(out=o1, in0=pos[tt],
                                     in1=x1_all[:, tt, dh * 512:(dh + 1) * 512])
                nc.vector.tensor_add(out=o1, in0=o1,
                                     in1=bf2bc[:, dh * 512:(dh + 1) * 512])
                nc.sync.dma_start(out=out_d[tt * 128:(tt + 1) * 128,
                                            dh * 512:(dh + 1) * 512], in_=o1)

    ctx.close()


# ======================= host-side driver =======================

def shard_inputs(inputs, S=S_FULL):
    """Build per-core in_maps from full inputs."""
    import ml_dtypes
    x = np.ascontiguousarray(inputs["x"], dtype=np.float32)
    w1b = np.ascontiguousarray(
        np.asarray(inputs["W1"], np.float32).astype(ml_dtypes.bfloat16))
    w2b = np.ascontiguousarray(
        np.asarray(inputs["W2"], np.float32).astype(ml_dtypes.bfloat16))
    TOK = S // 4
    in_maps = []
    for c in range(N_CORES):
        b = c // 4
        hg = c % 4
        hsl = slice(hg * NHL, (hg + 1) * NHL)            # global head indices
        csl = slice(hg * NHL * HD, (hg + 1) * NHL * HD)  # head cols in D
        rsl = slice(hg * TOK, (hg + 1) * TOK)            # FFN token rows
        xb16 = x[b].astype(ml_dtypes.bfloat16)
        m = {
            "x_full": xb16,
            "x_heads": np.ascontiguousarray(xb16[:, csl]),
            "x_tok": x[b][rsl, :],
            "wq": inputs["Wq"][hsl].reshape(NHL * HD, HD),
            "wk": inputs["Wk"][hsl].reshape(NHL * HD, HD),
            "wv": inputs["Wv"][hsl].reshape(NHL * HD, HD),
            "wo": inputs["Wo"][hsl].reshape(NHL * HD, HD),
            "omega": inputs["omega"][hsl],
            "g1h": inputs["g1"][csl],
            "be1h": inputs["be1"][csl],
            "g2": inputs["g2"], "be2": inputs["be2"],
            "w1b": w1b, "bf1": inputs["bf1"],
            "w2b": w2b, "bf2": inputs["bf2"],
            "gmask": np.array([1.0 if j // 4 == b else 0.0
                               for j in range(N_CORES)], dtype=np.float32),
        }
        in_maps.append({k: (v if k in ("w1b", "w2b", "x_full", "x_heads")
                            else np.ascontiguousarray(v, dtype=np.float32))
                        for k, v in m.items()})
    return in_maps


def assemble_output(results, S=S_FULL):
    TOK = S // 4
    out = np.zeros((B, S, D), dtype=np.float32)
    for c in range(N_CORES):
        b, hg = c // 4, c % 4
        out[b, hg * TOK:(hg + 1) * TOK, :] = results[c]["out"]
    return out


_NC_CACHE = {}


def kernel(**inputs):
    from concourse.bass_utils import run_bass_kernel_spmd
    S = inputs["x"].shape[1]
    if S not in _NC_CACHE:
        _NC_CACHE[S] = build_nc(S)
    nc = _NC_CACHE[S]
    in_maps = shard_inputs(inputs, S)
    res = run_bass_kernel_spmd(nc, in_maps, core_ids=list(range(N_CORES)))
    return assemble_output(res.results, S)

